# revision 21
# baseline (speedup 1.0000x reference)
"""GATv2 x3 + MLP (nn_GAT) on trn2, 8 NeuronCores.

v5 design: attn folded into projection weights (leaky_relu is positively
homogeneous; negative attn handled by a slope-5 prelu + per-node unscale),
l-innermost edge stream so every DVE op runs in 2x mode, den-reduce on the
Pool engine, MLP tail batched (stacked 14->1 matmul + one sigmoid pass).

 - Launch A: project x -> Y|fd''|res per node (block-diag matmuls);
   Y = attn-scaled source projection, fd'' = attn-scaled dest projection.
 - Host gathers Y[src] per edge into a node-major padded-ELL stream with
   edge slots INNERMOST: [P, S, C, L].
 - Launch B: GAT layer-1: z = Y[src]+fd''[dst] (pair-trick broadcast keeps
   2x), prelu with per-sign-run alphas, score tree (4 wide strided adds),
   exp, weighted feats in-place, halving reduce over slots; den reduce on
   gpsimd.  Fixup: pad-slot den fix, normalize, unscale, residual, elu.
   Epilogue projects fs2''|fd2''|res2 on the otherwise idle PE.
 - Launch C: d2 GAT pipeline + 14->196->196->14->1 MLP.  W2 fp8 DoubleRow;
   r3 staged so the 14->1 matmul runs 8 chunks per instruction and all
   sigmoids run in one table-load at the end.

Host only reorders/replicates/casts device-computed tensor bytes; the only
host arithmetic is on the tiny weight matrices (attn folding).
"""
import sys
sys.path.insert(0, '/opt/trn_rl_repo')
import numpy as np
import ml_dtypes

import concourse.bass as bass
import concourse.mybir as mybir
from concourse import bacc
from concourse.tile import TileContext
from concourse.bass_utils import run_bass_kernel_spmd
from concourse.masks import make_identity

bf16 = mybir.dt.bfloat16
fp8 = mybir.dt.float8e4
f32 = mybir.dt.float32
BF = ml_dtypes.bfloat16
FP8 = ml_dtypes.float8_e4m3
AL = mybir.AluOpType
AF = mybir.ActivationFunctionType
MPM = mybir.MatmulPerfMode

NCORE = 8
P = 128
SUPER = 16          # tiles per supertile
NEG_GAT = 0.2
NEG_MLP = 0.01
FP8_W2 = True


# ================================================================= host prep
def build_schedule(dst, n):
    nloc = n // NCORE
    core_of = dst // nloc
    scheds = []
    for c in range(NCORE):
        em = np.where(core_of == c)[0]
        ldst = dst[em] - c * nloc
        deg = np.bincount(ldst, minlength=nloc)
        nt = -(-nloc // P)
        nt = -(-nt // SUPER) * SUPER
        degp = np.concatenate([deg, np.zeros(nt * P - nloc, np.int64)])
        order = np.argsort(-degp, kind='stable')
        pos_of = np.empty_like(order)
        pos_of[order] = np.arange(len(order))
        scheds.append(dict(core=c, em=em, ldst=ldst, deg=degp, order=order,
                           pos_of=pos_of, nt=nt, nloc=nloc))
    nt = scheds[0]['nt']
    nst = nt // SUPER
    Ls = []
    for st in range(nst):
        L = 2
        for s in scheds:
            L = max(L, int(s['deg'][s['order'][st * SUPER * P]]))
        L = -(-L // 8) * 8   # mult-8: keeps halving-tree levels in DVE 2x mode
        Ls.append(L)
    return scheds, nst, Ls


def edge_slot_geom(s, Ls):
    """Per edge (in eo order): supertile, tile-in-supertile, slot rank, row."""
    order, deg = s['order'], s['deg']
    pos_e = s['pos_of'][s['ldst']]
    eo = np.lexsort((np.arange(len(pos_e)), pos_e))
    pos_sorted = pos_e[eo]
    starts = np.concatenate([[0], np.cumsum(deg[order])])
    rank = np.arange(len(eo)) - starts[pos_sorted]
    t_of = pos_sorted // P
    st_of = t_of // SUPER
    p_of = pos_sorted % P
    return (eo, st_of.astype(np.int64), (t_of % SUPER).astype(np.int64),
            rank.astype(np.int64), p_of.astype(np.int64))


def pack_G(vals_bf, st_of, s_of, rank, p_of, offsC, C, Ls, totc):
    """l-innermost: col = offs[st] + s*(C*L) + c*L + rank."""
    buf = np.zeros((P, totc), BF)
    L_e = np.asarray(Ls)[st_of]
    base = np.asarray(offsC)[st_of] + s_of * (C * L_e) + rank
    for c in range(C):
        buf[p_of, base + c * L_e] = vals_bf[:, c]
    return buf


def make_npad(s, Ls, nt):
    L_t = np.repeat(np.asarray(Ls, np.int64), SUPER)
    d = s['deg'][s['order']].reshape(nt, P)
    return np.ascontiguousarray((L_t[:, None] - d).T).astype(np.float32)


def pack_local(vals, nrow, nt):
    pk = np.zeros((8 * nrow, (nt // 8) * P), BF)
    nodes = np.arange(nt * P)
    a = (nodes // P) % 8
    col = (nodes // (8 * P)) * P + nodes % P
    v = vals.astype(BF)
    for f in range(nrow - 1):
        pk[a * nrow + f, col] = v[:, f]
    pk[a * nrow + (nrow - 1), col] = BF(1.0)
    return pk


def blockdiag(w, bias, nrow, sp=16):
    bd = np.zeros((8 * nrow, 8 * sp), np.float32)
    k = w.shape[1]
    for a in range(8):
        bd[a * nrow:a * nrow + w.shape[0], a * sp:a * sp + k] = w
        bd[a * nrow + nrow - 1, a * sp:a * sp + k] = bias
    return bd.astype(BF)


def pm(vals, nt):
    d = vals.shape[1]
    return np.ascontiguousarray(
        vals.reshape(nt, P, d).transpose(1, 0, 2).reshape(P, nt * d))


def pm_pair(vals, nt):
    """[nt*P, d] -> [P, nt*d*2] with each channel duplicated (pair trick)."""
    d = vals.shape[1]
    v = vals.reshape(nt, P, d).transpose(1, 0, 2)        # [P, nt, d]
    v2 = np.repeat(v, 2, axis=2)                          # [P, nt, 2d]
    return np.ascontiguousarray(v2.reshape(P, nt * d * 2))


def attn_fold(attn_hf, H, F):
    """Per (h,f): permuted order (pos-signs first within each head),
    channel scale, prelu alpha.  Returns (perm j-list, scale, alpha)."""
    perm, scale, alpha = [], [], []
    for h in range(H):
        # alternate pos-first / neg-first per head so prelu alpha-runs merge
        # across head boundaries (fewer Act instructions)
        first_pos = (h % 2 == 0)
        fs = sorted(range(F),
                    key=lambda f: 0 if (attn_hf[h, f] > 0) == first_pos else 1)
        for f in fs:
            a = float(attn_hf[h, f])
            if a > 0:
                aa = max(a, 1e-8)
                perm.append(h * F + f); scale.append(aa); alpha.append(NEG_GAT)
            else:
                aa = min(a, -1e-8)
                perm.append(h * F + f); scale.append(NEG_GAT * aa); alpha.append(1.0 / NEG_GAT)
    return perm, np.asarray(scale, np.float64), alpha


def alpha_runs(alphas):
    runs = []
    i = 0
    while i < len(alphas):
        j = i
        while j < len(alphas) and alphas[j] == alphas[i]:
            j += 1
        runs.append((i, j, float(alphas[i])))
        i = j
    return runs


# ================================================================ device bits
def halving_tree(tt, X, L, out_final):
    """In-place halving over innermost axis of X [P,...,L]; final add -> out_final."""
    cur = L
    while cur > 2:
        h = cur // 2
        tt(out=X[..., 0:h], in0=X[..., 0:h], in1=X[..., cur - h:cur], op=AL.add)
        cur -= h
    if cur == 2:
        tt(out=out_final, in0=X[..., 0:1], in1=X[..., 1:2], op=AL.add)
    else:
        tt(out=out_final, in0=X[..., 0:1], in1=X[..., 0:1], op=AL.bypass)


def pp_tree(nc, A, B, L, out_final, cp=None):
    """Ping-pong halving reduce over innermost axis: A [P,..,L] (input, even L),
    B [P,..,>=L/2] scratch.  Output buffer alternates so out never shares a
    buffer with an input (keeps the DVE 2x mode).  cp: engine copy fn for the
    odd middle element (defaults to vector tensor_copy)."""
    cp = cp or nc.vector.tensor_copy
    tt = nc.vector.tensor_tensor
    src, dst = A, B
    cur = L
    while cur > 2:
        h = cur // 2
        tt(out=dst[..., 0:h], in0=src[..., 0:h], in1=src[..., h:2 * h], op=AL.add)
        if cur & 1:
            cp(out=dst[..., h:h + 1], in_=src[..., 2 * h:2 * h + 1])
            cur = h + 1
        else:
            cur = h
        src, dst = dst, src
    if cur == 2:
        tt(out=out_final, in0=src[..., 0:1], in1=src[..., 1:2], op=AL.add)
    else:
        cp(out=out_final, in_=src[..., 0:1])


def emit_elu(nc, sbS, hflat, nelem, tag):
    tmp = sbS.tile([P, nelem], f32, tag=tag)
    nc.vector.tensor_scalar_min(out=tmp[:], in0=hflat, scalar1=0.0)
    nc.scalar.activation(out=tmp[:], in_=tmp[:], func=AF.Exp)
    nc.vector.tensor_scalar(out=hflat, in0=hflat, scalar1=0.0, scalar2=-1.0,
                            op0=AL.max, op1=AL.add)
    nc.vector.tensor_tensor(out=hflat, in0=hflat, in1=tmp[:], op=AL.add)


def emit_gat_st_B(nc, sbG, sbZ, sbEX, sbT, d_g, off, L, S, fdp_ap, runs,
                  h1v, denv):
    """Layer-1 supertile: C=12 channels [d1h0(5), d1h1(5), a1h0, a1h1]."""
    C = 12
    L2 = L // 2
    ncols = S * C * L
    G = sbG.tile([P, ncols], bf16, tag="G")
    nc.sync.dma_start(out=G[:], in_=d_g[:, off:off + ncols])
    G4 = G[:].rearrange("p (s c l) -> p s c l", s=S, c=C, l=L)
    G5 = G[:].rearrange("p (sc l2 j) -> p sc l2 j", sc=S * C, l2=L2, j=2)
    Z = sbZ.tile([P, ncols], bf16, tag="Z")
    Z4 = Z[:].rearrange("p (s c l) -> p s c l", s=S, c=C, l=L)
    Z5 = Z[:].rearrange("p (sc l2 j) -> p sc l2 j", sc=S * C, l2=L2, j=2)
    fdb = fdp_ap.rearrange("p s c j -> p (s c) j").unsqueeze(2) \
        .broadcast_to([P, S * C, L2, 2])
    nc.vector.tensor_tensor(out=Z5, in0=G5, in1=fdb, op=AL.add)
    for (c0, c1, al) in runs:
        nc.scalar.activation(out=Z4[:, :, c0:c1, :], in_=Z4[:, :, c0:c1, :],
                             func=AF.Prelu, alpha=al)
    # score tree: d1 heads = sum of 5 channels each
    T = sbT.tile([P, S * 4 * L], bf16, tag="T")
    T4 = T[:].rearrange("p (s c l) -> p s c l", s=S, c=4, l=L)
    Tp = T[:].rearrange("p (s c2 c l) -> p s c2 c l", s=S, c2=2, c=2, l=L)
    nc.vector.tensor_tensor(out=T4[:, :, 0:2, :], in0=Z4[:, :, 0:2, :],
                            in1=Z4[:, :, 2:4, :], op=AL.add)
    nc.vector.tensor_tensor(out=T4[:, :, 2:4, :], in0=Z4[:, :, 5:7, :],
                            in1=Z4[:, :, 7:9, :], op=AL.add)
    SC = sbT.tile([P, S * 2 * L], bf16, tag="SC")
    SC3 = SC[:].rearrange("p (s c l) -> p s c l", s=S, c=2, l=L)
    nc.vector.tensor_tensor(out=SC3, in0=Tp[:, :, :, 0, :],
                            in1=Tp[:, :, :, 1, :], op=AL.add)
    Zh = Z4[:, :, 0:10, :].rearrange("p s (h f) l -> p s h f l", h=2, f=5)
    nc.vector.tensor_tensor(out=SC3, in0=SC3, in1=Zh[:, :, :, 4, :], op=AL.add)
    EX = sbEX.tile([P, S * 4 * L], bf16, tag="EX")
    EX4 = EX[:].rearrange("p (s c l) -> p s c l", s=S, c=4, l=L)
    nc.scalar.activation(out=EX4[:, :, 0:2, :], in_=SC3, func=AF.Exp)
    nc.scalar.activation(out=EX4[:, :, 2:4, :], in_=Z4[:, :, 10:12, :], func=AF.Exp)
    # weighted feats in-place on G (one op per d1 head: <=3 free dims)
    for h in range(2):
        Gh = G4[:, :, 5 * h:5 * h + 5, :]
        exd = EX4[:, :, h:h + 1, :].broadcast_to([P, S, 5, L])
        nc.vector.tensor_tensor(out=Gh, in0=Gh, in1=exd, op=AL.mult)
    nc.vector.tensor_tensor(out=G4[:, :, 10:12, :], in0=G4[:, :, 10:12, :],
                            in1=EX4[:, :, 2:4, :], op=AL.mult)
    # ping-pong partner = Z (dead after score tree / exp)
    pp_tree(nc, G4, Z4[:, :, :, 0:L // 2 + 1], L, h1v.unsqueeze(3))
    halving_tree(nc.gpsimd.tensor_tensor, EX4, L, denv.unsqueeze(3))


def emit_gat_st_C(nc, sbG, sbZ, sbEX, sbT, d_g, off, L, S, fdp_ap, runs,
                  h2v, denv):
    """Layer-2 supertile: C=4 channels [d2h0(2), d2h1(2)]."""
    C = 4
    L2 = L // 2
    ncols = S * C * L
    G = sbG.tile([P, ncols], bf16, tag="G")
    nc.sync.dma_start(out=G[:], in_=d_g[:, off:off + ncols])
    G4 = G[:].rearrange("p (s c l) -> p s c l", s=S, c=C, l=L)
    G5 = G[:].rearrange("p (sc l2 j) -> p sc l2 j", sc=S * C, l2=L2, j=2)
    Z = sbZ.tile([P, ncols], bf16, tag="Z")
    Z4 = Z[:].rearrange("p (s c l) -> p s c l", s=S, c=C, l=L)
    Z5 = Z[:].rearrange("p (sc l2 j) -> p sc l2 j", sc=S * C, l2=L2, j=2)
    fdb = fdp_ap.rearrange("p s c j -> p (s c) j").unsqueeze(2) \
        .broadcast_to([P, S * C, L2, 2])
    nc.vector.tensor_tensor(out=Z5, in0=G5, in1=fdb, op=AL.add)
    for (c0, c1, al) in runs:
        nc.scalar.activation(out=Z4[:, :, c0:c1, :], in_=Z4[:, :, c0:c1, :],
                             func=AF.Prelu, alpha=al)
    Zp = Z4.rearrange("p s (h f) l -> p s h f l", h=2, f=2)
    SC = sbT.tile([P, S * 2 * L], bf16, tag="SC")
    SC3 = SC[:].rearrange("p (s c l) -> p s c l", s=S, c=2, l=L)
    nc.vector.tensor_tensor(out=SC3, in0=Zp[:, :, :, 0, :],
                            in1=Zp[:, :, :, 1, :], op=AL.add)
    EX = sbEX.tile([P, S * 2 * L], bf16, tag="EX")
    EX3 = EX[:].rearrange("p (s c l) -> p s c l", s=S, c=2, l=L)
    nc.scalar.activation(out=EX3, in_=SC3, func=AF.Exp)
    for h in range(2):
        Gh = G4[:, :, 2 * h:2 * h + 2, :]
        exd = EX3[:, :, h:h + 1, :].broadcast_to([P, S, 2, L])
        nc.vector.tensor_tensor(out=Gh, in0=Gh, in1=exd, op=AL.mult)
    # ping-pong partner = Z (dead after score tree)
    pp_tree(nc, G4, Z4[:, :, :, 0:L // 2 + 1], L, h2v.unsqueeze(3))
    halving_tree(nc.gpsimd.tensor_tensor, EX3, L, denv.unsqueeze(3))


def emit_fixup(nc, sbS, hv, hflat, dv, dflat, fd_ap, rs_flat, np_ap, isc_ap,
               T, C, NH, dF, runs, do_elu=True):
    """Pad-slot den fix + normalize + unscale + residual + elu over T tiles.
    fd_ap: [P, T, C] per-node scaled dest proj; isc_ap: [P, C] inv scales."""
    zp = sbS.tile([P, T * C], bf16, tag="zp")
    zp3 = zp[:].rearrange("p (t c) -> p t c", t=T, c=C)
    for (c0, c1, al) in runs:
        nc.scalar.activation(out=zp3[:, :, c0:c1], in_=fd_ap[:, :, c0:c1],
                             func=AF.Prelu, alpha=al)
    ep = sbS.tile([P, T * NH], bf16, tag="ep")
    ep3 = ep[:].rearrange("p (t h) -> p t h", t=T, h=NH)
    if C == 12:
        zph = zp3[:, :, 0:10].rearrange("p t (h f) -> p t h f", h=2, f=5)
        tp = sbS.tile([P, T * 2], bf16, tag="tp")
        tp3 = tp[:].rearrange("p (t h) -> p t h", t=T, h=2)
        nc.vector.tensor_tensor(out=tp3, in0=zph[:, :, :, 0], in1=zph[:, :, :, 1], op=AL.add)
        nc.vector.tensor_tensor(out=tp3, in0=tp3, in1=zph[:, :, :, 2], op=AL.add)
        nc.vector.tensor_tensor(out=tp3, in0=tp3, in1=zph[:, :, :, 3], op=AL.add)
        nc.vector.tensor_tensor(out=tp3, in0=tp3, in1=zph[:, :, :, 4], op=AL.add)
        nc.scalar.activation(out=ep3[:, :, 0:2], in_=tp3, func=AF.Exp)
        nc.scalar.activation(out=ep3[:, :, 2:4], in_=zp3[:, :, 10:12], func=AF.Exp)
    else:
        zph = zp3.rearrange("p t (h f) -> p t h f", h=2, f=2)
        tp = sbS.tile([P, T * 2], bf16, tag="tp")
        tp3 = tp[:].rearrange("p (t h) -> p t h", t=T, h=2)
        nc.vector.tensor_tensor(out=tp3, in0=zph[:, :, :, 0], in1=zph[:, :, :, 1], op=AL.add)
        nc.scalar.activation(out=ep3, in_=tp3, func=AF.Exp)
    padm = sbS.tile([P, T * NH], f32, tag="padm")
    pm3 = padm[:].rearrange("p (t h) -> p t h", t=T, h=NH)
    npb = np_ap.unsqueeze(2).broadcast_to([P, T, NH])
    nc.vector.tensor_tensor(out=pm3, in0=ep3, in1=npb, op=AL.mult)
    nc.vector.tensor_tensor(out=dv, in0=dv, in1=pm3, op=AL.subtract)
    nc.vector.tensor_scalar_max(out=dflat, in0=dflat, scalar1=1e-30)
    rec = sbS.tile([P, T * NH], f32, tag="rec")
    nc.vector.reciprocal(out=rec[:], in_=dflat)
    rec3 = rec[:].rearrange("p (t h) -> p t h", t=T, h=NH)
    # rec12 = rec[h(c)] * inv_scale_c
    rc = sbS.tile([P, T * C], f32, tag="rc")
    rc3 = rc[:].rearrange("p (t c) -> p t c", t=T, c=C)
    iscb = isc_ap.unsqueeze(1).broadcast_to([P, T, C])
    if C == 12:
        rch = rc3[:, :, 0:10].rearrange("p t (h f) -> p t h f", h=2, f=5)
        rb = rec3[:, :, 0:2].unsqueeze(3).broadcast_to([P, T, 2, 5])
        ib = iscb[:, :, 0:10].rearrange("p t (h f) -> p t h f", h=2, f=5)
        nc.vector.tensor_tensor(out=rch, in0=rb, in1=ib, op=AL.mult)
        nc.vector.tensor_tensor(out=rc3[:, :, 10:12], in0=rec3[:, :, 2:4],
                                in1=iscb[:, :, 10:12], op=AL.mult)
    else:
        rch = rc3.rearrange("p t (h f) -> p t h f", h=2, f=2)
        rb = rec3.unsqueeze(3).broadcast_to([P, T, 2, 2])
        ib = iscb.rearrange("p t (h f) -> p t h f", h=2, f=2)
        nc.vector.tensor_tensor(out=rch, in0=rb, in1=ib, op=AL.mult)
    nc.vector.tensor_tensor(out=hflat, in0=hflat, in1=rc[:], op=AL.mult)
    nc.vector.tensor_tensor(out=hflat, in0=hflat, in1=rs_flat, op=AL.add)
    if do_elu:
        emit_elu(nc, sbS, hflat, T * C, "elu")


# =============================================================== launch A
def build_launchA(nt):
    cols = nt * 16
    nc = bacc.Bacc("TRN2", target_bir_lowering=False, debug=False, num_devices=NCORE)
    d_x = nc.dram_tensor("x5l", [48, cols], bf16, kind="ExternalInput")
    d_bfs = nc.dram_tensor("bd_fs", [48, P], bf16, kind="ExternalInput")
    d_bfd = nc.dram_tensor("bd_fd", [48, P], bf16, kind="ExternalInput")
    d_brs = nc.dram_tensor("bd_rs", [48, P], bf16, kind="ExternalInput")
    d_fs = nc.dram_tensor("fs1cm", [P, cols], bf16, kind="ExternalOutput")
    d_fd = nc.dram_tensor("fd1cm", [P, cols], bf16, kind="ExternalOutput")
    d_rs = nc.dram_tensor("rs1cm", [P, cols], f32, kind="ExternalOutput")
    with TileContext(nc) as tc:
        with tc.tile_pool(name="res", bufs=1) as res, \
             tc.tile_pool(name="ps", bufs=2, space="PSUM") as ps:
            stg = res.tile([48, cols], bf16)
            nc.sync.dma_start(out=stg[:], in_=d_x[:, :])
            bfs = res.tile([48, P], bf16, tag="bfs")
            nc.sync.dma_start(out=bfs[:], in_=d_bfs[:, :])
            bfd = res.tile([48, P], bf16, tag="bfd")
            nc.sync.dma_start(out=bfd[:], in_=d_bfd[:, :])
            brs = res.tile([48, P], bf16, tag="brs")
            nc.sync.dma_start(out=brs[:], in_=d_brs[:, :])
            ofs = res.tile([P, cols], bf16, tag="ofs")
            ofd = res.tile([P, cols], bf16, tag="ofd")
            ors = res.tile([P, cols], f32, tag="ors")
            k = 0
            for j0 in range(0, cols, 512):
                w = min(512, cols - j0)
                for bd, ot in ((bfs, ofs), (bfd, ofd), (brs, ors)):
                    pmm = ps.tile([P, 512], f32, tag="mm")
                    nc.tensor.matmul(out=pmm[:, :w], lhsT=bd[:], rhs=stg[:, j0:j0 + w],
                                     start=True, stop=True)
                    if k % 2 == 0:
                        nc.vector.tensor_copy(out=ot[:, j0:j0 + w], in_=pmm[:, :w])
                    else:
                        nc.scalar.copy(out=ot[:, j0:j0 + w], in_=pmm[:, :w])
                    k += 1
            nc.sync.dma_start(out=d_fs[:, :], in_=ofs[:])
            nc.sync.dma_start(out=d_fd[:, :], in_=ofd[:])
            nc.sync.dma_start(out=d_rs[:, :], in_=ors[:])
    nc.compile()
    return nc


# =============================================================== launch B
def build_launchB(nst, Ls, offs12, nt, runs12):
    totc = int(offs12[-1])
    fgw = -(-nst // 4) * SUPER * P          # f2 output column width
    nc = bacc.Bacc("TRN2", target_bir_lowering=False, debug=False, num_devices=NCORE)
    d_g = nc.dram_tensor("g1", [P, totc], bf16, kind="ExternalInput")
    d_fdp = nc.dram_tensor("fdp1", [P, nt * 24], bf16, kind="ExternalInput")
    d_rs = nc.dram_tensor("rs1n", [P, nt * 12], f32, kind="ExternalInput")
    d_np = nc.dram_tensor("npad", [P, nt], f32, kind="ExternalInput")
    d_isc = nc.dram_tensor("isc12", [P, 12], f32, kind="ExternalInput")
    d_w2 = nc.dram_tensor("w2all", [10, 12], bf16, kind="ExternalInput")
    d_bc2 = nc.dram_tensor("bc2", [12], f32, kind="ExternalInput")
    d_f2 = nc.dram_tensor("f2cm", [48, fgw], f32, kind="ExternalOutput")
    d_ha = nc.dram_tensor("hattn", [P, nt * 2], bf16, kind="ExternalOutput")
    groups = [(0, 5), (5, 9), (9, nst)] if nst >= 9 else [(0, nst)]
    with TileContext(nc) as tc:
        with tc.tile_pool(name="res", bufs=1) as res, \
             tc.tile_pool(name="sbG", bufs=2) as sbG, \
             tc.tile_pool(name="sbZ", bufs=2) as sbZ, \
             tc.tile_pool(name="sbEX", bufs=3) as sbEX, \
             tc.tile_pool(name="sbT", bufs=2) as sbT, \
             tc.tile_pool(name="sbS", bufs=1) as sbS, \
             tc.tile_pool(name="sbT2", bufs=1) as sbT2, \
             tc.tile_pool(name="psT", bufs=1, space="PSUM") as psT, \
             tc.tile_pool(name="psF", bufs=1, space="PSUM") as psF:
            ident = res.tile([P, P], bf16)
            make_identity(nc, ident[:])
            npad = res.tile([P, nt], f32)
            nc.sync.dma_start(out=npad[:], in_=d_np[:, :])
            isc = res.tile([P, 12], f32)
            nc.sync.dma_start(out=isc[:], in_=d_isc[:, :])
            fdp = res.tile([P, nt * 24], bf16)
            nc.sync.dma_start(out=fdp[:], in_=d_fdp[:, :])
            rst = res.tile([P, nt * 12], f32)
            nc.sync.dma_start(out=rst[:], in_=d_rs[:, :])
            w2t = res.tile([10, 12], bf16, tag="w2t")
            nc.sync.dma_start(out=w2t[:], in_=d_w2[:, :])
            bc2 = res.tile([12, 1], f32, tag="bc2")
            nc.sync.dma_start(out=bc2[:], in_=d_bc2[:, None])
            h1 = res.tile([P, nt * 12], f32)
            den = res.tile([P, nt * 4], f32)
            hat = res.tile([P, nt * 2], bf16)
            h1v_all = h1[:].rearrange("p (t c) -> p t c", t=nt, c=12)
            denv_all = den[:].rearrange("p (t c) -> p t c", t=nt, c=4)
            fdp_all = fdp[:].rearrange("p (t c j) -> p t c j", t=nt, c=12, j=2)
            npv_all = npad[:].rearrange("p (t o) -> p t o", t=nt, o=1)
            for g0, g1 in groups:
                for st in range(g0, g1):
                    L = Ls[st]
                    t0 = st * SUPER
                    emit_gat_st_B(nc, sbG, sbZ, sbEX, sbT, d_g, int(offs12[st]),
                                  L, SUPER, fdp_all[:, t0:t0 + SUPER],
                                  runs12,
                                  h1v_all[:, t0:t0 + SUPER, :],
                                  denv_all[:, t0:t0 + SUPER, :])
                T = (g1 - g0) * SUPER
                t0 = g0 * SUPER
                emit_fixup(nc, sbS, h1v_all[:, t0:t0 + T, :],
                           h1[:, t0 * 12:(t0 + T) * 12],
                           denv_all[:, t0:t0 + T, :],
                           den[:, t0 * 4:(t0 + T) * 4],
                           fdp_all[:, t0:t0 + T, :, 0],
                           rst[:, t0 * 12:(t0 + T) * 12],
                           npv_all[:, t0:t0 + T, 0], isc[:], T, 12, 4, 5, runs12)
                hv = h1v_all[:, t0:t0 + T, :]
                nc.scalar.copy(out=hat[:].rearrange("p (t c) -> p t c", t=nt, c=2)[:, t0:t0 + T, :],
                               in_=hv[:, :, 10:12])
                h1b = sbS.tile([P, T * 10], bf16, tag="h1b")
                nc.scalar.copy(out=h1b[:].rearrange("p (t c) -> p t c", t=T, c=10),
                               in_=hv[:, :, 0:10])
                # epilogue: transpose h_def1 per tile, project fs2''|fd2''|res2
                for st in range(g0, g1):
                    pT = psT.tile([10, SUPER * P], bf16, tag="pT")
                    for b in range(SUPER):
                        trel = (st - g0) * SUPER + b
                        nc.tensor.transpose(out=pT[:, b * P:(b + 1) * P],
                                            in_=h1b[:, trel * 10:trel * 10 + 10],
                                            identity=ident[:])
                    hT = sbT2.tile([10, SUPER * P], bf16, tag="hT")
                    nc.vector.tensor_copy(out=hT[:], in_=pT[:])
                    pF = psF.tile([12, SUPER * P], f32, tag="pF")
                    for q in range(SUPER * P // 512):
                        nc.tensor.matmul(out=pF[:, q * 512:(q + 1) * 512], lhsT=w2t[:],
                                         rhs=hT[:, q * 512:(q + 1) * 512],
                                         start=True, stop=True)
                    f2s = sbT2.tile([12, SUPER * P], f32, tag="f2s")
                    nc.scalar.activation(out=f2s[:], in_=pF[:], func=AF.Prelu,
                                         alpha=1.0, bias=bc2[:])
                    nc.sync.dma_start(
                        out=d_f2[12 * (st % 4):12 * (st % 4) + 12,
                                 (st // 4) * SUPER * P:(st // 4 + 1) * SUPER * P],
                        in_=f2s[:])
            nc.sync.dma_start(out=d_ha[:, :], in_=hat[:])
    nc.compile()
    return nc


# =============================================================== launch C
def build_launchC(nst, Ls, offs4, nt, nmc, runs4):
    totc = int(offs4[-1])
    ngrp = -(-nmc // 4)
    nc = bacc.Bacc("TRN2", target_bir_lowering=False, debug=False, num_devices=NCORE)
    d_g = nc.dram_tensor("g2", [P, totc], bf16, kind="ExternalInput")
    d_fdp = nc.dram_tensor("fdp2", [P, nt * 8], bf16, kind="ExternalInput")
    d_rs = nc.dram_tensor("rs2n", [P, nt * 4], f32, kind="ExternalInput")
    d_np = nc.dram_tensor("npad", [P, nt], f32, kind="ExternalInput")
    d_isc = nc.dram_tensor("isc4", [P, 4], f32, kind="ExternalInput")
    d_ha = nc.dram_tensor("hattn", [P, nt * 2], bf16, kind="ExternalInput")
    d_x = nc.dram_tensor("xpm", [P, nt * 8], bf16, kind="ExternalInput")
    d_w1 = nc.dram_tensor("w1x", [32, 196], bf16, kind="ExternalInput")
    w2dt = fp8 if FP8_W2 else bf16
    d_w2a = nc.dram_tensor("w2dra", [P, 2 * 128], w2dt, kind="ExternalInput")
    d_w2b = nc.dram_tensor("w2drb", [P, 2 * 128], w2dt, kind="ExternalInput")
    d_w3a = nc.dram_tensor("w3a", [P, 14], bf16, kind="ExternalInput")
    d_w3b = nc.dram_tensor("w3b", [68, 14], bf16, kind="ExternalInput")
    d_w4 = nc.dram_tensor("w4blk", [110, 4], bf16, kind="ExternalInput")
    d_b1 = nc.dram_tensor("b1", [196], f32, kind="ExternalInput")
    d_b2 = nc.dram_tensor("b2", [196], f32, kind="ExternalInput")
    d_b3 = nc.dram_tensor("b3", [14], f32, kind="ExternalInput")
    d_b4 = nc.dram_tensor("b4r", [4], f32, kind="ExternalInput")
    d_out = nc.dram_tensor("out", [nmc, 512], f32, kind="ExternalOutput")
    groups = [(0, 3), (3, 6), (6, 9), (9, 11), (11, nst)] if nst >= 11 else [(0, nst)]
    with TileContext(nc) as tc:
        with tc.tile_pool(name="res", bufs=1) as res, \
             tc.tile_pool(name="sbG", bufs=2) as sbG, \
             tc.tile_pool(name="sbZ", bufs=2) as sbZ, \
             tc.tile_pool(name="sbEX", bufs=3) as sbEX, \
             tc.tile_pool(name="sbT", bufs=2) as sbT, \
             tc.tile_pool(name="sbS", bufs=1) as sbS, \
             tc.tile_pool(name="sbM", bufs=2) as sbM, \
             tc.tile_pool(name="psT", bufs=2, space="PSUM") as psT, \
             tc.tile_pool(name="psC", bufs=1, space="PSUM") as psC, \
             tc.tile_pool(name="psA", bufs=1, space="PSUM") as psA, \
             tc.tile_pool(name="psB", bufs=1, space="PSUM") as psB:
            ident = res.tile([P, P], bf16)
            make_identity(nc, ident[:])
            npad = res.tile([P, nt], f32)
            nc.sync.dma_start(out=npad[:], in_=d_np[:, :])
            isc = res.tile([P, 4], f32)
            nc.sync.dma_start(out=isc[:], in_=d_isc[:, :])
            fdp = res.tile([P, nt * 8], bf16)
            nc.sync.dma_start(out=fdp[:], in_=d_fdp[:, :])
            rst = res.tile([P, nt * 4], f32)
            nc.sync.dma_start(out=rst[:], in_=d_rs[:, :])
            hat = res.tile([P, nt * 2], bf16)
            nc.sync.dma_start(out=hat[:], in_=d_ha[:, :])
            xpm = res.tile([P, nt * 8], bf16)
            nc.sync.dma_start(out=xpm[:], in_=d_x[:, :])
            w1 = res.tile([32, 196], bf16, tag="w1")
            nc.sync.dma_start(out=w1[:], in_=d_w1[:, :])
            w2a = res.tile([P, 2 * 128], w2dt, tag="w2a")
            nc.sync.dma_start(out=w2a[:], in_=d_w2a[:, :])
            w2b = res.tile([P, 2 * 128], w2dt, tag="w2b")
            nc.sync.dma_start(out=w2b[:], in_=d_w2b[:, :])
            w3a = res.tile([P, 14], bf16, tag="w3a")
            nc.sync.dma_start(out=w3a[:], in_=d_w3a[:, :])
            w3b = res.tile([68, 14], bf16, tag="w3b")
            nc.sync.dma_start(out=w3b[:], in_=d_w3b[:, :])
            w4b = res.tile([110, 4], bf16, tag="w4b")
            nc.sync.dma_start(out=w4b[:], in_=d_w4[:, :])
            w2av = w2a[:].rearrange("p (k m) -> p k m", k=2, m=128)
            w2bv = w2b[:].rearrange("p (k m) -> p k m", k=2, m=128)
            b1ca = res.tile([P, 1], f32, tag="b1ca")
            nc.sync.dma_start(out=b1ca[:], in_=d_b1[0:128, None])
            b1cb = res.tile([68, 1], f32, tag="b1cb")
            nc.sync.dma_start(out=b1cb[:], in_=d_b1[128:196, None])
            b2ca = res.tile([P, 1], f32, tag="b2ca")
            nc.sync.dma_start(out=b2ca[:], in_=d_b2[0:128, None])
            b2cb = res.tile([68, 1], f32, tag="b2cb")
            nc.sync.dma_start(out=b2cb[:], in_=d_b2[128:196, None])
            b3c = res.tile([14, 1], f32, tag="b3c")
            nc.sync.dma_start(out=b3c[:], in_=d_b3[:, None])
            b4c = res.tile([4, 1], f32, tag="b4c")
            nc.sync.dma_start(out=b4c[:], in_=d_b4[:, None])
            h2 = res.tile([P, nt * 4], f32)
            den = res.tile([P, nt * 2], f32)
            m32 = res.tile([P, nt * 32], bf16)
            nc.gpsimd.memset(m32[:], 0.0)
            r3st = res.tile([110, ngrp * 512], bf16, tag="r3st")
            adt = fp8 if FP8_W2 else bf16
            r1t = [res.tile([P, 2048], adt, tag=f"r1_{i}", name=f"r1_{i}")
                   for i in range(2)]
            for t_ in r1t:
                nc.gpsimd.memset(t_[:], 0.0)
            h2v_all = h2[:].rearrange("p (t c) -> p t c", t=nt, c=4)
            denv_all = den[:].rearrange("p (t c) -> p t c", t=nt, c=2)
            fdp_all = fdp[:].rearrange("p (t c j) -> p t c j", t=nt, c=4, j=2)
            npv_all = npad[:].rearrange("p (t o) -> p t o", t=nt, o=1)
            m3_all = m32[:].rearrange("p (t c) -> p t c", t=nt, c=32)
            hav_all = hat[:].rearrange("p (t c) -> p t c", t=nt, c=2)
            xv_all = xpm[:].rearrange("p (t c) -> p t c", t=nt, c=8)
            nc.gpsimd.tensor_copy(out=m3_all[:, :, 0:2], in_=hav_all)
            nc.gpsimd.tensor_copy(out=m3_all[:, :, 6:14], in_=xv_all)
            for g0, g1 in groups:
                for st in range(g0, g1):
                    L = Ls[st]
                    t0 = st * SUPER
                    emit_gat_st_C(nc, sbG, sbZ, sbEX, sbT, d_g, int(offs4[st]),
                                  L, SUPER, fdp_all[:, t0:t0 + SUPER],
                                  runs4,
                                  h2v_all[:, t0:t0 + SUPER, :],
                                  denv_all[:, t0:t0 + SUPER, :])
                T = (g1 - g0) * SUPER
                t0 = g0 * SUPER
                emit_fixup(nc, sbS, h2v_all[:, t0:t0 + T, :],
                           h2[:, t0 * 4:(t0 + T) * 4],
                           denv_all[:, t0:t0 + T, :],
                           den[:, t0 * 2:(t0 + T) * 2],
                           fdp_all[:, t0:t0 + T, :, 0],
                           rst[:, t0 * 4:(t0 + T) * 4],
                           npv_all[:, t0:t0 + T, 0], isc[:], T, 4, 2, 2, runs4)
                nc.gpsimd.tensor_copy(out=m3_all[:, t0:t0 + T, 2:6],
                                      in_=h2v_all[:, t0:t0 + T, :])
                # MLP over this group's chunks, processed in pairs so each
                # activation instruction covers 1024 nodes (half the Act
                # instruction count)
                mc1 = min((t0 + T) // 4, nmc)
                for mcp in range(t0 // 4, mc1, 2):
                    pair = [mc for mc in (mcp, mcp + 1) if mc < mc1]
                    npair = len(pair)
                    p1a = psA.tile([P, 1024], f32, tag="pA")
                    p1b = psB.tile([P, 1024], f32, tag="pB")
                    for ci_, mc in enumerate(pair):
                        pT = psT.tile([64, 256], bf16, tag="pT")
                        nc.tensor.transpose(out=pT[:, 0:128],
                                            in_=m32[:, (mc * 4) * 32:(mc * 4 + 2) * 32],
                                            identity=ident[:])
                        nc.tensor.transpose(out=pT[:, 128:256],
                                            in_=m32[:, (mc * 4 + 2) * 32:(mc * 4 + 4) * 32],
                                            identity=ident[:])
                        r0 = sbM.tile([32, 512], bf16, tag=f"r0_{ci_}")
                        nc.vector.tensor_copy(out=r0[:, 0:128], in_=pT[0:32, 0:128])
                        nc.vector.tensor_copy(out=r0[:, 128:256], in_=pT[32:64, 0:128])
                        nc.vector.tensor_copy(out=r0[:, 256:384], in_=pT[0:32, 128:256])
                        nc.vector.tensor_copy(out=r0[:, 384:512], in_=pT[32:64, 128:256])
                        nc.tensor.matmul(out=p1a[:, ci_ * 512:(ci_ + 1) * 512],
                                         lhsT=w1[:, 0:128], rhs=r0[:], start=True, stop=True)
                        nc.tensor.matmul(out=p1b[0:68, ci_ * 512:(ci_ + 1) * 512],
                                         lhsT=w1[:, 128:196], rhs=r0[:], start=True, stop=True)
                    r1p = r1t[(mcp // 2) % 2]          # [P, 2048] fp8: cols (k, chunk, 512)
                    w = npair * 512
                    nc.scalar.activation(out=r1p[:, 0:w], in_=p1a[:, 0:w],
                                         func=AF.Prelu, alpha=NEG_MLP, bias=b1ca[:])
                    nc.scalar.activation(out=r1p[0:68, 1024:1024 + w], in_=p1b[0:68, 0:w],
                                         func=AF.Prelu, alpha=NEG_MLP, bias=b1cb[:])
                    r1v = r1p[:].rearrange("p (k c n) -> p k c n", k=2, c=2, n=512)
                    p2a = psA.tile([P, 1024], f32, tag="pA")
                    p2b = psB.tile([P, 1024], f32, tag="pB")
                    for ci_, mc in enumerate(pair):
                        nc.tensor.matmul(out=p2a[:, ci_ * 512:(ci_ + 1) * 512], lhsT=w2av,
                                         rhs=r1v[:, :, ci_, :],
                                         start=True, stop=True, perf_mode=MPM.DoubleRow)
                        nc.tensor.matmul(out=p2b[:, ci_ * 512:(ci_ + 1) * 512], lhsT=w2bv,
                                         rhs=r1v[:, :, ci_, :],
                                         start=True, stop=True, perf_mode=MPM.DoubleRow)
                    r2a = sbM.tile([P, 1024], bf16, tag="r2a")
                    nc.scalar.activation(out=r2a[:, 0:w], in_=p2a[:, 0:w], func=AF.Prelu,
                                         alpha=NEG_MLP, bias=b2ca[:])
                    r2b = sbM.tile([68, 1024], bf16, tag="r2b")
                    nc.scalar.activation(out=r2b[:, 0:w], in_=p2b[0:68, 0:w], func=AF.Prelu,
                                         alpha=NEG_MLP, bias=b2cb[:])
                    p3 = psC.tile([14, 1024], f32, tag="p3")
                    for ci_, mc in enumerate(pair):
                        nc.tensor.matmul(out=p3[:, ci_ * 512:(ci_ + 1) * 512], lhsT=w3a[:],
                                         rhs=r2a[:, ci_ * 512:(ci_ + 1) * 512], start=True, stop=False)
                        nc.tensor.matmul(out=p3[:, ci_ * 512:(ci_ + 1) * 512], lhsT=w3b[:],
                                         rhs=r2b[0:68, ci_ * 512:(ci_ + 1) * 512], start=False, stop=True)
                        ro = 32 * (mc % 4)
                        nc.scalar.activation(out=r3st[ro:ro + 14, (mc // 4) * 512:(mc // 4 + 1) * 512],
                                             in_=p3[:, ci_ * 512:(ci_ + 1) * 512], func=AF.Prelu,
                                             alpha=NEG_MLP, bias=b3c[:])
            # tail: stacked 14->1 matmuls + sigmoids (one act-table switch)
            for g in range(ngrp):
                k = min(4, nmc - 4 * g)
                kp = 32 * (k - 1) + 14
                po = psC.tile([14, 1024], f32, tag="p3")
                nc.tensor.matmul(out=po[0:k, 0:512], lhsT=w4b[0:kp, 0:k],
                                 rhs=r3st[0:kp, g * 512:(g + 1) * 512],
                                 start=True, stop=True)
                sg = sbM.tile([4, 512], f32, tag="sg")
                nc.scalar.activation(out=sg[0:k, :], in_=po[0:k, 0:512], func=AF.Sigmoid,
                                     bias=b4c[0:k, :])
                nc.sync.dma_start(out=d_out[4 * g:4 * g + k, :], in_=sg[0:k, :])
    nc.compile()
    return nc


# ================================================================== kernel
_cache = {}


def kernel(**inputs):
    x = np.asarray(inputs['x'], np.float32)
    src = np.asarray(inputs['src'], np.int32)
    dst = np.asarray(inputs['dst'], np.int32)
    n = x.shape[0]

    scheds, nst, Ls = build_schedule(dst, n)
    nt = scheds[0]['nt']
    nloc = scheds[0]['nloc']
    nmc = -(-nloc // 512)
    offs12 = np.concatenate([[0], np.cumsum([SUPER * L * 12 for L in Ls])]).astype(np.int64)
    offs4 = np.concatenate([[0], np.cumsum([SUPER * L * 4 for L in Ls])]).astype(np.int64)

    # ---- layer-1 attn folding: channels [d1h0(5), d1h1(5), a1h0, a1h1]
    d1_attn = np.asarray(inputs['d1_attn'], np.float64)     # [2, 5]
    a1_attn = np.asarray(inputs['a1_attn'], np.float64)     # [2, 1]
    perm_d1, scale_d1, alpha_d1 = attn_fold(d1_attn, 2, 5)
    perm_a1, scale_a1, alpha_a1 = attn_fold(a1_attn[:, :], 2, 1)
    scale12 = np.concatenate([scale_d1, scale_a1])
    alpha12 = alpha_d1 + alpha_a1
    runs12 = alpha_runs(alpha12)

    def l1_pack(a1_w, d1_w, scale=None):
        w = np.zeros((a1_w.shape[0], 12), np.float64)
        for p_, j in enumerate(perm_d1):
            w[:, p_] = d1_w[:, j]
        for p_, j in enumerate(perm_a1):
            w[:, 10 + p_] = a1_w[:, j]
        if scale is not None:
            w = w * scale[None, :]
        return w

    a1_Wsrc = np.asarray(inputs['a1_Wsrc'], np.float64)
    d1_Wsrc = np.asarray(inputs['d1_Wsrc'], np.float64)
    a1_Wdst = np.asarray(inputs['a1_Wdst'], np.float64)
    d1_Wdst = np.asarray(inputs['d1_Wdst'], np.float64)
    a1_Wres = np.asarray(inputs['a1_Wres'], np.float64)
    d1_Wres = np.asarray(inputs['d1_Wres'], np.float64)
    bY = l1_pack(np.asarray(inputs['a1_bsrc'], np.float64)[None, :],
                 np.asarray(inputs['d1_bsrc'], np.float64)[None, :], scale12)[0]
    bD = l1_pack(np.asarray(inputs['a1_bdst'], np.float64)[None, :],
                 np.asarray(inputs['d1_bdst'], np.float64)[None, :], scale12)[0]
    bR = l1_pack(np.asarray(inputs['a1_bias'], np.float64)[None, :],
                 np.asarray(inputs['d1_bias'], np.float64)[None, :])[0]
    bd_fs = blockdiag(l1_pack(a1_Wsrc, d1_Wsrc, scale12).astype(np.float32), bY.astype(np.float32), 6)
    bd_fd = blockdiag(l1_pack(a1_Wdst, d1_Wdst, scale12).astype(np.float32), bD.astype(np.float32), 6)
    bd_rs = blockdiag(l1_pack(a1_Wres, d1_Wres).astype(np.float32), bR.astype(np.float32), 6)
    isc12 = np.tile((1.0 / scale12).astype(np.float32), (P, 1))

    # ---- layer-2 attn folding: channels [d2h0(2), d2h1(2)]
    d2_attn = np.asarray(inputs['d2_attn'], np.float64)     # [2, 2]
    perm_d2, scale4, alpha4 = attn_fold(d2_attn, 2, 2)
    runs4 = alpha_runs(alpha4)

    def d2w(name):
        w = np.asarray(inputs[name], np.float64)            # [10, 4] native cols j=2h+f
        out = np.zeros((10, 4), np.float64)
        for p_, j in enumerate(perm_d2):
            out[:, p_] = w[:, j]
        return out

    def d2b(name):
        b = np.asarray(inputs[name], np.float64)
        return b[perm_d2]

    # rows of the [10, 12] projection are h_def1 in MY permuted order
    rowperm = perm_d1                                       # position -> native j=5h+f
    ws2 = d2w('d2_Wsrc')[rowperm] * scale4[None, :]
    wd2 = d2w('d2_Wdst')[rowperm] * scale4[None, :]
    wr2 = d2w('d2_Wres')[rowperm]
    w2all = np.concatenate([ws2, wd2, wr2], axis=1).astype(np.float32)
    bc2 = np.concatenate([d2b('d2_bsrc') * scale4, d2b('d2_bdst') * scale4,
                          d2b('d2_bias')]).astype(np.float32)
    isc4 = np.tile((1.0 / scale4).astype(np.float32), (P, 1))

    # ---- MLP weights: W1 rows 2:6 permuted to h_def2 order
    w1p = np.asarray(inputs['W1'], np.float64).copy()
    W1n = np.asarray(inputs['W1'], np.float64)
    for p_, j in enumerate(perm_d2):
        w1p[2 + p_] = W1n[2 + j]
    w1x = np.zeros((32, 196), np.float32)
    w1x[0:14] = w1p.astype(np.float32)
    W2 = np.asarray(inputs['W2'], np.float32)
    w2dra = np.zeros((P, 2, 128), np.float32)
    w2dra[:, 0, :] = W2[0:128, 0:128]
    w2dra[0:68, 1, :] = W2[128:196, 0:128]
    w2drb = np.zeros((P, 2, 128), np.float32)
    w2drb[:, 0, 0:68] = W2[0:128, 128:196]
    w2drb[0:68, 1, 0:68] = W2[128:196, 128:196]
    FPW = FP8 if FP8_W2 else BF
    w4 = np.asarray(inputs['W4'], np.float32)               # [14, 1]
    w4blk = np.zeros((110, 4), np.float32)
    for k in range(4):
        w4blk[32 * k:32 * k + 14, k] = w4[:, 0]
    b4r = np.full(4, float(np.asarray(inputs['b4'])[0]), np.float32)

    key = (n, len(src), nst, tuple(Ls), tuple(runs12), tuple(runs4))
    if key not in _cache:
        _cache.clear()
        _cache[key] = (build_launchA(nt), build_launchB(nst, Ls, offs12, nt, runs12),
                       build_launchC(nst, Ls, offs4, nt, nmc, runs4))
    ncA, ncB, ncC = _cache[key]

    # ---------------- launch A: per-node projections of x
    inA = []
    for s in scheds:
        orig = s['order']
        valid = orig < nloc
        xl = np.zeros((nt * P, 5), np.float32)
        xl[valid] = x[s['core'] * nloc + orig[valid], :5]
        inA.append(dict(x5l=pack_local(xl, 6, nt), bd_fs=bd_fs, bd_fd=bd_fd, bd_rs=bd_rs))
    rA = run_bass_kernel_spmd(ncA, inA, core_ids=list(range(NCORE)))
    tA = rA.exec_time_ns or 0

    i_all = np.arange(nt * P)
    a_i = (i_all // P) % 8
    col_i = (i_all // (8 * P)) * P + i_all % P
    rows12 = a_i[:, None] * 16 + np.arange(12)[None, :]
    fs1g = np.zeros((n, 12), BF)
    geoms, fdp1_l, rs1n_l, npad_l = [], [], [], []
    for ci, s in enumerate(scheds):
        fs_sorted = rA.results[ci]['fs1cm'][rows12, col_i[:, None]]
        fd_sorted = rA.results[ci]['fd1cm'][rows12, col_i[:, None]]
        rs_sorted = rA.results[ci]['rs1cm'][rows12, col_i[:, None]]
        orig = s['order']
        valid = orig < nloc
        fs1g[s['core'] * nloc + orig[valid]] = fs_sorted[valid]
        fdp1_l.append(pm_pair(fd_sorted, nt))
        rs1n_l.append(pm(rs_sorted.astype(np.float32), nt))
        geoms.append(edge_slot_geom(s, Ls))
        npad_l.append(make_npad(s, Ls, nt))

    inB = []
    for ci, s in enumerate(scheds):
        eo, st_of, s_of, rank, p_of = geoms[ci]
        v = fs1g[src[s['em']][eo]]
        g1 = pack_G(v, st_of, s_of, rank, p_of, offs12, 12, Ls, int(offs12[-1]))
        inB.append(dict(g1=g1, fdp1=fdp1_l[ci], rs1n=rs1n_l[ci], isc12=isc12,
                        npad=npad_l[ci], w2all=w2all.astype(BF), bc2=bc2))
    rB = run_bass_kernel_spmd(ncB, inB, core_ids=list(range(NCORE)))
    tB = rB.exec_time_ns or 0

    fgw = -(-nst // 4) * SUPER * P
    fs2g = np.zeros((n, 4), BF)
    fdp2_l, rs2n_l, ha_l, xpm_l = [], [], [], []
    for ci, s in enumerate(scheds):
        fb = rB.results[ci]['f2cm']              # [48, fgw]
        f2 = np.zeros((12, nt * P), np.float32)
        for st in range(nst):
            f2[:, st * SUPER * P:(st + 1) * SUPER * P] = \
                fb[12 * (st % 4):12 * (st % 4) + 12,
                   (st // 4) * SUPER * P:(st // 4 + 1) * SUPER * P]
        orig = s['order']
        valid = orig < nloc
        fs2g[s['core'] * nloc + orig[valid]] = f2[0:4, :].T[valid].astype(BF)
        fdp2_l.append(pm_pair(f2[4:8, :].T.astype(BF), nt))
        rs2n_l.append(pm(np.ascontiguousarray(f2[8:12, :].T), nt))
        ha_l.append(rB.results[ci]['hattn'])
        xl8 = np.zeros((nt * P, 8), np.float32)
        xl8[valid] = x[s['core'] * nloc + orig[valid], :]
        xpm_l.append(pm(xl8, nt).astype(BF))

    inC = []
    for ci, s in enumerate(scheds):
        eo, st_of, s_of, rank, p_of = geoms[ci]
        v = fs2g[src[s['em']][eo]]
        g2 = pack_G(v, st_of, s_of, rank, p_of, offs4, 4, Ls, int(offs4[-1]))
        inC.append(dict(g2=g2, fdp2=fdp2_l[ci], rs2n=rs2n_l[ci], isc4=isc4,
                        npad=npad_l[ci], hattn=ha_l[ci], xpm=xpm_l[ci],
                        w1x=w1x.astype(BF),
                        w2dra=w2dra.reshape(P, 256).astype(FPW),
                        w2drb=w2drb.reshape(P, 256).astype(FPW),
                        w3a=np.asarray(inputs['W3'], np.float32)[0:128].astype(BF),
                        w3b=np.asarray(inputs['W3'], np.float32)[128:196].astype(BF),
                        w4blk=w4blk.astype(BF),
                        b1=np.asarray(inputs['b1'], np.float32),
                        b2=np.asarray(inputs['b2'], np.float32),
                        b3=np.asarray(inputs['b3'], np.float32),
                        b4r=b4r))
    rC = run_bass_kernel_spmd(ncC, inC, core_ids=list(range(NCORE)))
    tC = rC.exec_time_ns or 0

    out = np.zeros((n, 1), np.float32)
    for ci, s in enumerate(scheds):
        y = rC.results[ci]['out'].reshape(nmc * 512)
        orig = s['order']
        valid = orig < nloc
        idx = np.arange(nt * P)[valid]
        out[s['core'] * nloc + orig[valid], 0] = y[idx]
    kernel.last_exec_ns = tA + tB + tC
    kernel.last_t12 = (tA, tB, tC)
    kernel.last_results = (rA, rB, rC)
    return out


# revision 22
# speedup vs baseline: 1.0641x; 1.0641x over previous
"""GATv2 x3 + MLP (nn_GAT) on trn2, 8 NeuronCores.

v5 design: attn folded into projection weights (leaky_relu is positively
homogeneous; negative attn handled by a slope-5 prelu + per-node unscale),
l-innermost edge stream so every DVE op runs in 2x mode, den-reduce on the
Pool engine, MLP tail batched (stacked 14->1 matmul + one sigmoid pass).

 - Launch A: project x -> Y|fd''|res per node (block-diag matmuls);
   Y = attn-scaled source projection, fd'' = attn-scaled dest projection.
 - Host gathers Y[src] per edge into a node-major padded-ELL stream with
   edge slots INNERMOST: [P, S, C, L].
 - Launch B: GAT layer-1: z = Y[src]+fd''[dst] (pair-trick broadcast keeps
   2x), prelu with per-sign-run alphas, score tree (4 wide strided adds),
   exp, weighted feats in-place, halving reduce over slots; den reduce on
   gpsimd.  Fixup: pad-slot den fix, normalize, unscale, residual, elu.
   Epilogue projects fs2''|fd2''|res2 on the otherwise idle PE.
 - Launch C: d2 GAT pipeline + 14->196->196->14->1 MLP.  W2 fp8 DoubleRow;
   r3 staged so the 14->1 matmul runs 8 chunks per instruction and all
   sigmoids run in one table-load at the end.

Host only reorders/replicates/casts device-computed tensor bytes; the only
host arithmetic is on the tiny weight matrices (attn folding).
"""
import sys
sys.path.insert(0, '/opt/trn_rl_repo')
import numpy as np
import ml_dtypes

import concourse.bass as bass
import concourse.mybir as mybir
from concourse import bacc
from concourse.tile import TileContext
from concourse.bass_utils import run_bass_kernel_spmd
from concourse.masks import make_identity

bf16 = mybir.dt.bfloat16
fp8 = mybir.dt.float8e4
f32 = mybir.dt.float32
BF = ml_dtypes.bfloat16
FP8 = ml_dtypes.float8_e4m3
AL = mybir.AluOpType
AF = mybir.ActivationFunctionType
MPM = mybir.MatmulPerfMode

NCORE = 8
P = 128
SUPER = 16          # tiles per supertile
NEG_GAT = 0.2
NEG_MLP = 0.01
FP8_W2 = True


# ================================================================= host prep
def build_schedule(dst, n):
    nloc = n // NCORE
    core_of = dst // nloc
    scheds = []
    for c in range(NCORE):
        em = np.where(core_of == c)[0]
        ldst = dst[em] - c * nloc
        deg = np.bincount(ldst, minlength=nloc)
        nt = -(-nloc // P)
        nt = -(-nt // SUPER) * SUPER
        degp = np.concatenate([deg, np.zeros(nt * P - nloc, np.int64)])
        order = np.argsort(-degp, kind='stable')
        pos_of = np.empty_like(order)
        pos_of[order] = np.arange(len(order))
        scheds.append(dict(core=c, em=em, ldst=ldst, deg=degp, order=order,
                           pos_of=pos_of, nt=nt, nloc=nloc))
    nt = scheds[0]['nt']
    nst = nt // SUPER
    Ls = []
    for st in range(nst):
        L = 2
        for s in scheds:
            L = max(L, int(s['deg'][s['order'][st * SUPER * P]]))
        L = -(-L // 8) * 8   # mult-8: keeps halving-tree levels in DVE 2x mode
        Ls.append(L)
    return scheds, nst, Ls


def edge_slot_geom(s, Ls):
    """Per edge (in eo order): supertile, tile-in-supertile, slot rank, row."""
    order, deg = s['order'], s['deg']
    pos_e = s['pos_of'][s['ldst']]
    eo = np.lexsort((np.arange(len(pos_e)), pos_e))
    pos_sorted = pos_e[eo]
    starts = np.concatenate([[0], np.cumsum(deg[order])])
    rank = np.arange(len(eo)) - starts[pos_sorted]
    t_of = pos_sorted // P
    st_of = t_of // SUPER
    p_of = pos_sorted % P
    return (eo, st_of.astype(np.int64), (t_of % SUPER).astype(np.int64),
            rank.astype(np.int64), p_of.astype(np.int64))


def pack_G(vals_bf, st_of, s_of, rank, p_of, offsC, C, Ls, totc):
    """l-innermost: col = offs[st] + s*(C*L) + c*L + rank."""
    buf = np.zeros((P, totc), BF)
    L_e = np.asarray(Ls)[st_of]
    base = np.asarray(offsC)[st_of] + s_of * (C * L_e) + rank
    for c in range(C):
        buf[p_of, base + c * L_e] = vals_bf[:, c]
    return buf


def make_npad(s, Ls, nt):
    L_t = np.repeat(np.asarray(Ls, np.int64), SUPER)
    d = s['deg'][s['order']].reshape(nt, P)
    return np.ascontiguousarray((L_t[:, None] - d).T).astype(np.float32)


def pack_local(vals, nrow, nt):
    pk = np.zeros((8 * nrow, (nt // 8) * P), BF)
    nodes = np.arange(nt * P)
    a = (nodes // P) % 8
    col = (nodes // (8 * P)) * P + nodes % P
    v = vals.astype(BF)
    for f in range(nrow - 1):
        pk[a * nrow + f, col] = v[:, f]
    pk[a * nrow + (nrow - 1), col] = BF(1.0)
    return pk


def blockdiag(w, bias, nrow, sp=16):
    bd = np.zeros((8 * nrow, 8 * sp), np.float32)
    k = w.shape[1]
    for a in range(8):
        bd[a * nrow:a * nrow + w.shape[0], a * sp:a * sp + k] = w
        bd[a * nrow + nrow - 1, a * sp:a * sp + k] = bias
    return bd.astype(BF)


def pm(vals, nt):
    d = vals.shape[1]
    return np.ascontiguousarray(
        vals.reshape(nt, P, d).transpose(1, 0, 2).reshape(P, nt * d))


def pm_pair(vals, nt):
    """[nt*P, d] -> [P, nt*d*2] with each channel duplicated (pair trick)."""
    d = vals.shape[1]
    v = vals.reshape(nt, P, d).transpose(1, 0, 2)        # [P, nt, d]
    v2 = np.repeat(v, 2, axis=2)                          # [P, nt, 2d]
    return np.ascontiguousarray(v2.reshape(P, nt * d * 2))


def attn_fold(attn_hf, H, F):
    """Per (h,f): permuted order (pos-signs first within each head),
    channel scale, prelu alpha.  Returns (perm j-list, scale, alpha)."""
    perm, scale, alpha = [], [], []
    for h in range(H):
        # alternate pos-first / neg-first per head so prelu alpha-runs merge
        # across head boundaries (fewer Act instructions)
        first_pos = (h % 2 == 0)
        fs = sorted(range(F),
                    key=lambda f: 0 if (attn_hf[h, f] > 0) == first_pos else 1)
        for f in fs:
            a = float(attn_hf[h, f])
            if a > 0:
                aa = max(a, 1e-8)
                perm.append(h * F + f); scale.append(aa); alpha.append(NEG_GAT)
            else:
                aa = min(a, -1e-8)
                perm.append(h * F + f); scale.append(NEG_GAT * aa); alpha.append(1.0 / NEG_GAT)
    return perm, np.asarray(scale, np.float64), alpha


def alpha_runs(alphas):
    runs = []
    i = 0
    while i < len(alphas):
        j = i
        while j < len(alphas) and alphas[j] == alphas[i]:
            j += 1
        runs.append((i, j, float(alphas[i])))
        i = j
    return runs


# ================================================================ device bits
def halving_tree(tt, X, L, out_final):
    """In-place halving over innermost axis of X [P,...,L]; final add -> out_final."""
    cur = L
    while cur > 2:
        h = cur // 2
        tt(out=X[..., 0:h], in0=X[..., 0:h], in1=X[..., cur - h:cur], op=AL.add)
        cur -= h
    if cur == 2:
        tt(out=out_final, in0=X[..., 0:1], in1=X[..., 1:2], op=AL.add)
    else:
        tt(out=out_final, in0=X[..., 0:1], in1=X[..., 0:1], op=AL.bypass)


def pp_tree(nc, A, B, L, out_final, cp=None):
    """Ping-pong halving reduce over innermost axis: A [P,..,L] (input, even L),
    B [P,..,>=L/2] scratch.  Output buffer alternates so out never shares a
    buffer with an input (keeps the DVE 2x mode).  cp: engine copy fn for the
    odd middle element (defaults to vector tensor_copy)."""
    cp = cp or nc.vector.tensor_copy
    tt = nc.vector.tensor_tensor
    src, dst = A, B
    cur = L
    while cur > 2:
        h = cur // 2
        tt(out=dst[..., 0:h], in0=src[..., 0:h], in1=src[..., h:2 * h], op=AL.add)
        if cur & 1:
            cp(out=dst[..., h:h + 1], in_=src[..., 2 * h:2 * h + 1])
            cur = h + 1
        else:
            cur = h
        src, dst = dst, src
    if cur == 2:
        tt(out=out_final, in0=src[..., 0:1], in1=src[..., 1:2], op=AL.add)
    else:
        cp(out=out_final, in_=src[..., 0:1])


def emit_elu(nc, sbS, hflat, nelem, tag):
    tmp = sbS.tile([P, nelem], f32, tag=tag)
    nc.vector.tensor_scalar_min(out=tmp[:], in0=hflat, scalar1=0.0)
    nc.scalar.activation(out=tmp[:], in_=tmp[:], func=AF.Exp)
    nc.vector.tensor_scalar(out=hflat, in0=hflat, scalar1=0.0, scalar2=-1.0,
                            op0=AL.max, op1=AL.add)
    nc.vector.tensor_tensor(out=hflat, in0=hflat, in1=tmp[:], op=AL.add)


def emit_gat_st_B(nc, sbG, sbZ, sbEX, sbT, d_g, off, L, S, fdp_ap, runs,
                  h1v, denv):
    """Layer-1 supertile: C=12 channels [d1h0(5), d1h1(5), a1h0, a1h1]."""
    C = 12
    L2 = L // 2
    ncols = S * C * L
    G = sbG.tile([P, ncols], bf16, tag="G")
    nc.sync.dma_start(out=G[:], in_=d_g[:, off:off + ncols])
    G4 = G[:].rearrange("p (s c l) -> p s c l", s=S, c=C, l=L)
    G5 = G[:].rearrange("p (sc l2 j) -> p sc l2 j", sc=S * C, l2=L2, j=2)
    Z = sbZ.tile([P, ncols], bf16, tag="Z")
    Z4 = Z[:].rearrange("p (s c l) -> p s c l", s=S, c=C, l=L)
    Z5 = Z[:].rearrange("p (sc l2 j) -> p sc l2 j", sc=S * C, l2=L2, j=2)
    fdb = fdp_ap.rearrange("p s c j -> p (s c) j").unsqueeze(2) \
        .broadcast_to([P, S * C, L2, 2])
    nc.vector.tensor_tensor(out=Z5, in0=G5, in1=fdb, op=AL.add)
    for (c0, c1, al) in runs:
        nc.scalar.activation(out=Z4[:, :, c0:c1, :], in_=Z4[:, :, c0:c1, :],
                             func=AF.Prelu, alpha=al)
    # score tree: d1 heads = sum of 5 channels each
    T = sbT.tile([P, S * 4 * L], bf16, tag="T")
    T4 = T[:].rearrange("p (s c l) -> p s c l", s=S, c=4, l=L)
    Tp = T[:].rearrange("p (s c2 c l) -> p s c2 c l", s=S, c2=2, c=2, l=L)
    nc.vector.tensor_tensor(out=T4[:, :, 0:2, :], in0=Z4[:, :, 0:2, :],
                            in1=Z4[:, :, 2:4, :], op=AL.add)
    nc.vector.tensor_tensor(out=T4[:, :, 2:4, :], in0=Z4[:, :, 5:7, :],
                            in1=Z4[:, :, 7:9, :], op=AL.add)
    SC = sbT.tile([P, S * 2 * L], bf16, tag="SC")
    SC3 = SC[:].rearrange("p (s c l) -> p s c l", s=S, c=2, l=L)
    nc.vector.tensor_tensor(out=SC3, in0=Tp[:, :, :, 0, :],
                            in1=Tp[:, :, :, 1, :], op=AL.add)
    Zh = Z4[:, :, 0:10, :].rearrange("p s (h f) l -> p s h f l", h=2, f=5)
    nc.vector.tensor_tensor(out=SC3, in0=SC3, in1=Zh[:, :, :, 4, :], op=AL.add)
    EX = sbEX.tile([P, S * 4 * L], bf16, tag="EX")
    EX4 = EX[:].rearrange("p (s c l) -> p s c l", s=S, c=4, l=L)
    nc.scalar.activation(out=EX4[:, :, 0:2, :], in_=SC3, func=AF.Exp)
    nc.scalar.activation(out=EX4[:, :, 2:4, :], in_=Z4[:, :, 10:12, :], func=AF.Exp)
    # weighted feats in-place on G (one op per d1 head: <=3 free dims)
    for h in range(2):
        Gh = G4[:, :, 5 * h:5 * h + 5, :]
        exd = EX4[:, :, h:h + 1, :].broadcast_to([P, S, 5, L])
        nc.vector.tensor_tensor(out=Gh, in0=Gh, in1=exd, op=AL.mult)
    nc.vector.tensor_tensor(out=G4[:, :, 10:12, :], in0=G4[:, :, 10:12, :],
                            in1=EX4[:, :, 2:4, :], op=AL.mult)
    # ping-pong partners = Z / T (both dead after score tree / exp)
    pp_tree(nc, G4, Z4[:, :, :, 0:L // 2 + 1], L, h1v.unsqueeze(3))
    pp_tree(nc, EX4, T4[:, :, :, 0:L // 2 + 1], L, denv.unsqueeze(3))


def emit_gat_st_C(nc, sbG, sbZ, sbEX, sbT, d_g, off, L, S, fdp_ap, runs,
                  h2v, denv):
    """Layer-2 supertile: C=4 channels [d2h0(2), d2h1(2)]."""
    C = 4
    L2 = L // 2
    ncols = S * C * L
    G = sbG.tile([P, ncols], bf16, tag="G")
    nc.sync.dma_start(out=G[:], in_=d_g[:, off:off + ncols])
    G4 = G[:].rearrange("p (s c l) -> p s c l", s=S, c=C, l=L)
    G5 = G[:].rearrange("p (sc l2 j) -> p sc l2 j", sc=S * C, l2=L2, j=2)
    Z = sbZ.tile([P, ncols], bf16, tag="Z")
    Z4 = Z[:].rearrange("p (s c l) -> p s c l", s=S, c=C, l=L)
    Z5 = Z[:].rearrange("p (sc l2 j) -> p sc l2 j", sc=S * C, l2=L2, j=2)
    fdb = fdp_ap.rearrange("p s c j -> p (s c) j").unsqueeze(2) \
        .broadcast_to([P, S * C, L2, 2])
    nc.vector.tensor_tensor(out=Z5, in0=G5, in1=fdb, op=AL.add)
    for (c0, c1, al) in runs:
        nc.scalar.activation(out=Z4[:, :, c0:c1, :], in_=Z4[:, :, c0:c1, :],
                             func=AF.Prelu, alpha=al)
    Zp = Z4.rearrange("p s (h f) l -> p s h f l", h=2, f=2)
    SC = sbT.tile([P, S * 2 * L], bf16, tag="SC")
    SC3 = SC[:].rearrange("p (s c l) -> p s c l", s=S, c=2, l=L)
    nc.vector.tensor_tensor(out=SC3, in0=Zp[:, :, :, 0, :],
                            in1=Zp[:, :, :, 1, :], op=AL.add)
    EX = sbEX.tile([P, S * 2 * L], bf16, tag="EX")
    EX3 = EX[:].rearrange("p (s c l) -> p s c l", s=S, c=2, l=L)
    nc.scalar.activation(out=EX3, in_=SC3, func=AF.Exp)
    for h in range(2):
        Gh = G4[:, :, 2 * h:2 * h + 2, :]
        exd = EX3[:, :, h:h + 1, :].broadcast_to([P, S, 2, L])
        nc.vector.tensor_tensor(out=Gh, in0=Gh, in1=exd, op=AL.mult)
    # ping-pong partners = Z / SC (both dead after score tree)
    pp_tree(nc, G4, Z4[:, :, :, 0:L // 2 + 1], L, h2v.unsqueeze(3))
    pp_tree(nc, EX3, SC3[:, :, :, 0:L // 2 + 1], L, denv.unsqueeze(3))


def emit_fixup(nc, sbS, hv, hflat, dv, dflat, fd_ap, rs_flat, np_ap, isc_ap,
               T, C, NH, dF, runs, do_elu=True):
    """Pad-slot den fix + normalize + unscale + residual + elu over T tiles.
    fd_ap: [P, T, C] per-node scaled dest proj; isc_ap: [P, C] inv scales."""
    zp = sbS.tile([P, T * C], bf16, tag="zp")
    zp3 = zp[:].rearrange("p (t c) -> p t c", t=T, c=C)
    for (c0, c1, al) in runs:
        nc.scalar.activation(out=zp3[:, :, c0:c1], in_=fd_ap[:, :, c0:c1],
                             func=AF.Prelu, alpha=al)
    ep = sbS.tile([P, T * NH], bf16, tag="ep")
    ep3 = ep[:].rearrange("p (t h) -> p t h", t=T, h=NH)
    if C == 12:
        zph = zp3[:, :, 0:10].rearrange("p t (h f) -> p t h f", h=2, f=5)
        tp = sbS.tile([P, T * 2], bf16, tag="tp")
        tp3 = tp[:].rearrange("p (t h) -> p t h", t=T, h=2)
        nc.vector.tensor_tensor(out=tp3, in0=zph[:, :, :, 0], in1=zph[:, :, :, 1], op=AL.add)
        nc.vector.tensor_tensor(out=tp3, in0=tp3, in1=zph[:, :, :, 2], op=AL.add)
        nc.vector.tensor_tensor(out=tp3, in0=tp3, in1=zph[:, :, :, 3], op=AL.add)
        nc.vector.tensor_tensor(out=tp3, in0=tp3, in1=zph[:, :, :, 4], op=AL.add)
        nc.scalar.activation(out=ep3[:, :, 0:2], in_=tp3, func=AF.Exp)
        nc.scalar.activation(out=ep3[:, :, 2:4], in_=zp3[:, :, 10:12], func=AF.Exp)
    else:
        zph = zp3.rearrange("p t (h f) -> p t h f", h=2, f=2)
        tp = sbS.tile([P, T * 2], bf16, tag="tp")
        tp3 = tp[:].rearrange("p (t h) -> p t h", t=T, h=2)
        nc.vector.tensor_tensor(out=tp3, in0=zph[:, :, :, 0], in1=zph[:, :, :, 1], op=AL.add)
        nc.scalar.activation(out=ep3, in_=tp3, func=AF.Exp)
    padm = sbS.tile([P, T * NH], f32, tag="padm")
    pm3 = padm[:].rearrange("p (t h) -> p t h", t=T, h=NH)
    npb = np_ap.unsqueeze(2).broadcast_to([P, T, NH])
    nc.vector.tensor_tensor(out=pm3, in0=ep3, in1=npb, op=AL.mult)
    nc.vector.tensor_tensor(out=dv, in0=dv, in1=pm3, op=AL.subtract)
    nc.vector.tensor_scalar_max(out=dflat, in0=dflat, scalar1=1e-30)
    rec = sbS.tile([P, T * NH], f32, tag="rec")
    nc.vector.reciprocal(out=rec[:], in_=dflat)
    rec3 = rec[:].rearrange("p (t h) -> p t h", t=T, h=NH)
    # rec12 = rec[h(c)] * inv_scale_c
    rc = sbS.tile([P, T * C], f32, tag="rc")
    rc3 = rc[:].rearrange("p (t c) -> p t c", t=T, c=C)
    iscb = isc_ap.unsqueeze(1).broadcast_to([P, T, C])
    if C == 12:
        rch = rc3[:, :, 0:10].rearrange("p t (h f) -> p t h f", h=2, f=5)
        rb = rec3[:, :, 0:2].unsqueeze(3).broadcast_to([P, T, 2, 5])
        ib = iscb[:, :, 0:10].rearrange("p t (h f) -> p t h f", h=2, f=5)
        nc.vector.tensor_tensor(out=rch, in0=rb, in1=ib, op=AL.mult)
        nc.vector.tensor_tensor(out=rc3[:, :, 10:12], in0=rec3[:, :, 2:4],
                                in1=iscb[:, :, 10:12], op=AL.mult)
    else:
        rch = rc3.rearrange("p t (h f) -> p t h f", h=2, f=2)
        rb = rec3.unsqueeze(3).broadcast_to([P, T, 2, 2])
        ib = iscb.rearrange("p t (h f) -> p t h f", h=2, f=2)
        nc.vector.tensor_tensor(out=rch, in0=rb, in1=ib, op=AL.mult)
    nc.vector.tensor_tensor(out=hflat, in0=hflat, in1=rc[:], op=AL.mult)
    nc.vector.tensor_tensor(out=hflat, in0=hflat, in1=rs_flat, op=AL.add)
    if do_elu:
        emit_elu(nc, sbS, hflat, T * C, "elu")


# =============================================================== launch A
def build_launchA(nt):
    cols = nt * 16
    nc = bacc.Bacc("TRN2", target_bir_lowering=False, debug=False, num_devices=NCORE)
    d_x = nc.dram_tensor("x5l", [48, cols], bf16, kind="ExternalInput")
    d_bfs = nc.dram_tensor("bd_fs", [48, P], bf16, kind="ExternalInput")
    d_bfd = nc.dram_tensor("bd_fd", [48, P], bf16, kind="ExternalInput")
    d_brs = nc.dram_tensor("bd_rs", [48, P], bf16, kind="ExternalInput")
    d_fs = nc.dram_tensor("fs1cm", [P, cols], bf16, kind="ExternalOutput")
    d_fd = nc.dram_tensor("fd1cm", [P, cols], bf16, kind="ExternalOutput")
    d_rs = nc.dram_tensor("rs1cm", [P, cols], f32, kind="ExternalOutput")
    with TileContext(nc) as tc:
        with tc.tile_pool(name="res", bufs=1) as res, \
             tc.tile_pool(name="ps", bufs=2, space="PSUM") as ps:
            stg = res.tile([48, cols], bf16)
            nc.sync.dma_start(out=stg[:], in_=d_x[:, :])
            bfs = res.tile([48, P], bf16, tag="bfs")
            nc.sync.dma_start(out=bfs[:], in_=d_bfs[:, :])
            bfd = res.tile([48, P], bf16, tag="bfd")
            nc.sync.dma_start(out=bfd[:], in_=d_bfd[:, :])
            brs = res.tile([48, P], bf16, tag="brs")
            nc.sync.dma_start(out=brs[:], in_=d_brs[:, :])
            ofs = res.tile([P, cols], bf16, tag="ofs")
            ofd = res.tile([P, cols], bf16, tag="ofd")
            ors = res.tile([P, cols], f32, tag="ors")
            k = 0
            for j0 in range(0, cols, 512):
                w = min(512, cols - j0)
                for bd, ot in ((bfs, ofs), (bfd, ofd), (brs, ors)):
                    pmm = ps.tile([P, 512], f32, tag="mm")
                    nc.tensor.matmul(out=pmm[:, :w], lhsT=bd[:], rhs=stg[:, j0:j0 + w],
                                     start=True, stop=True)
                    if k % 2 == 0:
                        nc.vector.tensor_copy(out=ot[:, j0:j0 + w], in_=pmm[:, :w])
                    else:
                        nc.scalar.copy(out=ot[:, j0:j0 + w], in_=pmm[:, :w])
                    k += 1
            nc.sync.dma_start(out=d_fs[:, :], in_=ofs[:])
            nc.sync.dma_start(out=d_fd[:, :], in_=ofd[:])
            nc.sync.dma_start(out=d_rs[:, :], in_=ors[:])
    nc.compile()
    return nc


# =============================================================== launch B
def build_launchB(nst, Ls, offs12, nt, runs12):
    totc = int(offs12[-1])
    fgw = -(-nst // 4) * SUPER * P          # f2 output column width
    nc = bacc.Bacc("TRN2", target_bir_lowering=False, debug=False, num_devices=NCORE)
    d_g = nc.dram_tensor("g1", [P, totc], bf16, kind="ExternalInput")
    d_fdp = nc.dram_tensor("fdp1", [P, nt * 24], bf16, kind="ExternalInput")
    d_rs = nc.dram_tensor("rs1n", [P, nt * 12], f32, kind="ExternalInput")
    d_np = nc.dram_tensor("npad", [P, nt], f32, kind="ExternalInput")
    d_isc = nc.dram_tensor("isc12", [P, 12], f32, kind="ExternalInput")
    d_w2 = nc.dram_tensor("w2all", [10, 12], bf16, kind="ExternalInput")
    d_bc2 = nc.dram_tensor("bc2", [12], f32, kind="ExternalInput")
    d_f2 = nc.dram_tensor("f2cm", [48, fgw], f32, kind="ExternalOutput")
    d_ha = nc.dram_tensor("hattn", [P, nt * 2], bf16, kind="ExternalOutput")
    groups = [(0, 5), (5, 9), (9, nst)] if nst >= 9 else [(0, nst)]
    with TileContext(nc) as tc:
        with tc.tile_pool(name="res", bufs=1) as res, \
             tc.tile_pool(name="sbG", bufs=2) as sbG, \
             tc.tile_pool(name="sbZ", bufs=2) as sbZ, \
             tc.tile_pool(name="sbEX", bufs=3) as sbEX, \
             tc.tile_pool(name="sbT", bufs=2) as sbT, \
             tc.tile_pool(name="sbS", bufs=1) as sbS, \
             tc.tile_pool(name="sbT2", bufs=1) as sbT2, \
             tc.tile_pool(name="psT", bufs=1, space="PSUM") as psT, \
             tc.tile_pool(name="psF", bufs=1, space="PSUM") as psF:
            ident = res.tile([P, P], bf16)
            make_identity(nc, ident[:])
            npad = res.tile([P, nt], f32)
            nc.sync.dma_start(out=npad[:], in_=d_np[:, :])
            isc = res.tile([P, 12], f32)
            nc.sync.dma_start(out=isc[:], in_=d_isc[:, :])
            fdp = res.tile([P, nt * 24], bf16)
            nc.sync.dma_start(out=fdp[:], in_=d_fdp[:, :])
            rst = res.tile([P, nt * 12], f32)
            nc.sync.dma_start(out=rst[:], in_=d_rs[:, :])
            w2t = res.tile([10, 12], bf16, tag="w2t")
            nc.sync.dma_start(out=w2t[:], in_=d_w2[:, :])
            bc2 = res.tile([12, 1], f32, tag="bc2")
            nc.sync.dma_start(out=bc2[:], in_=d_bc2[:, None])
            h1 = res.tile([P, nt * 12], f32)
            den = res.tile([P, nt * 4], f32)
            hat = res.tile([P, nt * 2], bf16)
            h1v_all = h1[:].rearrange("p (t c) -> p t c", t=nt, c=12)
            denv_all = den[:].rearrange("p (t c) -> p t c", t=nt, c=4)
            fdp_all = fdp[:].rearrange("p (t c j) -> p t c j", t=nt, c=12, j=2)
            npv_all = npad[:].rearrange("p (t o) -> p t o", t=nt, o=1)
            for g0, g1 in groups:
                for st in range(g0, g1):
                    L = Ls[st]
                    t0 = st * SUPER
                    emit_gat_st_B(nc, sbG, sbZ, sbEX, sbT, d_g, int(offs12[st]),
                                  L, SUPER, fdp_all[:, t0:t0 + SUPER],
                                  runs12,
                                  h1v_all[:, t0:t0 + SUPER, :],
                                  denv_all[:, t0:t0 + SUPER, :])
                T = (g1 - g0) * SUPER
                t0 = g0 * SUPER
                emit_fixup(nc, sbS, h1v_all[:, t0:t0 + T, :],
                           h1[:, t0 * 12:(t0 + T) * 12],
                           denv_all[:, t0:t0 + T, :],
                           den[:, t0 * 4:(t0 + T) * 4],
                           fdp_all[:, t0:t0 + T, :, 0],
                           rst[:, t0 * 12:(t0 + T) * 12],
                           npv_all[:, t0:t0 + T, 0], isc[:], T, 12, 4, 5, runs12)
                hv = h1v_all[:, t0:t0 + T, :]
                nc.scalar.copy(out=hat[:].rearrange("p (t c) -> p t c", t=nt, c=2)[:, t0:t0 + T, :],
                               in_=hv[:, :, 10:12])
                h1b = sbS.tile([P, T * 10], bf16, tag="h1b")
                nc.scalar.copy(out=h1b[:].rearrange("p (t c) -> p t c", t=T, c=10),
                               in_=hv[:, :, 0:10])
                # epilogue: transpose h_def1 per tile, project fs2''|fd2''|res2
                for st in range(g0, g1):
                    pT = psT.tile([10, SUPER * P], bf16, tag="pT")
                    for b in range(SUPER):
                        trel = (st - g0) * SUPER + b
                        nc.tensor.transpose(out=pT[:, b * P:(b + 1) * P],
                                            in_=h1b[:, trel * 10:trel * 10 + 10],
                                            identity=ident[:])
                    hT = sbT2.tile([10, SUPER * P], bf16, tag="hT")
                    nc.scalar.copy(out=hT[:], in_=pT[:])
                    pF = psF.tile([12, SUPER * P], f32, tag="pF")
                    for q in range(SUPER * P // 512):
                        nc.tensor.matmul(out=pF[:, q * 512:(q + 1) * 512], lhsT=w2t[:],
                                         rhs=hT[:, q * 512:(q + 1) * 512],
                                         start=True, stop=True)
                    f2s = sbT2.tile([12, SUPER * P], f32, tag="f2s")
                    nc.scalar.activation(out=f2s[:], in_=pF[:], func=AF.Prelu,
                                         alpha=1.0, bias=bc2[:])
                    nc.sync.dma_start(
                        out=d_f2[12 * (st % 4):12 * (st % 4) + 12,
                                 (st // 4) * SUPER * P:(st // 4 + 1) * SUPER * P],
                        in_=f2s[:])
            nc.sync.dma_start(out=d_ha[:, :], in_=hat[:])
    nc.compile()
    return nc


# =============================================================== launch C
def build_launchC(nst, Ls, offs4, nt, nmc, runs4):
    totc = int(offs4[-1])
    ngrp = -(-nmc // 4)
    nc = bacc.Bacc("TRN2", target_bir_lowering=False, debug=False, num_devices=NCORE)
    d_g = nc.dram_tensor("g2", [P, totc], bf16, kind="ExternalInput")
    d_fdp = nc.dram_tensor("fdp2", [P, nt * 8], bf16, kind="ExternalInput")
    d_rs = nc.dram_tensor("rs2n", [P, nt * 4], f32, kind="ExternalInput")
    d_np = nc.dram_tensor("npad", [P, nt], f32, kind="ExternalInput")
    d_isc = nc.dram_tensor("isc4", [P, 4], f32, kind="ExternalInput")
    d_ha = nc.dram_tensor("hattn", [P, nt * 2], bf16, kind="ExternalInput")
    d_x = nc.dram_tensor("xpm", [P, nt * 8], bf16, kind="ExternalInput")
    d_w1 = nc.dram_tensor("w1x", [32, 196], bf16, kind="ExternalInput")
    w2dt = fp8 if FP8_W2 else bf16
    d_w2a = nc.dram_tensor("w2dra", [P, 2 * 128], w2dt, kind="ExternalInput")
    d_w2b = nc.dram_tensor("w2drb", [P, 2 * 128], w2dt, kind="ExternalInput")
    d_w3a = nc.dram_tensor("w3a", [P, 14], bf16, kind="ExternalInput")
    d_w3b = nc.dram_tensor("w3b", [68, 14], bf16, kind="ExternalInput")
    d_w4 = nc.dram_tensor("w4blk", [110, 4], bf16, kind="ExternalInput")
    d_b1 = nc.dram_tensor("b1", [196], f32, kind="ExternalInput")
    d_b2 = nc.dram_tensor("b2", [196], f32, kind="ExternalInput")
    d_b3 = nc.dram_tensor("b3", [14], f32, kind="ExternalInput")
    d_b4 = nc.dram_tensor("b4r", [4], f32, kind="ExternalInput")
    d_out = nc.dram_tensor("out", [nmc, 512], f32, kind="ExternalOutput")
    groups = [(0, 3), (3, 6), (6, 9), (9, 11), (11, nst)] if nst >= 11 else [(0, nst)]
    with TileContext(nc) as tc:
        with tc.tile_pool(name="res", bufs=1) as res, \
             tc.tile_pool(name="sbG", bufs=2) as sbG, \
             tc.tile_pool(name="sbZ", bufs=2) as sbZ, \
             tc.tile_pool(name="sbEX", bufs=3) as sbEX, \
             tc.tile_pool(name="sbT", bufs=2) as sbT, \
             tc.tile_pool(name="sbS", bufs=1) as sbS, \
             tc.tile_pool(name="sbM", bufs=2) as sbM, \
             tc.tile_pool(name="psT", bufs=1, space="PSUM") as psT, \
             tc.tile_pool(name="psC", bufs=1, space="PSUM") as psC, \
             tc.tile_pool(name="psA", bufs=2, space="PSUM") as psA, \
             tc.tile_pool(name="psB", bufs=1, space="PSUM") as psB:
            ident = res.tile([P, P], bf16)
            make_identity(nc, ident[:])
            npad = res.tile([P, nt], f32)
            nc.sync.dma_start(out=npad[:], in_=d_np[:, :])
            isc = res.tile([P, 4], f32)
            nc.sync.dma_start(out=isc[:], in_=d_isc[:, :])
            fdp = res.tile([P, nt * 8], bf16)
            nc.sync.dma_start(out=fdp[:], in_=d_fdp[:, :])
            rst = res.tile([P, nt * 4], f32)
            nc.sync.dma_start(out=rst[:], in_=d_rs[:, :])
            hat = res.tile([P, nt * 2], bf16)
            nc.sync.dma_start(out=hat[:], in_=d_ha[:, :])
            xpm = res.tile([P, nt * 8], bf16)
            nc.sync.dma_start(out=xpm[:], in_=d_x[:, :])
            w1 = res.tile([32, 196], bf16, tag="w1")
            nc.sync.dma_start(out=w1[:], in_=d_w1[:, :])
            w2a = res.tile([P, 2 * 128], w2dt, tag="w2a")
            nc.sync.dma_start(out=w2a[:], in_=d_w2a[:, :])
            w2b = res.tile([P, 2 * 128], w2dt, tag="w2b")
            nc.sync.dma_start(out=w2b[:], in_=d_w2b[:, :])
            w3a = res.tile([P, 14], bf16, tag="w3a")
            nc.sync.dma_start(out=w3a[:], in_=d_w3a[:, :])
            w3b = res.tile([68, 14], bf16, tag="w3b")
            nc.sync.dma_start(out=w3b[:], in_=d_w3b[:, :])
            w4b = res.tile([110, 4], bf16, tag="w4b")
            nc.sync.dma_start(out=w4b[:], in_=d_w4[:, :])
            w2av = w2a[:].rearrange("p (k m) -> p k m", k=2, m=128)
            w2bv = w2b[:].rearrange("p (k m) -> p k m", k=2, m=128)
            b1ca = res.tile([P, 1], f32, tag="b1ca")
            nc.sync.dma_start(out=b1ca[:], in_=d_b1[0:128, None])
            b1cb = res.tile([68, 1], f32, tag="b1cb")
            nc.sync.dma_start(out=b1cb[:], in_=d_b1[128:196, None])
            b2ca = res.tile([P, 1], f32, tag="b2ca")
            nc.sync.dma_start(out=b2ca[:], in_=d_b2[0:128, None])
            b2cb = res.tile([68, 1], f32, tag="b2cb")
            nc.sync.dma_start(out=b2cb[:], in_=d_b2[128:196, None])
            b3c = res.tile([14, 1], f32, tag="b3c")
            nc.sync.dma_start(out=b3c[:], in_=d_b3[:, None])
            b4c = res.tile([4, 1], f32, tag="b4c")
            nc.sync.dma_start(out=b4c[:], in_=d_b4[:, None])
            h2 = res.tile([P, nt * 4], f32)
            den = res.tile([P, nt * 2], f32)
            m32 = res.tile([P, nt * 32], bf16)
            nc.gpsimd.memset(m32[:], 0.0)
            r3st = res.tile([110, ngrp * 512], bf16, tag="r3st")
            adt = fp8 if FP8_W2 else bf16
            r1t = [res.tile([P, 2048], adt, tag=f"r1_{i}", name=f"r1_{i}")
                   for i in range(2)]
            for t_ in r1t:
                nc.gpsimd.memset(t_[:], 0.0)
            h2v_all = h2[:].rearrange("p (t c) -> p t c", t=nt, c=4)
            denv_all = den[:].rearrange("p (t c) -> p t c", t=nt, c=2)
            fdp_all = fdp[:].rearrange("p (t c j) -> p t c j", t=nt, c=4, j=2)
            npv_all = npad[:].rearrange("p (t o) -> p t o", t=nt, o=1)
            m3_all = m32[:].rearrange("p (t c) -> p t c", t=nt, c=32)
            hav_all = hat[:].rearrange("p (t c) -> p t c", t=nt, c=2)
            xv_all = xpm[:].rearrange("p (t c) -> p t c", t=nt, c=8)
            nc.vector.tensor_copy(out=m3_all[:, :, 0:2], in_=hav_all)
            nc.vector.tensor_copy(out=m3_all[:, :, 6:14], in_=xv_all)
            for g0, g1 in groups:
                for st in range(g0, g1):
                    L = Ls[st]
                    t0 = st * SUPER
                    emit_gat_st_C(nc, sbG, sbZ, sbEX, sbT, d_g, int(offs4[st]),
                                  L, SUPER, fdp_all[:, t0:t0 + SUPER],
                                  runs4,
                                  h2v_all[:, t0:t0 + SUPER, :],
                                  denv_all[:, t0:t0 + SUPER, :])
                T = (g1 - g0) * SUPER
                t0 = g0 * SUPER
                emit_fixup(nc, sbS, h2v_all[:, t0:t0 + T, :],
                           h2[:, t0 * 4:(t0 + T) * 4],
                           denv_all[:, t0:t0 + T, :],
                           den[:, t0 * 2:(t0 + T) * 2],
                           fdp_all[:, t0:t0 + T, :, 0],
                           rst[:, t0 * 4:(t0 + T) * 4],
                           npv_all[:, t0:t0 + T, 0], isc[:], T, 4, 2, 2, runs4)
                nc.vector.tensor_copy(out=m3_all[:, t0:t0 + T, 2:6],
                                      in_=h2v_all[:, t0:t0 + T, :])
                # MLP over this group's chunks, processed in pairs so each
                # activation instruction covers 1024 nodes (half the Act
                # instruction count)
                mc1 = min((t0 + T) // 4, nmc)
                for mcp in range(t0 // 4, mc1, 2):
                    pair = [mc for mc in (mcp, mcp + 1) if mc < mc1]
                    npair = len(pair)
                    p1a = psA.tile([P, 1024], f32, tag="pA")
                    p1b = psB.tile([P, 1024], f32, tag="pB")
                    for ci_, mc in enumerate(pair):
                        pT = psT.tile([64, 256], bf16, tag="pT")
                        nc.tensor.transpose(out=pT[:, 0:128],
                                            in_=m32[:, (mc * 4) * 32:(mc * 4 + 2) * 32],
                                            identity=ident[:])
                        nc.tensor.transpose(out=pT[:, 128:256],
                                            in_=m32[:, (mc * 4 + 2) * 32:(mc * 4 + 4) * 32],
                                            identity=ident[:])
                        r0 = sbM.tile([32, 512], bf16, tag=f"r0_{ci_}")
                        nc.vector.tensor_copy(out=r0[:, 0:128], in_=pT[0:32, 0:128])
                        nc.vector.tensor_copy(out=r0[:, 128:256], in_=pT[32:64, 0:128])
                        nc.vector.tensor_copy(out=r0[:, 256:384], in_=pT[0:32, 128:256])
                        nc.vector.tensor_copy(out=r0[:, 384:512], in_=pT[32:64, 128:256])
                        nc.tensor.matmul(out=p1a[:, ci_ * 512:(ci_ + 1) * 512],
                                         lhsT=w1[:, 0:128], rhs=r0[:], start=True, stop=True)
                        nc.tensor.matmul(out=p1b[0:68, ci_ * 512:(ci_ + 1) * 512],
                                         lhsT=w1[:, 128:196], rhs=r0[:], start=True, stop=True)
                    r1p = r1t[(mcp // 2) % 2]          # [P, 2048] fp8: cols (k, chunk, 512)
                    w = npair * 512
                    nc.scalar.activation(out=r1p[:, 0:w], in_=p1a[:, 0:w],
                                         func=AF.Prelu, alpha=NEG_MLP, bias=b1ca[:])
                    nc.scalar.activation(out=r1p[0:68, 1024:1024 + w], in_=p1b[0:68, 0:w],
                                         func=AF.Prelu, alpha=NEG_MLP, bias=b1cb[:])
                    r1v = r1p[:].rearrange("p (k c n) -> p k c n", k=2, c=2, n=512)
                    p2a = psA.tile([P, 1024], f32, tag="pA")
                    p2b = psB.tile([P, 1024], f32, tag="pB")
                    for ci_, mc in enumerate(pair):
                        nc.tensor.matmul(out=p2a[:, ci_ * 512:(ci_ + 1) * 512], lhsT=w2av,
                                         rhs=r1v[:, :, ci_, :],
                                         start=True, stop=True, perf_mode=MPM.DoubleRow)
                        nc.tensor.matmul(out=p2b[:, ci_ * 512:(ci_ + 1) * 512], lhsT=w2bv,
                                         rhs=r1v[:, :, ci_, :],
                                         start=True, stop=True, perf_mode=MPM.DoubleRow)
                    r2a = sbM.tile([P, 1024], bf16, tag="r2a")
                    nc.scalar.activation(out=r2a[:, 0:w], in_=p2a[:, 0:w], func=AF.Prelu,
                                         alpha=NEG_MLP, bias=b2ca[:])
                    r2b = sbM.tile([68, 1024], bf16, tag="r2b")
                    nc.scalar.activation(out=r2b[:, 0:w], in_=p2b[0:68, 0:w], func=AF.Prelu,
                                         alpha=NEG_MLP, bias=b2cb[:])
                    for ci_, mc in enumerate(pair):
                        p3 = psC.tile([14, 512], f32, tag="p3")
                        nc.tensor.matmul(out=p3[:], lhsT=w3a[:],
                                         rhs=r2a[:, ci_ * 512:(ci_ + 1) * 512], start=True, stop=False)
                        nc.tensor.matmul(out=p3[:], lhsT=w3b[:],
                                         rhs=r2b[0:68, ci_ * 512:(ci_ + 1) * 512], start=False, stop=True)
                        ro = 32 * (mc % 4)
                        nc.scalar.activation(out=r3st[ro:ro + 14, (mc // 4) * 512:(mc // 4 + 1) * 512],
                                             in_=p3[:], func=AF.Prelu,
                                             alpha=NEG_MLP, bias=b3c[:])
            # tail: stacked 14->1 matmuls + sigmoids (one act-table switch)
            for g in range(ngrp):
                k = min(4, nmc - 4 * g)
                kp = 32 * (k - 1) + 14
                po = psC.tile([14, 512], f32, tag="p3")
                nc.tensor.matmul(out=po[0:k, 0:512], lhsT=w4b[0:kp, 0:k],
                                 rhs=r3st[0:kp, g * 512:(g + 1) * 512],
                                 start=True, stop=True)
                sg = sbM.tile([4, 512], f32, tag="sg")
                nc.scalar.activation(out=sg[0:k, :], in_=po[0:k, 0:512], func=AF.Sigmoid,
                                     bias=b4c[0:k, :])
                nc.sync.dma_start(out=d_out[4 * g:4 * g + k, :], in_=sg[0:k, :])
    nc.compile()
    return nc


# ================================================================== kernel
_cache = {}


def kernel(**inputs):
    x = np.asarray(inputs['x'], np.float32)
    src = np.asarray(inputs['src'], np.int32)
    dst = np.asarray(inputs['dst'], np.int32)
    n = x.shape[0]

    scheds, nst, Ls = build_schedule(dst, n)
    nt = scheds[0]['nt']
    nloc = scheds[0]['nloc']
    nmc = -(-nloc // 512)
    offs12 = np.concatenate([[0], np.cumsum([SUPER * L * 12 for L in Ls])]).astype(np.int64)
    offs4 = np.concatenate([[0], np.cumsum([SUPER * L * 4 for L in Ls])]).astype(np.int64)

    # ---- layer-1 attn folding: channels [d1h0(5), d1h1(5), a1h0, a1h1]
    d1_attn = np.asarray(inputs['d1_attn'], np.float64)     # [2, 5]
    a1_attn = np.asarray(inputs['a1_attn'], np.float64)     # [2, 1]
    perm_d1, scale_d1, alpha_d1 = attn_fold(d1_attn, 2, 5)
    perm_a1, scale_a1, alpha_a1 = attn_fold(a1_attn[:, :], 2, 1)
    scale12 = np.concatenate([scale_d1, scale_a1])
    alpha12 = alpha_d1 + alpha_a1
    runs12 = alpha_runs(alpha12)

    def l1_pack(a1_w, d1_w, scale=None):
        w = np.zeros((a1_w.shape[0], 12), np.float64)
        for p_, j in enumerate(perm_d1):
            w[:, p_] = d1_w[:, j]
        for p_, j in enumerate(perm_a1):
            w[:, 10 + p_] = a1_w[:, j]
        if scale is not None:
            w = w * scale[None, :]
        return w

    a1_Wsrc = np.asarray(inputs['a1_Wsrc'], np.float64)
    d1_Wsrc = np.asarray(inputs['d1_Wsrc'], np.float64)
    a1_Wdst = np.asarray(inputs['a1_Wdst'], np.float64)
    d1_Wdst = np.asarray(inputs['d1_Wdst'], np.float64)
    a1_Wres = np.asarray(inputs['a1_Wres'], np.float64)
    d1_Wres = np.asarray(inputs['d1_Wres'], np.float64)
    bY = l1_pack(np.asarray(inputs['a1_bsrc'], np.float64)[None, :],
                 np.asarray(inputs['d1_bsrc'], np.float64)[None, :], scale12)[0]
    bD = l1_pack(np.asarray(inputs['a1_bdst'], np.float64)[None, :],
                 np.asarray(inputs['d1_bdst'], np.float64)[None, :], scale12)[0]
    bR = l1_pack(np.asarray(inputs['a1_bias'], np.float64)[None, :],
                 np.asarray(inputs['d1_bias'], np.float64)[None, :])[0]
    bd_fs = blockdiag(l1_pack(a1_Wsrc, d1_Wsrc, scale12).astype(np.float32), bY.astype(np.float32), 6)
    bd_fd = blockdiag(l1_pack(a1_Wdst, d1_Wdst, scale12).astype(np.float32), bD.astype(np.float32), 6)
    bd_rs = blockdiag(l1_pack(a1_Wres, d1_Wres).astype(np.float32), bR.astype(np.float32), 6)
    isc12 = np.tile((1.0 / scale12).astype(np.float32), (P, 1))

    # ---- layer-2 attn folding: channels [d2h0(2), d2h1(2)]
    d2_attn = np.asarray(inputs['d2_attn'], np.float64)     # [2, 2]
    perm_d2, scale4, alpha4 = attn_fold(d2_attn, 2, 2)
    runs4 = alpha_runs(alpha4)

    def d2w(name):
        w = np.asarray(inputs[name], np.float64)            # [10, 4] native cols j=2h+f
        out = np.zeros((10, 4), np.float64)
        for p_, j in enumerate(perm_d2):
            out[:, p_] = w[:, j]
        return out

    def d2b(name):
        b = np.asarray(inputs[name], np.float64)
        return b[perm_d2]

    # rows of the [10, 12] projection are h_def1 in MY permuted order
    rowperm = perm_d1                                       # position -> native j=5h+f
    ws2 = d2w('d2_Wsrc')[rowperm] * scale4[None, :]
    wd2 = d2w('d2_Wdst')[rowperm] * scale4[None, :]
    wr2 = d2w('d2_Wres')[rowperm]
    w2all = np.concatenate([ws2, wd2, wr2], axis=1).astype(np.float32)
    bc2 = np.concatenate([d2b('d2_bsrc') * scale4, d2b('d2_bdst') * scale4,
                          d2b('d2_bias')]).astype(np.float32)
    isc4 = np.tile((1.0 / scale4).astype(np.float32), (P, 1))

    # ---- MLP weights: W1 rows 2:6 permuted to h_def2 order
    w1p = np.asarray(inputs['W1'], np.float64).copy()
    W1n = np.asarray(inputs['W1'], np.float64)
    for p_, j in enumerate(perm_d2):
        w1p[2 + p_] = W1n[2 + j]
    w1x = np.zeros((32, 196), np.float32)
    w1x[0:14] = w1p.astype(np.float32)
    W2 = np.asarray(inputs['W2'], np.float32)
    w2dra = np.zeros((P, 2, 128), np.float32)
    w2dra[:, 0, :] = W2[0:128, 0:128]
    w2dra[0:68, 1, :] = W2[128:196, 0:128]
    w2drb = np.zeros((P, 2, 128), np.float32)
    w2drb[:, 0, 0:68] = W2[0:128, 128:196]
    w2drb[0:68, 1, 0:68] = W2[128:196, 128:196]
    FPW = FP8 if FP8_W2 else BF
    w4 = np.asarray(inputs['W4'], np.float32)               # [14, 1]
    w4blk = np.zeros((110, 4), np.float32)
    for k in range(4):
        w4blk[32 * k:32 * k + 14, k] = w4[:, 0]
    b4r = np.full(4, float(np.asarray(inputs['b4'])[0]), np.float32)

    key = (n, len(src), nst, tuple(Ls), tuple(runs12), tuple(runs4))
    if key not in _cache:
        _cache.clear()
        _cache[key] = (build_launchA(nt), build_launchB(nst, Ls, offs12, nt, runs12),
                       build_launchC(nst, Ls, offs4, nt, nmc, runs4))
    ncA, ncB, ncC = _cache[key]

    # ---------------- launch A: per-node projections of x
    inA = []
    for s in scheds:
        orig = s['order']
        valid = orig < nloc
        xl = np.zeros((nt * P, 5), np.float32)
        xl[valid] = x[s['core'] * nloc + orig[valid], :5]
        inA.append(dict(x5l=pack_local(xl, 6, nt), bd_fs=bd_fs, bd_fd=bd_fd, bd_rs=bd_rs))
    rA = run_bass_kernel_spmd(ncA, inA, core_ids=list(range(NCORE)))
    tA = rA.exec_time_ns or 0

    i_all = np.arange(nt * P)
    a_i = (i_all // P) % 8
    col_i = (i_all // (8 * P)) * P + i_all % P
    rows12 = a_i[:, None] * 16 + np.arange(12)[None, :]
    fs1g = np.zeros((n, 12), BF)
    geoms, fdp1_l, rs1n_l, npad_l = [], [], [], []
    for ci, s in enumerate(scheds):
        fs_sorted = rA.results[ci]['fs1cm'][rows12, col_i[:, None]]
        fd_sorted = rA.results[ci]['fd1cm'][rows12, col_i[:, None]]
        rs_sorted = rA.results[ci]['rs1cm'][rows12, col_i[:, None]]
        orig = s['order']
        valid = orig < nloc
        fs1g[s['core'] * nloc + orig[valid]] = fs_sorted[valid]
        fdp1_l.append(pm_pair(fd_sorted, nt))
        rs1n_l.append(pm(rs_sorted.astype(np.float32), nt))
        geoms.append(edge_slot_geom(s, Ls))
        npad_l.append(make_npad(s, Ls, nt))

    inB = []
    for ci, s in enumerate(scheds):
        eo, st_of, s_of, rank, p_of = geoms[ci]
        v = fs1g[src[s['em']][eo]]
        g1 = pack_G(v, st_of, s_of, rank, p_of, offs12, 12, Ls, int(offs12[-1]))
        inB.append(dict(g1=g1, fdp1=fdp1_l[ci], rs1n=rs1n_l[ci], isc12=isc12,
                        npad=npad_l[ci], w2all=w2all.astype(BF), bc2=bc2))
    rB = run_bass_kernel_spmd(ncB, inB, core_ids=list(range(NCORE)))
    tB = rB.exec_time_ns or 0

    fgw = -(-nst // 4) * SUPER * P
    fs2g = np.zeros((n, 4), BF)
    fdp2_l, rs2n_l, ha_l, xpm_l = [], [], [], []
    for ci, s in enumerate(scheds):
        fb = rB.results[ci]['f2cm']              # [48, fgw]
        f2 = np.zeros((12, nt * P), np.float32)
        for st in range(nst):
            f2[:, st * SUPER * P:(st + 1) * SUPER * P] = \
                fb[12 * (st % 4):12 * (st % 4) + 12,
                   (st // 4) * SUPER * P:(st // 4 + 1) * SUPER * P]
        orig = s['order']
        valid = orig < nloc
        fs2g[s['core'] * nloc + orig[valid]] = f2[0:4, :].T[valid].astype(BF)
        fdp2_l.append(pm_pair(f2[4:8, :].T.astype(BF), nt))
        rs2n_l.append(pm(np.ascontiguousarray(f2[8:12, :].T), nt))
        ha_l.append(rB.results[ci]['hattn'])
        xl8 = np.zeros((nt * P, 8), np.float32)
        xl8[valid] = x[s['core'] * nloc + orig[valid], :]
        xpm_l.append(pm(xl8, nt).astype(BF))

    inC = []
    for ci, s in enumerate(scheds):
        eo, st_of, s_of, rank, p_of = geoms[ci]
        v = fs2g[src[s['em']][eo]]
        g2 = pack_G(v, st_of, s_of, rank, p_of, offs4, 4, Ls, int(offs4[-1]))
        inC.append(dict(g2=g2, fdp2=fdp2_l[ci], rs2n=rs2n_l[ci], isc4=isc4,
                        npad=npad_l[ci], hattn=ha_l[ci], xpm=xpm_l[ci],
                        w1x=w1x.astype(BF),
                        w2dra=w2dra.reshape(P, 256).astype(FPW),
                        w2drb=w2drb.reshape(P, 256).astype(FPW),
                        w3a=np.asarray(inputs['W3'], np.float32)[0:128].astype(BF),
                        w3b=np.asarray(inputs['W3'], np.float32)[128:196].astype(BF),
                        w4blk=w4blk.astype(BF),
                        b1=np.asarray(inputs['b1'], np.float32),
                        b2=np.asarray(inputs['b2'], np.float32),
                        b3=np.asarray(inputs['b3'], np.float32),
                        b4r=b4r))
    rC = run_bass_kernel_spmd(ncC, inC, core_ids=list(range(NCORE)))
    tC = rC.exec_time_ns or 0

    out = np.zeros((n, 1), np.float32)
    for ci, s in enumerate(scheds):
        y = rC.results[ci]['out'].reshape(nmc * 512)
        orig = s['order']
        valid = orig < nloc
        idx = np.arange(nt * P)[valid]
        out[s['core'] * nloc + orig[valid], 0] = y[idx]
    kernel.last_exec_ns = tA + tB + tC
    kernel.last_t12 = (tA, tB, tC)
    kernel.last_results = (rA, rB, rC)
    return out


# revision 23
# speedup vs baseline: 1.0949x; 1.0289x over previous
"""GATv2 x3 + MLP (nn_GAT) on trn2, 8 NeuronCores.

v5 design: attn folded into projection weights (leaky_relu is positively
homogeneous; negative attn handled by a slope-5 prelu + per-node unscale),
l-innermost edge stream so every DVE op runs in 2x mode, den-reduce on the
Pool engine, MLP tail batched (stacked 14->1 matmul + one sigmoid pass).

 - Launch A: project x -> Y|fd''|res per node (block-diag matmuls);
   Y = attn-scaled source projection, fd'' = attn-scaled dest projection.
 - Host gathers Y[src] per edge into a node-major padded-ELL stream with
   edge slots INNERMOST: [P, S, C, L].
 - Launch B: GAT layer-1: z = Y[src]+fd''[dst] (pair-trick broadcast keeps
   2x), prelu with per-sign-run alphas, score tree (4 wide strided adds),
   exp, weighted feats in-place, halving reduce over slots; den reduce on
   gpsimd.  Fixup: pad-slot den fix, normalize, unscale, residual, elu.
   Epilogue projects fs2''|fd2''|res2 on the otherwise idle PE.
 - Launch C: d2 GAT pipeline + 14->196->196->14->1 MLP.  W2 fp8 DoubleRow;
   r3 staged so the 14->1 matmul runs 8 chunks per instruction and all
   sigmoids run in one table-load at the end.

Host only reorders/replicates/casts device-computed tensor bytes; the only
host arithmetic is on the tiny weight matrices (attn folding).
"""
import sys
sys.path.insert(0, '/opt/trn_rl_repo')
import numpy as np
import ml_dtypes

import concourse.bass as bass
import concourse.mybir as mybir
from concourse import bacc
from concourse.tile import TileContext
from concourse.bass_utils import run_bass_kernel_spmd
from concourse.masks import make_identity

bf16 = mybir.dt.bfloat16
fp8 = mybir.dt.float8e4
f32 = mybir.dt.float32
BF = ml_dtypes.bfloat16
FP8 = ml_dtypes.float8_e4m3
AL = mybir.AluOpType
AF = mybir.ActivationFunctionType
MPM = mybir.MatmulPerfMode

NCORE = 8
P = 128
SUPER = 8           # tiles per supertile
NEG_GAT = 0.2
NEG_MLP = 0.01
FP8_W2 = True


# ================================================================= host prep
def build_schedule(dst, n):
    nloc = n // NCORE
    core_of = dst // nloc
    scheds = []
    for c in range(NCORE):
        em = np.where(core_of == c)[0]
        ldst = dst[em] - c * nloc
        deg = np.bincount(ldst, minlength=nloc)
        nt = -(-nloc // P)
        nt = -(-nt // SUPER) * SUPER
        degp = np.concatenate([deg, np.zeros(nt * P - nloc, np.int64)])
        order = np.argsort(-degp, kind='stable')
        pos_of = np.empty_like(order)
        pos_of[order] = np.arange(len(order))
        scheds.append(dict(core=c, em=em, ldst=ldst, deg=degp, order=order,
                           pos_of=pos_of, nt=nt, nloc=nloc))
    nt = scheds[0]['nt']
    nst = nt // SUPER
    Ls = []
    for st in range(nst):
        L = 2
        for s in scheds:
            L = max(L, int(s['deg'][s['order'][st * SUPER * P]]))
        L = -(-L // 8) * 8   # mult-8: keeps halving-tree levels in DVE 2x mode
        Ls.append(L)
    return scheds, nst, Ls


def edge_slot_geom(s, Ls):
    """Per edge (in eo order): supertile, tile-in-supertile, slot rank, row."""
    order, deg = s['order'], s['deg']
    pos_e = s['pos_of'][s['ldst']]
    eo = np.lexsort((np.arange(len(pos_e)), pos_e))
    pos_sorted = pos_e[eo]
    starts = np.concatenate([[0], np.cumsum(deg[order])])
    rank = np.arange(len(eo)) - starts[pos_sorted]
    t_of = pos_sorted // P
    st_of = t_of // SUPER
    p_of = pos_sorted % P
    return (eo, st_of.astype(np.int64), (t_of % SUPER).astype(np.int64),
            rank.astype(np.int64), p_of.astype(np.int64))


def pack_G(vals_bf, st_of, s_of, rank, p_of, offsC, C, Ls, totc):
    """l-innermost: col = offs[st] + s*(C*L) + c*L + rank."""
    buf = np.zeros((P, totc), BF)
    L_e = np.asarray(Ls)[st_of]
    base = np.asarray(offsC)[st_of] + s_of * (C * L_e) + rank
    for c in range(C):
        buf[p_of, base + c * L_e] = vals_bf[:, c]
    return buf


def make_npad(s, Ls, nt):
    L_t = np.repeat(np.asarray(Ls, np.int64), SUPER)
    d = s['deg'][s['order']].reshape(nt, P)
    return np.ascontiguousarray((L_t[:, None] - d).T).astype(np.float32)


def pack_local(vals, nrow, nt):
    pk = np.zeros((8 * nrow, (nt // 8) * P), BF)
    nodes = np.arange(nt * P)
    a = (nodes // P) % 8
    col = (nodes // (8 * P)) * P + nodes % P
    v = vals.astype(BF)
    for f in range(nrow - 1):
        pk[a * nrow + f, col] = v[:, f]
    pk[a * nrow + (nrow - 1), col] = BF(1.0)
    return pk


def blockdiag(w, bias, nrow, sp=16):
    bd = np.zeros((8 * nrow, 8 * sp), np.float32)
    k = w.shape[1]
    for a in range(8):
        bd[a * nrow:a * nrow + w.shape[0], a * sp:a * sp + k] = w
        bd[a * nrow + nrow - 1, a * sp:a * sp + k] = bias
    return bd.astype(BF)


def pm(vals, nt):
    d = vals.shape[1]
    return np.ascontiguousarray(
        vals.reshape(nt, P, d).transpose(1, 0, 2).reshape(P, nt * d))


def pm_pair(vals, nt):
    """[nt*P, d] -> [P, nt*d*2] with each channel duplicated (pair trick)."""
    d = vals.shape[1]
    v = vals.reshape(nt, P, d).transpose(1, 0, 2)        # [P, nt, d]
    v2 = np.repeat(v, 2, axis=2)                          # [P, nt, 2d]
    return np.ascontiguousarray(v2.reshape(P, nt * d * 2))


def attn_fold(attn_hf, H, F):
    """Per (h,f): permuted order (pos-signs first within each head),
    channel scale, prelu alpha.  Returns (perm j-list, scale, alpha)."""
    perm, scale, alpha = [], [], []
    for h in range(H):
        # alternate pos-first / neg-first per head so prelu alpha-runs merge
        # across head boundaries (fewer Act instructions)
        first_pos = (h % 2 == 0)
        fs = sorted(range(F),
                    key=lambda f: 0 if (attn_hf[h, f] > 0) == first_pos else 1)
        for f in fs:
            a = float(attn_hf[h, f])
            if a > 0:
                aa = max(a, 1e-8)
                perm.append(h * F + f); scale.append(aa); alpha.append(NEG_GAT)
            else:
                aa = min(a, -1e-8)
                perm.append(h * F + f); scale.append(NEG_GAT * aa); alpha.append(1.0 / NEG_GAT)
    return perm, np.asarray(scale, np.float64), alpha


def alpha_runs(alphas):
    runs = []
    i = 0
    while i < len(alphas):
        j = i
        while j < len(alphas) and alphas[j] == alphas[i]:
            j += 1
        runs.append((i, j, float(alphas[i])))
        i = j
    return runs


# ================================================================ device bits
def halving_tree(tt, X, L, out_final):
    """In-place halving over innermost axis of X [P,...,L]; final add -> out_final."""
    cur = L
    while cur > 2:
        h = cur // 2
        tt(out=X[..., 0:h], in0=X[..., 0:h], in1=X[..., cur - h:cur], op=AL.add)
        cur -= h
    if cur == 2:
        tt(out=out_final, in0=X[..., 0:1], in1=X[..., 1:2], op=AL.add)
    else:
        tt(out=out_final, in0=X[..., 0:1], in1=X[..., 0:1], op=AL.bypass)


def pp_tree(nc, A, B, L, out_final, cp=None):
    """Ping-pong halving reduce over innermost axis: A [P,..,L] (input, even L),
    B [P,..,>=L/2] scratch.  Output buffer alternates so out never shares a
    buffer with an input (keeps the DVE 2x mode).  cp: engine copy fn for the
    odd middle element (defaults to vector tensor_copy)."""
    cp = cp or nc.vector.tensor_copy
    tt = nc.vector.tensor_tensor
    src, dst = A, B
    cur = L
    while cur > 2:
        h = cur // 2
        tt(out=dst[..., 0:h], in0=src[..., 0:h], in1=src[..., h:2 * h], op=AL.add)
        if cur & 1:
            cp(out=dst[..., h:h + 1], in_=src[..., 2 * h:2 * h + 1])
            cur = h + 1
        else:
            cur = h
        src, dst = dst, src
    if cur == 2:
        tt(out=out_final, in0=src[..., 0:1], in1=src[..., 1:2], op=AL.add)
    else:
        cp(out=out_final, in_=src[..., 0:1])


def emit_elu(nc, sbS, hflat, nelem, tag):
    tmp = sbS.tile([P, nelem], f32, tag=tag)
    nc.vector.tensor_scalar_min(out=tmp[:], in0=hflat, scalar1=0.0)
    nc.scalar.activation(out=tmp[:], in_=tmp[:], func=AF.Exp)
    nc.vector.tensor_scalar(out=hflat, in0=hflat, scalar1=0.0, scalar2=-1.0,
                            op0=AL.max, op1=AL.add)
    nc.vector.tensor_tensor(out=hflat, in0=hflat, in1=tmp[:], op=AL.add)


def emit_gat_st_B(nc, sbG, sbZ, sbEX, sbT, d_g, off, L, S, fdp_ap, runs,
                  h1v, denv):
    """Layer-1 supertile: C=12 channels [d1h0(5), d1h1(5), a1h0, a1h1]."""
    C = 12
    L2 = L // 2
    ncols = S * C * L
    G = sbG.tile([P, ncols], bf16, tag="G")
    nc.sync.dma_start(out=G[:], in_=d_g[:, off:off + ncols])
    G4 = G[:].rearrange("p (s c l) -> p s c l", s=S, c=C, l=L)
    G5 = G[:].rearrange("p (sc l2 j) -> p sc l2 j", sc=S * C, l2=L2, j=2)
    Z = sbZ.tile([P, ncols], bf16, tag="Z")
    Z4 = Z[:].rearrange("p (s c l) -> p s c l", s=S, c=C, l=L)
    Z5 = Z[:].rearrange("p (sc l2 j) -> p sc l2 j", sc=S * C, l2=L2, j=2)
    fdb = fdp_ap.rearrange("p s c j -> p (s c) j").unsqueeze(2) \
        .broadcast_to([P, S * C, L2, 2])
    nc.vector.tensor_tensor(out=Z5, in0=G5, in1=fdb, op=AL.add)
    for (c0, c1, al) in runs:
        nc.scalar.activation(out=Z4[:, :, c0:c1, :], in_=Z4[:, :, c0:c1, :],
                             func=AF.Prelu, alpha=al)
    # score tree: d1 heads = sum of 5 channels each
    T = sbT.tile([P, S * 4 * L], bf16, tag="T")
    T4 = T[:].rearrange("p (s c l) -> p s c l", s=S, c=4, l=L)
    Tp = T[:].rearrange("p (s c2 c l) -> p s c2 c l", s=S, c2=2, c=2, l=L)
    nc.vector.tensor_tensor(out=T4[:, :, 0:2, :], in0=Z4[:, :, 0:2, :],
                            in1=Z4[:, :, 2:4, :], op=AL.add)
    nc.vector.tensor_tensor(out=T4[:, :, 2:4, :], in0=Z4[:, :, 5:7, :],
                            in1=Z4[:, :, 7:9, :], op=AL.add)
    SC = sbT.tile([P, S * 2 * L], bf16, tag="SC")
    SC3 = SC[:].rearrange("p (s c l) -> p s c l", s=S, c=2, l=L)
    nc.vector.tensor_tensor(out=SC3, in0=Tp[:, :, :, 0, :],
                            in1=Tp[:, :, :, 1, :], op=AL.add)
    Zh = Z4[:, :, 0:10, :].rearrange("p s (h f) l -> p s h f l", h=2, f=5)
    nc.vector.tensor_tensor(out=SC3, in0=SC3, in1=Zh[:, :, :, 4, :], op=AL.add)
    EX = sbEX.tile([P, S * 4 * L], bf16, tag="EX")
    EX4 = EX[:].rearrange("p (s c l) -> p s c l", s=S, c=4, l=L)
    nc.scalar.activation(out=EX4[:, :, 0:2, :], in_=SC3, func=AF.Exp)
    nc.scalar.activation(out=EX4[:, :, 2:4, :], in_=Z4[:, :, 10:12, :], func=AF.Exp)
    # weighted feats in-place on G (one op per d1 head: <=3 free dims)
    for h in range(2):
        Gh = G4[:, :, 5 * h:5 * h + 5, :]
        exd = EX4[:, :, h:h + 1, :].broadcast_to([P, S, 5, L])
        nc.vector.tensor_tensor(out=Gh, in0=Gh, in1=exd, op=AL.mult)
    nc.vector.tensor_tensor(out=G4[:, :, 10:12, :], in0=G4[:, :, 10:12, :],
                            in1=EX4[:, :, 2:4, :], op=AL.mult)
    # ping-pong partners = Z / T (both dead after score tree / exp)
    pp_tree(nc, G4, Z4[:, :, :, 0:L // 2 + 1], L, h1v.unsqueeze(3))
    pp_tree(nc, EX4, T4[:, :, :, 0:L // 2 + 1], L, denv.unsqueeze(3))


def emit_gat_st_C(nc, sbG, sbZ, sbEX, sbT, d_g, off, L, S, fdp_ap, runs,
                  h2v, denv):
    """Layer-2 supertile: C=4 channels [d2h0(2), d2h1(2)]."""
    C = 4
    L2 = L // 2
    ncols = S * C * L
    G = sbG.tile([P, ncols], bf16, tag="G")
    nc.sync.dma_start(out=G[:], in_=d_g[:, off:off + ncols])
    G4 = G[:].rearrange("p (s c l) -> p s c l", s=S, c=C, l=L)
    G5 = G[:].rearrange("p (sc l2 j) -> p sc l2 j", sc=S * C, l2=L2, j=2)
    Z = sbZ.tile([P, ncols], bf16, tag="Z")
    Z4 = Z[:].rearrange("p (s c l) -> p s c l", s=S, c=C, l=L)
    Z5 = Z[:].rearrange("p (sc l2 j) -> p sc l2 j", sc=S * C, l2=L2, j=2)
    fdb = fdp_ap.rearrange("p s c j -> p (s c) j").unsqueeze(2) \
        .broadcast_to([P, S * C, L2, 2])
    nc.vector.tensor_tensor(out=Z5, in0=G5, in1=fdb, op=AL.add)
    for (c0, c1, al) in runs:
        nc.scalar.activation(out=Z4[:, :, c0:c1, :], in_=Z4[:, :, c0:c1, :],
                             func=AF.Prelu, alpha=al)
    Zp = Z4.rearrange("p s (h f) l -> p s h f l", h=2, f=2)
    SC = sbT.tile([P, S * 2 * L], bf16, tag="SC")
    SC3 = SC[:].rearrange("p (s c l) -> p s c l", s=S, c=2, l=L)
    nc.vector.tensor_tensor(out=SC3, in0=Zp[:, :, :, 0, :],
                            in1=Zp[:, :, :, 1, :], op=AL.add)
    EX = sbEX.tile([P, S * 2 * L], bf16, tag="EX")
    EX3 = EX[:].rearrange("p (s c l) -> p s c l", s=S, c=2, l=L)
    nc.scalar.activation(out=EX3, in_=SC3, func=AF.Exp)
    for h in range(2):
        Gh = G4[:, :, 2 * h:2 * h + 2, :]
        exd = EX3[:, :, h:h + 1, :].broadcast_to([P, S, 2, L])
        nc.vector.tensor_tensor(out=Gh, in0=Gh, in1=exd, op=AL.mult)
    # ping-pong partners = Z / SC (both dead after score tree)
    pp_tree(nc, G4, Z4[:, :, :, 0:L // 2 + 1], L, h2v.unsqueeze(3))
    pp_tree(nc, EX3, SC3[:, :, :, 0:L // 2 + 1], L, denv.unsqueeze(3))


def emit_fixup(nc, sbS, hv, hflat, dv, dflat, fd_ap, rs_flat, np_ap, isc_ap,
               T, C, NH, dF, runs, do_elu=True):
    """Pad-slot den fix + normalize + unscale + residual + elu over T tiles.
    fd_ap: [P, T, C] per-node scaled dest proj; isc_ap: [P, C] inv scales."""
    zp = sbS.tile([P, T * C], bf16, tag="zp")
    zp3 = zp[:].rearrange("p (t c) -> p t c", t=T, c=C)
    for (c0, c1, al) in runs:
        nc.scalar.activation(out=zp3[:, :, c0:c1], in_=fd_ap[:, :, c0:c1],
                             func=AF.Prelu, alpha=al)
    ep = sbS.tile([P, T * NH], bf16, tag="ep")
    ep3 = ep[:].rearrange("p (t h) -> p t h", t=T, h=NH)
    if C == 12:
        zph = zp3[:, :, 0:10].rearrange("p t (h f) -> p t h f", h=2, f=5)
        tp = sbS.tile([P, T * 2], bf16, tag="tp")
        tp3 = tp[:].rearrange("p (t h) -> p t h", t=T, h=2)
        nc.vector.tensor_tensor(out=tp3, in0=zph[:, :, :, 0], in1=zph[:, :, :, 1], op=AL.add)
        nc.vector.tensor_tensor(out=tp3, in0=tp3, in1=zph[:, :, :, 2], op=AL.add)
        nc.vector.tensor_tensor(out=tp3, in0=tp3, in1=zph[:, :, :, 3], op=AL.add)
        nc.vector.tensor_tensor(out=tp3, in0=tp3, in1=zph[:, :, :, 4], op=AL.add)
        nc.scalar.activation(out=ep3[:, :, 0:2], in_=tp3, func=AF.Exp)
        nc.scalar.activation(out=ep3[:, :, 2:4], in_=zp3[:, :, 10:12], func=AF.Exp)
    else:
        zph = zp3.rearrange("p t (h f) -> p t h f", h=2, f=2)
        tp = sbS.tile([P, T * 2], bf16, tag="tp")
        tp3 = tp[:].rearrange("p (t h) -> p t h", t=T, h=2)
        nc.vector.tensor_tensor(out=tp3, in0=zph[:, :, :, 0], in1=zph[:, :, :, 1], op=AL.add)
        nc.scalar.activation(out=ep3, in_=tp3, func=AF.Exp)
    padm = sbS.tile([P, T * NH], f32, tag="padm")
    pm3 = padm[:].rearrange("p (t h) -> p t h", t=T, h=NH)
    npb = np_ap.unsqueeze(2).broadcast_to([P, T, NH])
    nc.vector.tensor_tensor(out=pm3, in0=ep3, in1=npb, op=AL.mult)
    nc.vector.tensor_tensor(out=dv, in0=dv, in1=pm3, op=AL.subtract)
    nc.vector.tensor_scalar_max(out=dflat, in0=dflat, scalar1=1e-30)
    rec = sbS.tile([P, T * NH], f32, tag="rec")
    nc.vector.reciprocal(out=rec[:], in_=dflat)
    rec3 = rec[:].rearrange("p (t h) -> p t h", t=T, h=NH)
    # rec12 = rec[h(c)] * inv_scale_c
    rc = sbS.tile([P, T * C], f32, tag="rc")
    rc3 = rc[:].rearrange("p (t c) -> p t c", t=T, c=C)
    iscb = isc_ap.unsqueeze(1).broadcast_to([P, T, C])
    if C == 12:
        rch = rc3[:, :, 0:10].rearrange("p t (h f) -> p t h f", h=2, f=5)
        rb = rec3[:, :, 0:2].unsqueeze(3).broadcast_to([P, T, 2, 5])
        ib = iscb[:, :, 0:10].rearrange("p t (h f) -> p t h f", h=2, f=5)
        nc.vector.tensor_tensor(out=rch, in0=rb, in1=ib, op=AL.mult)
        nc.vector.tensor_tensor(out=rc3[:, :, 10:12], in0=rec3[:, :, 2:4],
                                in1=iscb[:, :, 10:12], op=AL.mult)
    else:
        rch = rc3.rearrange("p t (h f) -> p t h f", h=2, f=2)
        rb = rec3.unsqueeze(3).broadcast_to([P, T, 2, 2])
        ib = iscb.rearrange("p t (h f) -> p t h f", h=2, f=2)
        nc.vector.tensor_tensor(out=rch, in0=rb, in1=ib, op=AL.mult)
    nc.vector.tensor_tensor(out=hflat, in0=hflat, in1=rc[:], op=AL.mult)
    nc.vector.tensor_tensor(out=hflat, in0=hflat, in1=rs_flat, op=AL.add)
    if do_elu:
        emit_elu(nc, sbS, hflat, T * C, "elu")


# =============================================================== launch A
def build_launchA(nt):
    cols = nt * 16
    nc = bacc.Bacc("TRN2", target_bir_lowering=False, debug=False, num_devices=NCORE)
    d_x = nc.dram_tensor("x5l", [48, cols], bf16, kind="ExternalInput")
    d_bfs = nc.dram_tensor("bd_fs", [48, P], bf16, kind="ExternalInput")
    d_bfd = nc.dram_tensor("bd_fd", [48, P], bf16, kind="ExternalInput")
    d_brs = nc.dram_tensor("bd_rs", [48, P], bf16, kind="ExternalInput")
    d_fs = nc.dram_tensor("fs1cm", [P, cols], bf16, kind="ExternalOutput")
    d_fd = nc.dram_tensor("fd1cm", [P, cols], bf16, kind="ExternalOutput")
    d_rs = nc.dram_tensor("rs1cm", [P, cols], f32, kind="ExternalOutput")
    with TileContext(nc) as tc:
        with tc.tile_pool(name="res", bufs=1) as res, \
             tc.tile_pool(name="ps", bufs=2, space="PSUM") as ps:
            stg = res.tile([48, cols], bf16)
            nc.sync.dma_start(out=stg[:], in_=d_x[:, :])
            bfs = res.tile([48, P], bf16, tag="bfs")
            nc.sync.dma_start(out=bfs[:], in_=d_bfs[:, :])
            bfd = res.tile([48, P], bf16, tag="bfd")
            nc.sync.dma_start(out=bfd[:], in_=d_bfd[:, :])
            brs = res.tile([48, P], bf16, tag="brs")
            nc.sync.dma_start(out=brs[:], in_=d_brs[:, :])
            ofs = res.tile([P, cols], bf16, tag="ofs")
            ofd = res.tile([P, cols], bf16, tag="ofd")
            ors = res.tile([P, cols], f32, tag="ors")
            k = 0
            for j0 in range(0, cols, 512):
                w = min(512, cols - j0)
                for bd, ot in ((bfs, ofs), (bfd, ofd), (brs, ors)):
                    pmm = ps.tile([P, 512], f32, tag="mm")
                    nc.tensor.matmul(out=pmm[:, :w], lhsT=bd[:], rhs=stg[:, j0:j0 + w],
                                     start=True, stop=True)
                    if k % 2 == 0:
                        nc.vector.tensor_copy(out=ot[:, j0:j0 + w], in_=pmm[:, :w])
                    else:
                        nc.scalar.copy(out=ot[:, j0:j0 + w], in_=pmm[:, :w])
                    k += 1
            nc.sync.dma_start(out=d_fs[:, :], in_=ofs[:])
            nc.sync.dma_start(out=d_fd[:, :], in_=ofd[:])
            nc.sync.dma_start(out=d_rs[:, :], in_=ors[:])
    nc.compile()
    return nc


# =============================================================== launch B
def build_launchB(nst, Ls, offs12, nt, runs12):
    totc = int(offs12[-1])
    fgw = -(-nst // 4) * SUPER * P          # f2 output column width
    nc = bacc.Bacc("TRN2", target_bir_lowering=False, debug=False, num_devices=NCORE)
    d_g = nc.dram_tensor("g1", [P, totc], bf16, kind="ExternalInput")
    d_fdp = nc.dram_tensor("fdp1", [P, nt * 24], bf16, kind="ExternalInput")
    d_rs = nc.dram_tensor("rs1n", [P, nt * 12], f32, kind="ExternalInput")
    d_np = nc.dram_tensor("npad", [P, nt], f32, kind="ExternalInput")
    d_isc = nc.dram_tensor("isc12", [P, 12], f32, kind="ExternalInput")
    d_w2 = nc.dram_tensor("w2all", [10, 12], bf16, kind="ExternalInput")
    d_bc2 = nc.dram_tensor("bc2", [12], f32, kind="ExternalInput")
    d_f2 = nc.dram_tensor("f2cm", [48, fgw], f32, kind="ExternalOutput")
    d_ha = nc.dram_tensor("hattn", [P, nt * 2], bf16, kind="ExternalOutput")
    groups = [(0, 10), (10, 18), (18, nst)] if nst >= 18 else [(0, nst)]
    with TileContext(nc) as tc:
        with tc.tile_pool(name="res", bufs=1) as res, \
             tc.tile_pool(name="sbG", bufs=3) as sbG, \
             tc.tile_pool(name="sbZ", bufs=3) as sbZ, \
             tc.tile_pool(name="sbEX", bufs=3) as sbEX, \
             tc.tile_pool(name="sbT", bufs=2) as sbT, \
             tc.tile_pool(name="sbS", bufs=1) as sbS, \
             tc.tile_pool(name="sbT2", bufs=1) as sbT2, \
             tc.tile_pool(name="psT", bufs=1, space="PSUM") as psT, \
             tc.tile_pool(name="psF", bufs=1, space="PSUM") as psF:
            ident = res.tile([P, P], bf16)
            make_identity(nc, ident[:])
            npad = res.tile([P, nt], f32)
            nc.sync.dma_start(out=npad[:], in_=d_np[:, :])
            isc = res.tile([P, 12], f32)
            nc.sync.dma_start(out=isc[:], in_=d_isc[:, :])
            fdp = res.tile([P, nt * 24], bf16)
            nc.sync.dma_start(out=fdp[:], in_=d_fdp[:, :])
            rst = res.tile([P, nt * 12], f32)
            nc.sync.dma_start(out=rst[:], in_=d_rs[:, :])
            w2t = res.tile([10, 12], bf16, tag="w2t")
            nc.sync.dma_start(out=w2t[:], in_=d_w2[:, :])
            bc2 = res.tile([12, 1], f32, tag="bc2")
            nc.sync.dma_start(out=bc2[:], in_=d_bc2[:, None])
            h1 = res.tile([P, nt * 12], f32)
            den = res.tile([P, nt * 4], f32)
            hat = res.tile([P, nt * 2], bf16)
            h1v_all = h1[:].rearrange("p (t c) -> p t c", t=nt, c=12)
            denv_all = den[:].rearrange("p (t c) -> p t c", t=nt, c=4)
            fdp_all = fdp[:].rearrange("p (t c j) -> p t c j", t=nt, c=12, j=2)
            npv_all = npad[:].rearrange("p (t o) -> p t o", t=nt, o=1)
            for g0, g1 in groups:
                for st in range(g0, g1):
                    L = Ls[st]
                    t0 = st * SUPER
                    emit_gat_st_B(nc, sbG, sbZ, sbEX, sbT, d_g, int(offs12[st]),
                                  L, SUPER, fdp_all[:, t0:t0 + SUPER],
                                  runs12,
                                  h1v_all[:, t0:t0 + SUPER, :],
                                  denv_all[:, t0:t0 + SUPER, :])
                T = (g1 - g0) * SUPER
                t0 = g0 * SUPER
                emit_fixup(nc, sbS, h1v_all[:, t0:t0 + T, :],
                           h1[:, t0 * 12:(t0 + T) * 12],
                           denv_all[:, t0:t0 + T, :],
                           den[:, t0 * 4:(t0 + T) * 4],
                           fdp_all[:, t0:t0 + T, :, 0],
                           rst[:, t0 * 12:(t0 + T) * 12],
                           npv_all[:, t0:t0 + T, 0], isc[:], T, 12, 4, 5, runs12)
                hv = h1v_all[:, t0:t0 + T, :]
                nc.scalar.copy(out=hat[:].rearrange("p (t c) -> p t c", t=nt, c=2)[:, t0:t0 + T, :],
                               in_=hv[:, :, 10:12])
                h1b = sbS.tile([P, T * 10], bf16, tag="h1b")
                nc.scalar.copy(out=h1b[:].rearrange("p (t c) -> p t c", t=T, c=10),
                               in_=hv[:, :, 0:10])
                # epilogue: transpose h_def1 per tile, project fs2''|fd2''|res2
                for st in range(g0, g1):
                    pT = psT.tile([10, SUPER * P], bf16, tag="pT")
                    for b in range(SUPER):
                        trel = (st - g0) * SUPER + b
                        nc.tensor.transpose(out=pT[:, b * P:(b + 1) * P],
                                            in_=h1b[:, trel * 10:trel * 10 + 10],
                                            identity=ident[:])
                    hT = sbT2.tile([10, SUPER * P], bf16, tag="hT")
                    nc.scalar.copy(out=hT[:], in_=pT[:])
                    pF = psF.tile([12, SUPER * P], f32, tag="pF")
                    for q in range(SUPER * P // 512):
                        nc.tensor.matmul(out=pF[:, q * 512:(q + 1) * 512], lhsT=w2t[:],
                                         rhs=hT[:, q * 512:(q + 1) * 512],
                                         start=True, stop=True)
                    f2s = sbT2.tile([12, SUPER * P], f32, tag="f2s")
                    nc.scalar.activation(out=f2s[:], in_=pF[:], func=AF.Prelu,
                                         alpha=1.0, bias=bc2[:])
                    nc.sync.dma_start(
                        out=d_f2[12 * (st % 4):12 * (st % 4) + 12,
                                 (st // 4) * SUPER * P:(st // 4 + 1) * SUPER * P],
                        in_=f2s[:])
            nc.sync.dma_start(out=d_ha[:, :], in_=hat[:])
    nc.compile()
    return nc


# =============================================================== launch C
def build_launchC(nst, Ls, offs4, nt, nmc, runs4):
    totc = int(offs4[-1])
    ngrp = -(-nmc // 4)
    nc = bacc.Bacc("TRN2", target_bir_lowering=False, debug=False, num_devices=NCORE)
    d_g = nc.dram_tensor("g2", [P, totc], bf16, kind="ExternalInput")
    d_fdp = nc.dram_tensor("fdp2", [P, nt * 8], bf16, kind="ExternalInput")
    d_rs = nc.dram_tensor("rs2n", [P, nt * 4], f32, kind="ExternalInput")
    d_np = nc.dram_tensor("npad", [P, nt], f32, kind="ExternalInput")
    d_isc = nc.dram_tensor("isc4", [P, 4], f32, kind="ExternalInput")
    d_ha = nc.dram_tensor("hattn", [P, nt * 2], bf16, kind="ExternalInput")
    d_x = nc.dram_tensor("xpm", [P, nt * 8], bf16, kind="ExternalInput")
    d_w1 = nc.dram_tensor("w1x", [32, 196], bf16, kind="ExternalInput")
    w2dt = fp8 if FP8_W2 else bf16
    d_w2a = nc.dram_tensor("w2dra", [P, 2 * 128], w2dt, kind="ExternalInput")
    d_w2b = nc.dram_tensor("w2drb", [P, 2 * 128], w2dt, kind="ExternalInput")
    d_w3a = nc.dram_tensor("w3a", [P, 14], bf16, kind="ExternalInput")
    d_w3b = nc.dram_tensor("w3b", [68, 14], bf16, kind="ExternalInput")
    d_w4 = nc.dram_tensor("w4blk", [110, 4], bf16, kind="ExternalInput")
    d_b1 = nc.dram_tensor("b1", [196], f32, kind="ExternalInput")
    d_b2 = nc.dram_tensor("b2", [196], f32, kind="ExternalInput")
    d_b3 = nc.dram_tensor("b3", [14], f32, kind="ExternalInput")
    d_b4 = nc.dram_tensor("b4r", [4], f32, kind="ExternalInput")
    d_out = nc.dram_tensor("out", [nmc, 512], f32, kind="ExternalOutput")
    groups = [(0, 6), (6, 12), (12, 18), (18, 22), (22, nst)] if nst >= 22 else [(0, nst)]
    with TileContext(nc) as tc:
        with tc.tile_pool(name="res", bufs=1) as res, \
             tc.tile_pool(name="sbG", bufs=3) as sbG, \
             tc.tile_pool(name="sbZ", bufs=3) as sbZ, \
             tc.tile_pool(name="sbEX", bufs=3) as sbEX, \
             tc.tile_pool(name="sbT", bufs=2) as sbT, \
             tc.tile_pool(name="sbS", bufs=1) as sbS, \
             tc.tile_pool(name="sbM", bufs=2) as sbM, \
             tc.tile_pool(name="psT", bufs=1, space="PSUM") as psT, \
             tc.tile_pool(name="psC", bufs=1, space="PSUM") as psC, \
             tc.tile_pool(name="psA", bufs=2, space="PSUM") as psA, \
             tc.tile_pool(name="psB", bufs=1, space="PSUM") as psB:
            ident = res.tile([P, P], bf16)
            make_identity(nc, ident[:])
            npad = res.tile([P, nt], f32)
            nc.sync.dma_start(out=npad[:], in_=d_np[:, :])
            isc = res.tile([P, 4], f32)
            nc.sync.dma_start(out=isc[:], in_=d_isc[:, :])
            fdp = res.tile([P, nt * 8], bf16)
            nc.sync.dma_start(out=fdp[:], in_=d_fdp[:, :])
            rst = res.tile([P, nt * 4], f32)
            nc.sync.dma_start(out=rst[:], in_=d_rs[:, :])
            hat = res.tile([P, nt * 2], bf16)
            nc.sync.dma_start(out=hat[:], in_=d_ha[:, :])
            xpm = res.tile([P, nt * 8], bf16)
            nc.sync.dma_start(out=xpm[:], in_=d_x[:, :])
            w1 = res.tile([32, 196], bf16, tag="w1")
            nc.sync.dma_start(out=w1[:], in_=d_w1[:, :])
            w2a = res.tile([P, 2 * 128], w2dt, tag="w2a")
            nc.sync.dma_start(out=w2a[:], in_=d_w2a[:, :])
            w2b = res.tile([P, 2 * 128], w2dt, tag="w2b")
            nc.sync.dma_start(out=w2b[:], in_=d_w2b[:, :])
            w3a = res.tile([P, 14], bf16, tag="w3a")
            nc.sync.dma_start(out=w3a[:], in_=d_w3a[:, :])
            w3b = res.tile([68, 14], bf16, tag="w3b")
            nc.sync.dma_start(out=w3b[:], in_=d_w3b[:, :])
            w4b = res.tile([110, 4], bf16, tag="w4b")
            nc.sync.dma_start(out=w4b[:], in_=d_w4[:, :])
            w2av = w2a[:].rearrange("p (k m) -> p k m", k=2, m=128)
            w2bv = w2b[:].rearrange("p (k m) -> p k m", k=2, m=128)
            b1ca = res.tile([P, 1], f32, tag="b1ca")
            nc.sync.dma_start(out=b1ca[:], in_=d_b1[0:128, None])
            b1cb = res.tile([68, 1], f32, tag="b1cb")
            nc.sync.dma_start(out=b1cb[:], in_=d_b1[128:196, None])
            b2ca = res.tile([P, 1], f32, tag="b2ca")
            nc.sync.dma_start(out=b2ca[:], in_=d_b2[0:128, None])
            b2cb = res.tile([68, 1], f32, tag="b2cb")
            nc.sync.dma_start(out=b2cb[:], in_=d_b2[128:196, None])
            b3c = res.tile([14, 1], f32, tag="b3c")
            nc.sync.dma_start(out=b3c[:], in_=d_b3[:, None])
            b4c = res.tile([4, 1], f32, tag="b4c")
            nc.sync.dma_start(out=b4c[:], in_=d_b4[:, None])
            h2 = res.tile([P, nt * 4], f32)
            den = res.tile([P, nt * 2], f32)
            m32 = res.tile([P, nt * 32], bf16)
            nc.gpsimd.memset(m32[:], 0.0)
            r3st = res.tile([110, ngrp * 512], bf16, tag="r3st")
            adt = fp8 if FP8_W2 else bf16
            r1t = [res.tile([P, 2048], adt, tag=f"r1_{i}", name=f"r1_{i}")
                   for i in range(2)]
            for t_ in r1t:
                nc.gpsimd.memset(t_[:], 0.0)
            h2v_all = h2[:].rearrange("p (t c) -> p t c", t=nt, c=4)
            denv_all = den[:].rearrange("p (t c) -> p t c", t=nt, c=2)
            fdp_all = fdp[:].rearrange("p (t c j) -> p t c j", t=nt, c=4, j=2)
            npv_all = npad[:].rearrange("p (t o) -> p t o", t=nt, o=1)
            m3_all = m32[:].rearrange("p (t c) -> p t c", t=nt, c=32)
            hav_all = hat[:].rearrange("p (t c) -> p t c", t=nt, c=2)
            xv_all = xpm[:].rearrange("p (t c) -> p t c", t=nt, c=8)
            nc.vector.tensor_copy(out=m3_all[:, :, 0:2], in_=hav_all)
            nc.vector.tensor_copy(out=m3_all[:, :, 6:14], in_=xv_all)
            for g0, g1 in groups:
                for st in range(g0, g1):
                    L = Ls[st]
                    t0 = st * SUPER
                    emit_gat_st_C(nc, sbG, sbZ, sbEX, sbT, d_g, int(offs4[st]),
                                  L, SUPER, fdp_all[:, t0:t0 + SUPER],
                                  runs4,
                                  h2v_all[:, t0:t0 + SUPER, :],
                                  denv_all[:, t0:t0 + SUPER, :])
                T = (g1 - g0) * SUPER
                t0 = g0 * SUPER
                emit_fixup(nc, sbS, h2v_all[:, t0:t0 + T, :],
                           h2[:, t0 * 4:(t0 + T) * 4],
                           denv_all[:, t0:t0 + T, :],
                           den[:, t0 * 2:(t0 + T) * 2],
                           fdp_all[:, t0:t0 + T, :, 0],
                           rst[:, t0 * 4:(t0 + T) * 4],
                           npv_all[:, t0:t0 + T, 0], isc[:], T, 4, 2, 2, runs4)
                nc.vector.tensor_copy(out=m3_all[:, t0:t0 + T, 2:6],
                                      in_=h2v_all[:, t0:t0 + T, :])
                # MLP over this group's chunks, processed in pairs so each
                # activation instruction covers 1024 nodes (half the Act
                # instruction count)
                mc1 = min((t0 + T) // 4, nmc)
                for mcp in range(t0 // 4, mc1, 2):
                    pair = [mc for mc in (mcp, mcp + 1) if mc < mc1]
                    npair = len(pair)
                    p1a = psA.tile([P, 1024], f32, tag="pA")
                    p1b = psB.tile([P, 1024], f32, tag="pB")
                    for ci_, mc in enumerate(pair):
                        pT = psT.tile([64, 256], bf16, tag="pT")
                        nc.tensor.transpose(out=pT[:, 0:128],
                                            in_=m32[:, (mc * 4) * 32:(mc * 4 + 2) * 32],
                                            identity=ident[:])
                        nc.tensor.transpose(out=pT[:, 128:256],
                                            in_=m32[:, (mc * 4 + 2) * 32:(mc * 4 + 4) * 32],
                                            identity=ident[:])
                        r0 = sbM.tile([32, 512], bf16, tag=f"r0_{ci_}")
                        r0v = r0[:].rearrange("r (b c) -> r b c", b=4, c=128)
                        pTv0 = pT[0:32, :].rearrange("r (b c) -> r b c", b=2, c=128)
                        pTv1 = pT[32:64, :].rearrange("r (b c) -> r b c", b=2, c=128)
                        nc.vector.tensor_copy(out=r0v[:, 0:4:2, :], in_=pTv0)
                        nc.vector.tensor_copy(out=r0v[:, 1:4:2, :], in_=pTv1)
                        nc.tensor.matmul(out=p1a[:, ci_ * 512:(ci_ + 1) * 512],
                                         lhsT=w1[:, 0:128], rhs=r0[:], start=True, stop=True)
                        nc.tensor.matmul(out=p1b[0:68, ci_ * 512:(ci_ + 1) * 512],
                                         lhsT=w1[:, 128:196], rhs=r0[:], start=True, stop=True)
                    r1p = r1t[(mcp // 2) % 2]          # [P, 2048] fp8: cols (k, chunk, 512)
                    w = npair * 512
                    nc.scalar.activation(out=r1p[:, 0:w], in_=p1a[:, 0:w],
                                         func=AF.Prelu, alpha=NEG_MLP, bias=b1ca[:])
                    nc.scalar.activation(out=r1p[0:68, 1024:1024 + w], in_=p1b[0:68, 0:w],
                                         func=AF.Prelu, alpha=NEG_MLP, bias=b1cb[:])
                    r1v = r1p[:].rearrange("p (k c n) -> p k c n", k=2, c=2, n=512)
                    p2a = psA.tile([P, 1024], f32, tag="pA")
                    p2b = psB.tile([P, 1024], f32, tag="pB")
                    for ci_, mc in enumerate(pair):
                        nc.tensor.matmul(out=p2a[:, ci_ * 512:(ci_ + 1) * 512], lhsT=w2av,
                                         rhs=r1v[:, :, ci_, :],
                                         start=True, stop=True, perf_mode=MPM.DoubleRow)
                        nc.tensor.matmul(out=p2b[:, ci_ * 512:(ci_ + 1) * 512], lhsT=w2bv,
                                         rhs=r1v[:, :, ci_, :],
                                         start=True, stop=True, perf_mode=MPM.DoubleRow)
                    r2a = sbM.tile([P, 1024], bf16, tag="r2a")
                    nc.scalar.activation(out=r2a[:, 0:w], in_=p2a[:, 0:w], func=AF.Prelu,
                                         alpha=NEG_MLP, bias=b2ca[:])
                    r2b = sbM.tile([68, 1024], bf16, tag="r2b")
                    nc.scalar.activation(out=r2b[:, 0:w], in_=p2b[0:68, 0:w], func=AF.Prelu,
                                         alpha=NEG_MLP, bias=b2cb[:])
                    for ci_, mc in enumerate(pair):
                        p3 = psC.tile([14, 512], f32, tag="p3")
                        nc.tensor.matmul(out=p3[:], lhsT=w3a[:],
                                         rhs=r2a[:, ci_ * 512:(ci_ + 1) * 512], start=True, stop=False)
                        nc.tensor.matmul(out=p3[:], lhsT=w3b[:],
                                         rhs=r2b[0:68, ci_ * 512:(ci_ + 1) * 512], start=False, stop=True)
                        ro = 32 * (mc % 4)
                        nc.scalar.activation(out=r3st[ro:ro + 14, (mc // 4) * 512:(mc // 4 + 1) * 512],
                                             in_=p3[:], func=AF.Prelu,
                                             alpha=NEG_MLP, bias=b3c[:])
            # tail: stacked 14->1 matmuls + sigmoids (one act-table switch)
            for g in range(ngrp):
                k = min(4, nmc - 4 * g)
                kp = 32 * (k - 1) + 14
                po = psC.tile([14, 512], f32, tag="p3")
                nc.tensor.matmul(out=po[0:k, 0:512], lhsT=w4b[0:kp, 0:k],
                                 rhs=r3st[0:kp, g * 512:(g + 1) * 512],
                                 start=True, stop=True)
                sg = sbM.tile([4, 512], f32, tag="sg")
                nc.scalar.activation(out=sg[0:k, :], in_=po[0:k, 0:512], func=AF.Sigmoid,
                                     bias=b4c[0:k, :])
                nc.sync.dma_start(out=d_out[4 * g:4 * g + k, :], in_=sg[0:k, :])
    nc.compile()
    return nc


# ================================================================== kernel
_cache = {}


def kernel(**inputs):
    x = np.asarray(inputs['x'], np.float32)
    src = np.asarray(inputs['src'], np.int32)
    dst = np.asarray(inputs['dst'], np.int32)
    n = x.shape[0]

    scheds, nst, Ls = build_schedule(dst, n)
    nt = scheds[0]['nt']
    nloc = scheds[0]['nloc']
    nmc = -(-nloc // 512)
    offs12 = np.concatenate([[0], np.cumsum([SUPER * L * 12 for L in Ls])]).astype(np.int64)
    offs4 = np.concatenate([[0], np.cumsum([SUPER * L * 4 for L in Ls])]).astype(np.int64)

    # ---- layer-1 attn folding: channels [d1h0(5), d1h1(5), a1h0, a1h1]
    d1_attn = np.asarray(inputs['d1_attn'], np.float64)     # [2, 5]
    a1_attn = np.asarray(inputs['a1_attn'], np.float64)     # [2, 1]
    perm_d1, scale_d1, alpha_d1 = attn_fold(d1_attn, 2, 5)
    perm_a1, scale_a1, alpha_a1 = attn_fold(a1_attn[:, :], 2, 1)
    scale12 = np.concatenate([scale_d1, scale_a1])
    alpha12 = alpha_d1 + alpha_a1
    runs12 = alpha_runs(alpha12)

    def l1_pack(a1_w, d1_w, scale=None):
        w = np.zeros((a1_w.shape[0], 12), np.float64)
        for p_, j in enumerate(perm_d1):
            w[:, p_] = d1_w[:, j]
        for p_, j in enumerate(perm_a1):
            w[:, 10 + p_] = a1_w[:, j]
        if scale is not None:
            w = w * scale[None, :]
        return w

    a1_Wsrc = np.asarray(inputs['a1_Wsrc'], np.float64)
    d1_Wsrc = np.asarray(inputs['d1_Wsrc'], np.float64)
    a1_Wdst = np.asarray(inputs['a1_Wdst'], np.float64)
    d1_Wdst = np.asarray(inputs['d1_Wdst'], np.float64)
    a1_Wres = np.asarray(inputs['a1_Wres'], np.float64)
    d1_Wres = np.asarray(inputs['d1_Wres'], np.float64)
    bY = l1_pack(np.asarray(inputs['a1_bsrc'], np.float64)[None, :],
                 np.asarray(inputs['d1_bsrc'], np.float64)[None, :], scale12)[0]
    bD = l1_pack(np.asarray(inputs['a1_bdst'], np.float64)[None, :],
                 np.asarray(inputs['d1_bdst'], np.float64)[None, :], scale12)[0]
    bR = l1_pack(np.asarray(inputs['a1_bias'], np.float64)[None, :],
                 np.asarray(inputs['d1_bias'], np.float64)[None, :])[0]
    bd_fs = blockdiag(l1_pack(a1_Wsrc, d1_Wsrc, scale12).astype(np.float32), bY.astype(np.float32), 6)
    bd_fd = blockdiag(l1_pack(a1_Wdst, d1_Wdst, scale12).astype(np.float32), bD.astype(np.float32), 6)
    bd_rs = blockdiag(l1_pack(a1_Wres, d1_Wres).astype(np.float32), bR.astype(np.float32), 6)
    isc12 = np.tile((1.0 / scale12).astype(np.float32), (P, 1))

    # ---- layer-2 attn folding: channels [d2h0(2), d2h1(2)]
    d2_attn = np.asarray(inputs['d2_attn'], np.float64)     # [2, 2]
    perm_d2, scale4, alpha4 = attn_fold(d2_attn, 2, 2)
    runs4 = alpha_runs(alpha4)

    def d2w(name):
        w = np.asarray(inputs[name], np.float64)            # [10, 4] native cols j=2h+f
        out = np.zeros((10, 4), np.float64)
        for p_, j in enumerate(perm_d2):
            out[:, p_] = w[:, j]
        return out

    def d2b(name):
        b = np.asarray(inputs[name], np.float64)
        return b[perm_d2]

    # rows of the [10, 12] projection are h_def1 in MY permuted order
    rowperm = perm_d1                                       # position -> native j=5h+f
    ws2 = d2w('d2_Wsrc')[rowperm] * scale4[None, :]
    wd2 = d2w('d2_Wdst')[rowperm] * scale4[None, :]
    wr2 = d2w('d2_Wres')[rowperm]
    w2all = np.concatenate([ws2, wd2, wr2], axis=1).astype(np.float32)
    bc2 = np.concatenate([d2b('d2_bsrc') * scale4, d2b('d2_bdst') * scale4,
                          d2b('d2_bias')]).astype(np.float32)
    isc4 = np.tile((1.0 / scale4).astype(np.float32), (P, 1))

    # ---- MLP weights: W1 rows 2:6 permuted to h_def2 order
    w1p = np.asarray(inputs['W1'], np.float64).copy()
    W1n = np.asarray(inputs['W1'], np.float64)
    for p_, j in enumerate(perm_d2):
        w1p[2 + p_] = W1n[2 + j]
    w1x = np.zeros((32, 196), np.float32)
    w1x[0:14] = w1p.astype(np.float32)
    W2 = np.asarray(inputs['W2'], np.float32)
    w2dra = np.zeros((P, 2, 128), np.float32)
    w2dra[:, 0, :] = W2[0:128, 0:128]
    w2dra[0:68, 1, :] = W2[128:196, 0:128]
    w2drb = np.zeros((P, 2, 128), np.float32)
    w2drb[:, 0, 0:68] = W2[0:128, 128:196]
    w2drb[0:68, 1, 0:68] = W2[128:196, 128:196]
    FPW = FP8 if FP8_W2 else BF
    w4 = np.asarray(inputs['W4'], np.float32)               # [14, 1]
    w4blk = np.zeros((110, 4), np.float32)
    for k in range(4):
        w4blk[32 * k:32 * k + 14, k] = w4[:, 0]
    b4r = np.full(4, float(np.asarray(inputs['b4'])[0]), np.float32)

    key = (n, len(src), nst, tuple(Ls), tuple(runs12), tuple(runs4))
    if key not in _cache:
        _cache.clear()
        _cache[key] = (build_launchA(nt), build_launchB(nst, Ls, offs12, nt, runs12),
                       build_launchC(nst, Ls, offs4, nt, nmc, runs4))
    ncA, ncB, ncC = _cache[key]

    # ---------------- launch A: per-node projections of x
    inA = []
    for s in scheds:
        orig = s['order']
        valid = orig < nloc
        xl = np.zeros((nt * P, 5), np.float32)
        xl[valid] = x[s['core'] * nloc + orig[valid], :5]
        inA.append(dict(x5l=pack_local(xl, 6, nt), bd_fs=bd_fs, bd_fd=bd_fd, bd_rs=bd_rs))
    rA = run_bass_kernel_spmd(ncA, inA, core_ids=list(range(NCORE)))
    tA = rA.exec_time_ns or 0

    i_all = np.arange(nt * P)
    a_i = (i_all // P) % 8
    col_i = (i_all // (8 * P)) * P + i_all % P
    rows12 = a_i[:, None] * 16 + np.arange(12)[None, :]
    fs1g = np.zeros((n, 12), BF)
    geoms, fdp1_l, rs1n_l, npad_l = [], [], [], []
    for ci, s in enumerate(scheds):
        fs_sorted = rA.results[ci]['fs1cm'][rows12, col_i[:, None]]
        fd_sorted = rA.results[ci]['fd1cm'][rows12, col_i[:, None]]
        rs_sorted = rA.results[ci]['rs1cm'][rows12, col_i[:, None]]
        orig = s['order']
        valid = orig < nloc
        fs1g[s['core'] * nloc + orig[valid]] = fs_sorted[valid]
        fdp1_l.append(pm_pair(fd_sorted, nt))
        rs1n_l.append(pm(rs_sorted.astype(np.float32), nt))
        geoms.append(edge_slot_geom(s, Ls))
        npad_l.append(make_npad(s, Ls, nt))

    inB = []
    for ci, s in enumerate(scheds):
        eo, st_of, s_of, rank, p_of = geoms[ci]
        v = fs1g[src[s['em']][eo]]
        g1 = pack_G(v, st_of, s_of, rank, p_of, offs12, 12, Ls, int(offs12[-1]))
        inB.append(dict(g1=g1, fdp1=fdp1_l[ci], rs1n=rs1n_l[ci], isc12=isc12,
                        npad=npad_l[ci], w2all=w2all.astype(BF), bc2=bc2))
    rB = run_bass_kernel_spmd(ncB, inB, core_ids=list(range(NCORE)))
    tB = rB.exec_time_ns or 0

    fgw = -(-nst // 4) * SUPER * P
    fs2g = np.zeros((n, 4), BF)
    fdp2_l, rs2n_l, ha_l, xpm_l = [], [], [], []
    for ci, s in enumerate(scheds):
        fb = rB.results[ci]['f2cm']              # [48, fgw]
        f2 = np.zeros((12, nt * P), np.float32)
        for st in range(nst):
            f2[:, st * SUPER * P:(st + 1) * SUPER * P] = \
                fb[12 * (st % 4):12 * (st % 4) + 12,
                   (st // 4) * SUPER * P:(st // 4 + 1) * SUPER * P]
        orig = s['order']
        valid = orig < nloc
        fs2g[s['core'] * nloc + orig[valid]] = f2[0:4, :].T[valid].astype(BF)
        fdp2_l.append(pm_pair(f2[4:8, :].T.astype(BF), nt))
        rs2n_l.append(pm(np.ascontiguousarray(f2[8:12, :].T), nt))
        ha_l.append(rB.results[ci]['hattn'])
        xl8 = np.zeros((nt * P, 8), np.float32)
        xl8[valid] = x[s['core'] * nloc + orig[valid], :]
        xpm_l.append(pm(xl8, nt).astype(BF))

    inC = []
    for ci, s in enumerate(scheds):
        eo, st_of, s_of, rank, p_of = geoms[ci]
        v = fs2g[src[s['em']][eo]]
        g2 = pack_G(v, st_of, s_of, rank, p_of, offs4, 4, Ls, int(offs4[-1]))
        inC.append(dict(g2=g2, fdp2=fdp2_l[ci], rs2n=rs2n_l[ci], isc4=isc4,
                        npad=npad_l[ci], hattn=ha_l[ci], xpm=xpm_l[ci],
                        w1x=w1x.astype(BF),
                        w2dra=w2dra.reshape(P, 256).astype(FPW),
                        w2drb=w2drb.reshape(P, 256).astype(FPW),
                        w3a=np.asarray(inputs['W3'], np.float32)[0:128].astype(BF),
                        w3b=np.asarray(inputs['W3'], np.float32)[128:196].astype(BF),
                        w4blk=w4blk.astype(BF),
                        b1=np.asarray(inputs['b1'], np.float32),
                        b2=np.asarray(inputs['b2'], np.float32),
                        b3=np.asarray(inputs['b3'], np.float32),
                        b4r=b4r))
    rC = run_bass_kernel_spmd(ncC, inC, core_ids=list(range(NCORE)))
    tC = rC.exec_time_ns or 0

    out = np.zeros((n, 1), np.float32)
    for ci, s in enumerate(scheds):
        y = rC.results[ci]['out'].reshape(nmc * 512)
        orig = s['order']
        valid = orig < nloc
        idx = np.arange(nt * P)[valid]
        out[s['core'] * nloc + orig[valid], 0] = y[idx]
    kernel.last_exec_ns = tA + tB + tC
    kernel.last_t12 = (tA, tB, tC)
    kernel.last_results = (rA, rB, rC)
    return out


# revision 42
# speedup vs baseline: 1.2135x; 1.1083x over previous
"""GATv2 x3 + MLP (nn_GAT) on trn2, 8 NeuronCores.

v5 design: attn folded into projection weights (leaky_relu is positively
homogeneous; negative attn handled by a slope-5 prelu + per-node unscale),
l-innermost edge stream so every DVE op runs in 2x mode, den-reduce on the
Pool engine, MLP tail batched (stacked 14->1 matmul + one sigmoid pass).

 - Launch A: project x -> Y|fd''|res per node (block-diag matmuls);
   Y = attn-scaled source projection, fd'' = attn-scaled dest projection.
 - Host gathers Y[src] per edge into a node-major padded-ELL stream with
   edge slots INNERMOST: [P, S, C, L].
 - Launch B: GAT layer-1: z = Y[src]+fd''[dst] (pair-trick broadcast keeps
   2x), prelu with per-sign-run alphas, score tree (4 wide strided adds),
   exp, weighted feats in-place, halving reduce over slots; den reduce on
   gpsimd.  Fixup: pad-slot den fix, normalize, unscale, residual, elu.
   Epilogue projects fs2''|fd2''|res2 on the otherwise idle PE.
 - Launch C: d2 GAT pipeline + 14->196->196->14->1 MLP.  W2 fp8 DoubleRow;
   r3 staged so the 14->1 matmul runs 8 chunks per instruction and all
   sigmoids run in one table-load at the end.

Host only reorders/replicates/casts device-computed tensor bytes; the only
host arithmetic is on the tiny weight matrices (attn folding).
"""
import sys
sys.path.insert(0, '/opt/trn_rl_repo')
import numpy as np
import ml_dtypes

import concourse.bass as bass
import concourse.mybir as mybir
from concourse import bacc
from concourse.tile import TileContext
from concourse.bass_utils import run_bass_kernel_spmd
from concourse.masks import make_identity

bf16 = mybir.dt.bfloat16
fp8 = mybir.dt.float8e4
f32 = mybir.dt.float32
BF = ml_dtypes.bfloat16
FP8 = ml_dtypes.float8_e4m3
AL = mybir.AluOpType
AF = mybir.ActivationFunctionType
MPM = mybir.MatmulPerfMode

NCORE = 8
P = 128
SUPER = 8           # tiles per supertile
NEG_GAT = 0.2
NEG_MLP = 0.01
FP8_W2 = True


# ================================================================= host prep
def build_schedule(dst, n, S):
    nloc = n // NCORE
    core_of = dst // nloc
    scheds = []
    for c in range(NCORE):
        em = np.where(core_of == c)[0]
        ldst = dst[em] - c * nloc
        deg = np.bincount(ldst, minlength=nloc)
        nt = -(-nloc // P)
        nt = -(-nt // 16) * 16
        degp = np.concatenate([deg, np.zeros(nt * P - nloc, np.int64)])
        order = np.argsort(-degp, kind='stable')
        pos_of = np.empty_like(order)
        pos_of[order] = np.arange(len(order))
        scheds.append(dict(core=c, em=em, ldst=ldst, deg=degp, order=order,
                           pos_of=pos_of, nt=nt, nloc=nloc, S=S))
    nt = scheds[0]['nt']
    nst = nt // S
    Ls = []
    for st in range(nst):
        L = 2
        for s in scheds:
            L = max(L, int(s['deg'][s['order'][st * S * P]]))
        L = -(-L // 8) * 8   # mult-8: keeps halving-tree levels in DVE 2x mode
        Ls.append(L)
    return scheds, nst, Ls


def edge_slot_geom(s, Ls):
    """Per edge (in eo order): supertile, tile-in-supertile, slot rank, row."""
    order, deg = s['order'], s['deg']
    S = s['S']
    pos_e = s['pos_of'][s['ldst']]
    eo = np.lexsort((np.arange(len(pos_e)), pos_e))
    pos_sorted = pos_e[eo]
    starts = np.concatenate([[0], np.cumsum(deg[order])])
    rank = np.arange(len(eo)) - starts[pos_sorted]
    t_of = pos_sorted // P
    st_of = t_of // S
    p_of = pos_sorted % P
    return (eo, st_of.astype(np.int64), (t_of % S).astype(np.int64),
            rank.astype(np.int64), p_of.astype(np.int64))


def pack_G(vals_bf, st_of, s_of, rank, p_of, offsC, C, Ls, totc):
    """l-innermost: col = offs[st] + s*(C*L) + c*L + rank."""
    buf = np.zeros((P, totc), BF)
    L_e = np.asarray(Ls)[st_of]
    base = np.asarray(offsC)[st_of] + s_of * (C * L_e) + rank
    for c in range(C):
        buf[p_of, base + c * L_e] = vals_bf[:, c]
    return buf


def make_npad(s, Ls, nt):
    L_t = np.repeat(np.asarray(Ls, np.int64), s['S'])
    d = s['deg'][s['order']].reshape(nt, P)
    return np.ascontiguousarray((L_t[:, None] - d).T).astype(np.float32)


def pack_local(vals, nrow, nt):
    pk = np.zeros((8 * nrow, (nt // 8) * P), BF)
    nodes = np.arange(nt * P)
    a = (nodes // P) % 8
    col = (nodes // (8 * P)) * P + nodes % P
    v = vals.astype(BF)
    for f in range(nrow - 1):
        pk[a * nrow + f, col] = v[:, f]
    pk[a * nrow + (nrow - 1), col] = BF(1.0)
    return pk


def blockdiag(w, bias, nrow, sp=16):
    bd = np.zeros((8 * nrow, 8 * sp), np.float32)
    k = w.shape[1]
    for a in range(8):
        bd[a * nrow:a * nrow + w.shape[0], a * sp:a * sp + k] = w
        bd[a * nrow + nrow - 1, a * sp:a * sp + k] = bias
    return bd.astype(BF)


def pm(vals, nt):
    d = vals.shape[1]
    return np.ascontiguousarray(
        vals.reshape(nt, P, d).transpose(1, 0, 2).reshape(P, nt * d))


def pm_pair(vals, nt):
    """[nt*P, d] -> [P, nt*d*2] with each channel duplicated (pair trick)."""
    d = vals.shape[1]
    v = vals.reshape(nt, P, d).transpose(1, 0, 2)        # [P, nt, d]
    v2 = np.repeat(v, 2, axis=2)                          # [P, nt, 2d]
    return np.ascontiguousarray(v2.reshape(P, nt * d * 2))


def attn_fold(attn_hf, H, F):
    """Per (h,f): permuted order (pos-signs first within each head),
    channel scale, prelu alpha.  Returns (perm j-list, scale, alpha)."""
    perm, scale, alpha = [], [], []
    for h in range(H):
        # alternate pos-first / neg-first per head so prelu alpha-runs merge
        # across head boundaries (fewer Act instructions)
        first_pos = (h % 2 == 0)
        fs = sorted(range(F),
                    key=lambda f: 0 if (attn_hf[h, f] > 0) == first_pos else 1)
        for f in fs:
            a = float(attn_hf[h, f])
            if a > 0:
                aa = max(a, 1e-8)
                perm.append(h * F + f); scale.append(aa); alpha.append(NEG_GAT)
            else:
                aa = min(a, -1e-8)
                perm.append(h * F + f); scale.append(NEG_GAT * aa); alpha.append(1.0 / NEG_GAT)
    return perm, np.asarray(scale, np.float64), alpha


def alpha_runs(alphas):
    runs = []
    i = 0
    while i < len(alphas):
        j = i
        while j < len(alphas) and alphas[j] == alphas[i]:
            j += 1
        runs.append((i, j, float(alphas[i])))
        i = j
    return runs


# ================================================================ device bits
def halving_tree(tt, X, L, out_final):
    """In-place halving over innermost axis of X [P,...,L]; final add -> out_final."""
    cur = L
    while cur > 2:
        h = cur // 2
        tt(out=X[..., 0:h], in0=X[..., 0:h], in1=X[..., cur - h:cur], op=AL.add)
        cur -= h
    if cur == 2:
        tt(out=out_final, in0=X[..., 0:1], in1=X[..., 1:2], op=AL.add)
    else:
        tt(out=out_final, in0=X[..., 0:1], in1=X[..., 0:1], op=AL.bypass)


def pp_tree(nc, A, B, L, out_final, cp=None):
    """Ping-pong halving reduce over innermost axis: A [P,..,L] (input, even L),
    B [P,..,>=L/2] scratch.  Output buffer alternates so out never shares a
    buffer with an input (keeps the DVE 2x mode).  cp: engine copy fn for the
    odd middle element (defaults to vector tensor_copy)."""
    cp = cp or nc.vector.tensor_copy
    tt = nc.vector.tensor_tensor
    src, dst = A, B
    cur = L
    while cur > 2:
        h = cur // 2
        tt(out=dst[..., 0:h], in0=src[..., 0:h], in1=src[..., h:2 * h], op=AL.add)
        if cur & 1:
            cp(out=dst[..., h:h + 1], in_=src[..., 2 * h:2 * h + 1])
            cur = h + 1
        else:
            cur = h
        src, dst = dst, src
    if cur == 2:
        tt(out=out_final, in0=src[..., 0:1], in1=src[..., 1:2], op=AL.add)
    else:
        cp(out=out_final, in_=src[..., 0:1])


def emit_elu(nc, sbS, hflat, nelem, tag):
    tmp = sbS.tile([P, nelem], f32, tag=tag)
    nc.gpsimd.tensor_scalar_min(out=tmp[:], in0=hflat, scalar1=0.0)
    nc.scalar.activation(out=tmp[:], in_=tmp[:], func=AF.Exp)
    nc.gpsimd.tensor_scalar(out=hflat, in0=hflat, scalar1=0.0, scalar2=-1.0,
                            op0=AL.max, op1=AL.add)
    nc.gpsimd.tensor_tensor(out=hflat, in0=hflat, in1=tmp[:], op=AL.add)


def emit_B_stage1(nc, sbG, sbZ, d_g, off, L, S, fdp_ap):
    """Stage 1: DMA + z-add (DVE work with no cross-engine wait)."""
    C = 12
    L2 = L // 2
    ncols = S * C * L
    G = sbG.tile([P, ncols], bf16, tag="G")
    nc.sync.dma_start(out=G[:], in_=d_g[:, off:off + ncols])
    G4 = G[:].rearrange("p (s c l) -> p s c l", s=S, c=C, l=L)
    G5 = G[:].rearrange("p (sc l2 j) -> p sc l2 j", sc=S * C, l2=L2, j=2)
    Z = sbZ.tile([P, ncols], bf16, tag="Z")
    Z4 = Z[:].rearrange("p (s c l) -> p s c l", s=S, c=C, l=L)
    Z5 = Z[:].rearrange("p (sc l2 j) -> p sc l2 j", sc=S * C, l2=L2, j=2)
    fdb = fdp_ap.rearrange("p s c j -> p (s c) j").unsqueeze(2) \
        .broadcast_to([P, S * C, L2, 2])
    nc.vector.tensor_tensor(out=Z5, in0=G5, in1=fdb, op=AL.add)
    return dict(G4=G4, Z4=Z4, L=L, S=S)


def emit_B_stage2a(nc, sbEX, sbT, ctx, runs):
    """Stage 2a: prelu (Act), score tree (DVE), exp (Act)."""
    Z4, L, S = ctx['Z4'], ctx['L'], ctx['S']
    for (c0, c1, al) in runs:
        nc.scalar.activation(out=Z4[:, :, c0:c1, :], in_=Z4[:, :, c0:c1, :],
                             func=AF.Prelu, alpha=al)
    T = sbT.tile([P, S * 4 * L], bf16, tag="T")
    T4 = T[:].rearrange("p (s c l) -> p s c l", s=S, c=4, l=L)
    Tp = T[:].rearrange("p (s c2 c l) -> p s c2 c l", s=S, c2=2, c=2, l=L)
    nc.vector.tensor_tensor(out=T4[:, :, 0:2, :], in0=Z4[:, :, 0:2, :],
                            in1=Z4[:, :, 2:4, :], op=AL.add)
    nc.vector.tensor_tensor(out=T4[:, :, 2:4, :], in0=Z4[:, :, 5:7, :],
                            in1=Z4[:, :, 7:9, :], op=AL.add)
    SC = sbT.tile([P, S * 2 * L], bf16, tag="SC")
    SC3 = SC[:].rearrange("p (s c l) -> p s c l", s=S, c=2, l=L)
    nc.vector.tensor_tensor(out=SC3, in0=Tp[:, :, :, 0, :],
                            in1=Tp[:, :, :, 1, :], op=AL.add)
    Zh = Z4[:, :, 0:10, :].rearrange("p s (h f) l -> p s h f l", h=2, f=5)
    nc.vector.tensor_tensor(out=SC3, in0=SC3, in1=Zh[:, :, :, 4, :], op=AL.add)
    EX = sbEX.tile([P, S * 4 * L], bf16, tag="EX")
    EX4 = EX[:].rearrange("p (s c l) -> p s c l", s=S, c=4, l=L)
    nc.scalar.activation(out=EX4[:, :, 0:2, :], in_=SC3, func=AF.Exp)
    nc.scalar.activation(out=EX4[:, :, 2:4, :], in_=Z4[:, :, 10:12, :], func=AF.Exp)
    ctx['EX4'] = EX4
    ctx['T4'] = T4


def emit_B_stage2b(nc, ctx, h1v, denv):
    """Stage 2b: weighted feats + both reduces (pure DVE, exp long done)."""
    G4, Z4, EX4, T4, L, S = (ctx['G4'], ctx['Z4'], ctx['EX4'], ctx['T4'],
                             ctx['L'], ctx['S'])
    for h in range(2):
        Gh = G4[:, :, 5 * h:5 * h + 5, :]
        exd = EX4[:, :, h:h + 1, :].broadcast_to([P, S, 5, L])
        nc.vector.tensor_tensor(out=Gh, in0=Gh, in1=exd, op=AL.mult)
    nc.vector.tensor_tensor(out=G4[:, :, 10:12, :], in0=G4[:, :, 10:12, :],
                            in1=EX4[:, :, 2:4, :], op=AL.mult)
    pp_tree(nc, G4, Z4[:, :, :, 0:L // 2 + 1], L, h1v.unsqueeze(3))
    pp_tree(nc, EX4, T4[:, :, :, 0:L // 2 + 1], L, denv.unsqueeze(3))


def emit_C_stage1(nc, sbG, sbZ, d_g, off, L, S, fdp_ap):
    """Layer-2 stage 1: DMA + z-add."""
    C = 4
    L2 = L // 2
    ncols = S * C * L
    G = sbG.tile([P, ncols], bf16, tag="G")
    nc.sync.dma_start(out=G[:], in_=d_g[:, off:off + ncols])
    G4 = G[:].rearrange("p (s c l) -> p s c l", s=S, c=C, l=L)
    G5 = G[:].rearrange("p (sc l2 j) -> p sc l2 j", sc=S * C, l2=L2, j=2)
    Z = sbZ.tile([P, ncols], bf16, tag="Z")
    Z4 = Z[:].rearrange("p (s c l) -> p s c l", s=S, c=C, l=L)
    Z5 = Z[:].rearrange("p (sc l2 j) -> p sc l2 j", sc=S * C, l2=L2, j=2)
    fdb = fdp_ap.rearrange("p s c j -> p (s c) j").unsqueeze(2) \
        .broadcast_to([P, S * C, L2, 2])
    nc.vector.tensor_tensor(out=Z5, in0=G5, in1=fdb, op=AL.add)
    return dict(G4=G4, Z4=Z4, L=L, S=S)


def emit_C_stage2a(nc, sbEX, sbT, ctx, runs):
    """Layer-2 stage 2a: prelu, score sum, exp."""
    Z4, L, S = ctx['Z4'], ctx['L'], ctx['S']
    for (c0, c1, al) in runs:
        nc.scalar.activation(out=Z4[:, :, c0:c1, :], in_=Z4[:, :, c0:c1, :],
                             func=AF.Prelu, alpha=al)
    Zp = Z4.rearrange("p s (h f) l -> p s h f l", h=2, f=2)
    SC = sbT.tile([P, S * 2 * L], bf16, tag="SC")
    SC3 = SC[:].rearrange("p (s c l) -> p s c l", s=S, c=2, l=L)
    nc.vector.tensor_tensor(out=SC3, in0=Zp[:, :, :, 0, :],
                            in1=Zp[:, :, :, 1, :], op=AL.add)
    EX = sbEX.tile([P, S * 2 * L], bf16, tag="EX")
    EX3 = EX[:].rearrange("p (s c l) -> p s c l", s=S, c=2, l=L)
    nc.scalar.activation(out=EX3, in_=SC3, func=AF.Exp)
    ctx['EX3'] = EX3
    ctx['SC3'] = SC3


def emit_C_stage2b(nc, ctx, h2v, denv):
    """Layer-2 stage 2b: weighted feats + reduces."""
    G4, Z4, EX3, SC3, L, S = (ctx['G4'], ctx['Z4'], ctx['EX3'], ctx['SC3'],
                              ctx['L'], ctx['S'])
    for h in range(2):
        Gh = G4[:, :, 2 * h:2 * h + 2, :]
        exd = EX3[:, :, h:h + 1, :].broadcast_to([P, S, 2, L])
        nc.vector.tensor_tensor(out=Gh, in0=Gh, in1=exd, op=AL.mult)
    pp_tree(nc, G4, Z4[:, :, :, 0:L // 2 + 1], L, h2v.unsqueeze(3))
    pp_tree(nc, EX3, SC3[:, :, :, 0:L // 2 + 1], L, denv.unsqueeze(3))


def emit_fixup(nc, sbS, hv, hflat, dv, dflat, fd_ap, rs_flat, np_ap, isc_ap,
               T, C, NH, dF, runs, do_elu=True):
    """Pad-slot den fix + normalize + unscale + residual + elu over T tiles.
    fd_ap: [P, T, C] per-node scaled dest proj; isc_ap: [P, C] inv scales."""
    zp = sbS.tile([P, T * C], bf16, tag="zp")
    zp3 = zp[:].rearrange("p (t c) -> p t c", t=T, c=C)
    for (c0, c1, al) in runs:
        nc.scalar.activation(out=zp3[:, :, c0:c1], in_=fd_ap[:, :, c0:c1],
                             func=AF.Prelu, alpha=al)
    ep = sbS.tile([P, T * NH], bf16, tag="ep")
    ep3 = ep[:].rearrange("p (t h) -> p t h", t=T, h=NH)
    if C == 12:
        zph = zp3[:, :, 0:10].rearrange("p t (h f) -> p t h f", h=2, f=5)
        tp = sbS.tile([P, T * 2], bf16, tag="tp")
        tp3 = tp[:].rearrange("p (t h) -> p t h", t=T, h=2)
        nc.vector.tensor_tensor(out=tp3, in0=zph[:, :, :, 0], in1=zph[:, :, :, 1], op=AL.add)
        nc.vector.tensor_tensor(out=tp3, in0=tp3, in1=zph[:, :, :, 2], op=AL.add)
        nc.vector.tensor_tensor(out=tp3, in0=tp3, in1=zph[:, :, :, 3], op=AL.add)
        nc.vector.tensor_tensor(out=tp3, in0=tp3, in1=zph[:, :, :, 4], op=AL.add)
        nc.scalar.activation(out=ep3[:, :, 0:2], in_=tp3, func=AF.Exp)
        nc.scalar.activation(out=ep3[:, :, 2:4], in_=zp3[:, :, 10:12], func=AF.Exp)
    else:
        zph = zp3.rearrange("p t (h f) -> p t h f", h=2, f=2)
        tp = sbS.tile([P, T * 2], bf16, tag="tp")
        tp3 = tp[:].rearrange("p (t h) -> p t h", t=T, h=2)
        nc.vector.tensor_tensor(out=tp3, in0=zph[:, :, :, 0], in1=zph[:, :, :, 1], op=AL.add)
        nc.scalar.activation(out=ep3, in_=tp3, func=AF.Exp)
    padm = sbS.tile([P, T * NH], f32, tag="padm")
    pm3 = padm[:].rearrange("p (t h) -> p t h", t=T, h=NH)
    npb = np_ap.unsqueeze(2).broadcast_to([P, T, NH])
    nc.gpsimd.tensor_tensor(out=pm3, in0=ep3, in1=npb, op=AL.mult)
    nc.gpsimd.tensor_tensor(out=dv, in0=dv, in1=pm3, op=AL.subtract)
    nc.gpsimd.tensor_scalar_max(out=dflat, in0=dflat, scalar1=1e-30)
    rec = sbS.tile([P, T * NH], f32, tag="rec")
    nc.vector.reciprocal(out=rec[:], in_=dflat)
    rec3 = rec[:].rearrange("p (t h) -> p t h", t=T, h=NH)
    # rec12 = rec[h(c)] * inv_scale_c
    rc = sbS.tile([P, T * C], f32, tag="rc")
    rc3 = rc[:].rearrange("p (t c) -> p t c", t=T, c=C)
    iscb = isc_ap.unsqueeze(1).broadcast_to([P, T, C])
    if C == 12:
        rch = rc3[:, :, 0:10].rearrange("p t (h f) -> p t h f", h=2, f=5)
        rb = rec3[:, :, 0:2].unsqueeze(3).broadcast_to([P, T, 2, 5])
        ib = iscb[:, :, 0:10].rearrange("p t (h f) -> p t h f", h=2, f=5)
        nc.gpsimd.tensor_tensor(out=rch, in0=rb, in1=ib, op=AL.mult)
        nc.gpsimd.tensor_tensor(out=rc3[:, :, 10:12], in0=rec3[:, :, 2:4],
                                in1=iscb[:, :, 10:12], op=AL.mult)
    else:
        rch = rc3.rearrange("p t (h f) -> p t h f", h=2, f=2)
        rb = rec3.unsqueeze(3).broadcast_to([P, T, 2, 2])
        ib = iscb.rearrange("p t (h f) -> p t h f", h=2, f=2)
        nc.gpsimd.tensor_tensor(out=rch, in0=rb, in1=ib, op=AL.mult)
    nc.gpsimd.tensor_tensor(out=hflat, in0=hflat, in1=rc[:], op=AL.mult)
    nc.gpsimd.tensor_tensor(out=hflat, in0=hflat, in1=rs_flat, op=AL.add)
    if do_elu:
        emit_elu(nc, sbS, hflat, T * C, "elu")


# =============================================================== launch A
def build_launchA(nt):
    cols = nt * 16
    nc = bacc.Bacc("TRN2", target_bir_lowering=False, debug=False, num_devices=NCORE)
    d_x = nc.dram_tensor("x5l", [48, cols], bf16, kind="ExternalInput")
    d_bfs = nc.dram_tensor("bd_fs", [48, P], bf16, kind="ExternalInput")
    d_bfd = nc.dram_tensor("bd_fd", [48, P], bf16, kind="ExternalInput")
    d_brs = nc.dram_tensor("bd_rs", [48, P], bf16, kind="ExternalInput")
    d_fs = nc.dram_tensor("fs1cm", [P, cols], bf16, kind="ExternalOutput")
    d_fd = nc.dram_tensor("fd1cm", [P, cols], bf16, kind="ExternalOutput")
    d_rs = nc.dram_tensor("rs1cm", [P, cols], f32, kind="ExternalOutput")
    with TileContext(nc) as tc:
        with tc.tile_pool(name="res", bufs=1) as res, \
             tc.tile_pool(name="ps", bufs=2, space="PSUM") as ps:
            stg = res.tile([48, cols], bf16)
            nc.sync.dma_start(out=stg[:], in_=d_x[:, :])
            bfs = res.tile([48, P], bf16, tag="bfs")
            nc.sync.dma_start(out=bfs[:], in_=d_bfs[:, :])
            bfd = res.tile([48, P], bf16, tag="bfd")
            nc.sync.dma_start(out=bfd[:], in_=d_bfd[:, :])
            brs = res.tile([48, P], bf16, tag="brs")
            nc.sync.dma_start(out=brs[:], in_=d_brs[:, :])
            ofs = res.tile([P, cols], bf16, tag="ofs")
            ofd = res.tile([P, cols], bf16, tag="ofd")
            ors = res.tile([P, cols], f32, tag="ors")
            k = 0
            for j0 in range(0, cols, 512):
                w = min(512, cols - j0)
                for bd, ot, dt_ in ((bfs, ofs, d_fs), (bfd, ofd, d_fd),
                                    (brs, ors, d_rs)):
                    pmm = ps.tile([P, 512], f32, tag="mm")
                    nc.tensor.matmul(out=pmm[:, :w], lhsT=bd[:], rhs=stg[:, j0:j0 + w],
                                     start=True, stop=True)
                    if k % 2 == 0:
                        nc.vector.tensor_copy(out=ot[:, j0:j0 + w], in_=pmm[:, :w])
                    else:
                        nc.scalar.copy(out=ot[:, j0:j0 + w], in_=pmm[:, :w])
                    # stream each slice out as soon as it drains (overlaps the
                    # remaining matmuls instead of three serial DMAs at the end)
                    nc.sync.dma_start(out=dt_[:, j0:j0 + w], in_=ot[:, j0:j0 + w])
                    k += 1
    nc.compile()
    return nc


# =============================================================== launch B
def build_launchB(nst, Ls, offs12, nt, runs12, SUPER=8):
    totc = int(offs12[-1])
    fgw = -(-nst // 4) * SUPER * P          # f2 output column width
    nc = bacc.Bacc("TRN2", target_bir_lowering=False, debug=False, num_devices=NCORE)
    d_g = nc.dram_tensor("g1", [P, totc], bf16, kind="ExternalInput")
    d_fdp = nc.dram_tensor("fdp1", [P, nt * 24], bf16, kind="ExternalInput")
    d_rs = nc.dram_tensor("rs1n", [P, nt * 12], f32, kind="ExternalInput")
    d_np = nc.dram_tensor("npad", [P, nt], f32, kind="ExternalInput")
    d_isc = nc.dram_tensor("isc12", [P, 12], f32, kind="ExternalInput")
    d_w2 = nc.dram_tensor("w2all", [10, 12], bf16, kind="ExternalInput")
    d_bc2 = nc.dram_tensor("bc2", [12], f32, kind="ExternalInput")
    d_f2 = nc.dram_tensor("f2cm", [48, fgw], f32, kind="ExternalOutput")
    d_ha = nc.dram_tensor("hattn", [P, nt * 2], bf16, kind="ExternalOutput")
    groups = ([(0, 6), (6, 11), (11, 16), (16, 20), (20, 23), (23, 25), (25, nst)]
              if nst >= 25 else [(0, nst)])  # 8-tile sts, shrinking tail
    with TileContext(nc) as tc:
        with tc.tile_pool(name="res", bufs=1) as res, \
             tc.tile_pool(name="sbG", bufs=4) as sbG, \
             tc.tile_pool(name="sbZ", bufs=4) as sbZ, \
             tc.tile_pool(name="sbEX", bufs=4) as sbEX, \
             tc.tile_pool(name="sbT", bufs=2) as sbT, \
             tc.tile_pool(name="sbS", bufs=1) as sbS, \
             tc.tile_pool(name="sbT2", bufs=1) as sbT2, \
             tc.tile_pool(name="psT", bufs=1, space="PSUM") as psT, \
             tc.tile_pool(name="psF", bufs=1, space="PSUM") as psF:
            ident = res.tile([P, P], bf16)
            make_identity(nc, ident[:])
            npad = res.tile([P, nt], f32)
            nc.sync.dma_start(out=npad[:], in_=d_np[:, :])
            isc = res.tile([P, 12], f32)
            nc.sync.dma_start(out=isc[:], in_=d_isc[:, :])
            fdp = res.tile([P, nt * 24], bf16)
            nc.sync.dma_start(out=fdp[:], in_=d_fdp[:, :])
            rst = res.tile([P, nt * 12], f32)
            nc.sync.dma_start(out=rst[:], in_=d_rs[:, :])
            w2t = res.tile([10, 12], bf16, tag="w2t")
            nc.sync.dma_start(out=w2t[:], in_=d_w2[:, :])
            bc2 = res.tile([12, 1], f32, tag="bc2")
            nc.sync.dma_start(out=bc2[:], in_=d_bc2[:, None])
            h1 = res.tile([P, nt * 12], f32)
            den = res.tile([P, nt * 4], f32)
            hat = res.tile([P, nt * 2], bf16)
            h1v_all = h1[:].rearrange("p (t c) -> p t c", t=nt, c=12)
            denv_all = den[:].rearrange("p (t c) -> p t c", t=nt, c=4)
            fdp_all = fdp[:].rearrange("p (t c j) -> p t c j", t=nt, c=12, j=2)
            npv_all = npad[:].rearrange("p (t o) -> p t o", t=nt, o=1)
            ctxs = []

            def do2a(c):
                emit_B_stage2a(nc, sbEX, sbT, c, runs12)

            def do2b(c):
                emit_B_stage2b(nc, c,
                               h1v_all[:, c['t0']:c['t0'] + SUPER, :],
                               denv_all[:, c['t0']:c['t0'] + SUPER, :])

            group_work = []
            for gidx, (g0, g1) in enumerate(groups):
                def gw(g0=g0, g1=g1):
                    T = (g1 - g0) * SUPER
                    t0 = g0 * SUPER
                    emit_fixup(nc, sbS, h1v_all[:, t0:t0 + T, :],
                               h1[:, t0 * 12:(t0 + T) * 12],
                               denv_all[:, t0:t0 + T, :],
                               den[:, t0 * 4:(t0 + T) * 4],
                               fdp_all[:, t0:t0 + T, :, 0],
                               rst[:, t0 * 12:(t0 + T) * 12],
                               npv_all[:, t0:t0 + T, 0], isc[:], T, 12, 4, 5, runs12)
                    hv = h1v_all[:, t0:t0 + T, :]
                    nc.scalar.copy(out=hat[:].rearrange("p (t c) -> p t c", t=nt, c=2)[:, t0:t0 + T, :],
                                   in_=hv[:, :, 10:12])
                    h1b = sbS.tile([P, T * 10], bf16, tag="h1b")
                    nc.scalar.copy(out=h1b[:].rearrange("p (t c) -> p t c", t=T, c=10),
                                   in_=hv[:, :, 0:10])
                    # epilogue: transpose h_def1 per tile, project fs2''|fd2''|res2
                    for st in range(g0, g1):
                        pT = psT.tile([10, SUPER * P], bf16, tag="pT")
                        for b in range(SUPER):
                            trel = (st - g0) * SUPER + b
                            nc.tensor.transpose(out=pT[:, b * P:(b + 1) * P],
                                                in_=h1b[:, trel * 10:trel * 10 + 10],
                                                identity=ident[:])
                        hT = sbT2.tile([10, SUPER * P], bf16, tag="hT")
                        nc.scalar.copy(out=hT[:], in_=pT[:])
                        pF = psF.tile([12, SUPER * P], f32, tag="pF")
                        for q in range(SUPER * P // 512):
                            nc.tensor.matmul(out=pF[:, q * 512:(q + 1) * 512], lhsT=w2t[:],
                                             rhs=hT[:, q * 512:(q + 1) * 512],
                                             start=True, stop=True)
                        f2s = sbT2.tile([12, SUPER * P], f32, tag="f2s")
                        nc.scalar.activation(out=f2s[:], in_=pF[:], func=AF.Prelu,
                                             alpha=1.0, bias=bc2[:])
                        nc.sync.dma_start(
                            out=d_f2[12 * (st % 4):12 * (st % 4) + 12,
                                     (st // 4) * SUPER * P:(st // 4 + 1) * SUPER * P],
                            in_=f2s[:])
                group_work.append((g1, gw))

            # 3-stage software pipeline across ALL supertiles; fixup+epilogue
            # of each group emitted as soon as its last supertile completes 2b
            gi = 0
            for k in range(nst):
                t0 = k * SUPER
                ctx = emit_B_stage1(nc, sbG, sbZ, d_g, int(offs12[k]),
                                    Ls[k], SUPER, fdp_all[:, t0:t0 + SUPER])
                ctx['t0'] = t0
                ctxs.append(ctx)
                if k >= 1:
                    do2a(ctxs[k - 1])
                if k >= 2:
                    do2b(ctxs[k - 2])
                    while gi < len(group_work) and group_work[gi][0] <= k - 1:
                        group_work[gi][1]()
                        gi += 1
            do2a(ctxs[nst - 1])
            do2b(ctxs[nst - 2])
            do2b(ctxs[nst - 1])
            while gi < len(group_work):
                group_work[gi][1]()
                gi += 1
            nc.sync.dma_start(out=d_ha[:, :], in_=hat[:])
    nc.compile()
    return nc


# =============================================================== launch C
def build_launchC(nst, Ls, offs4, nt, nmc, runs4, SUPER=16):
    totc = int(offs4[-1])
    ngrp = -(-nmc // 4)
    nc = bacc.Bacc("TRN2", target_bir_lowering=False, debug=False, num_devices=NCORE)
    d_g = nc.dram_tensor("g2", [P, totc], bf16, kind="ExternalInput")
    d_fdp = nc.dram_tensor("fdp2", [P, nt * 8], bf16, kind="ExternalInput")
    d_rs = nc.dram_tensor("rs2n", [P, nt * 4], f32, kind="ExternalInput")
    d_np = nc.dram_tensor("npad", [P, nt], f32, kind="ExternalInput")
    d_isc = nc.dram_tensor("isc4", [P, 4], f32, kind="ExternalInput")
    d_ha = nc.dram_tensor("hattn", [P, nt * 2], bf16, kind="ExternalInput")
    d_x = nc.dram_tensor("xpm", [P, nt * 8], bf16, kind="ExternalInput")
    d_w1 = nc.dram_tensor("w1x", [32, 196], bf16, kind="ExternalInput")
    w2dt = fp8 if FP8_W2 else bf16
    d_w2a = nc.dram_tensor("w2dra", [P, 2 * 128], w2dt, kind="ExternalInput")
    d_w2b = nc.dram_tensor("w2drb", [P, 2 * 128], w2dt, kind="ExternalInput")
    d_w3a = nc.dram_tensor("w3a", [P, 14], bf16, kind="ExternalInput")
    d_w3b = nc.dram_tensor("w3b", [68, 14], bf16, kind="ExternalInput")
    d_w4 = nc.dram_tensor("w4blk", [110, 4], bf16, kind="ExternalInput")
    d_b1 = nc.dram_tensor("b1", [196], f32, kind="ExternalInput")
    d_b2 = nc.dram_tensor("b2", [196], f32, kind="ExternalInput")
    d_b3 = nc.dram_tensor("b3", [14], f32, kind="ExternalInput")
    d_b4 = nc.dram_tensor("b4r", [4], f32, kind="ExternalInput")
    d_out = nc.dram_tensor("out", [nmc, 512], f32, kind="ExternalOutput")
    groups = ([(i, min(i + 2, nst)) for i in range(0, nst - 3, 2)]
              + [(nst - 3, nst - 2), (nst - 2, nst - 1), (nst - 1, nst)]
              if nst >= 11 else [(0, nst)])
    with TileContext(nc) as tc:
        with tc.tile_pool(name="res", bufs=1) as res, \
             tc.tile_pool(name="sbG", bufs=3) as sbG, \
             tc.tile_pool(name="sbZ", bufs=3) as sbZ, \
             tc.tile_pool(name="sbEX", bufs=3) as sbEX, \
             tc.tile_pool(name="sbT", bufs=2) as sbT, \
             tc.tile_pool(name="sbS", bufs=1) as sbS, \
             tc.tile_pool(name="sbM", bufs=2) as sbM, \
             tc.tile_pool(name="psT", bufs=1, space="PSUM") as psT, \
             tc.tile_pool(name="psC", bufs=1, space="PSUM") as psC, \
             tc.tile_pool(name="psA", bufs=2, space="PSUM") as psA, \
             tc.tile_pool(name="psB", bufs=1, space="PSUM") as psB:
            ident = res.tile([P, P], bf16)
            make_identity(nc, ident[:])
            npad = res.tile([P, nt], f32)
            nc.sync.dma_start(out=npad[:], in_=d_np[:, :])
            isc = res.tile([P, 4], f32)
            nc.sync.dma_start(out=isc[:], in_=d_isc[:, :])
            fdp = res.tile([P, nt * 8], bf16)
            nc.sync.dma_start(out=fdp[:], in_=d_fdp[:, :])
            rst = res.tile([P, nt * 4], f32)
            nc.sync.dma_start(out=rst[:], in_=d_rs[:, :])
            hat = res.tile([P, nt * 2], bf16)
            nc.sync.dma_start(out=hat[:], in_=d_ha[:, :])
            xpm = res.tile([P, nt * 8], bf16)
            nc.sync.dma_start(out=xpm[:], in_=d_x[:, :])
            w1 = res.tile([32, 196], bf16, tag="w1")
            nc.sync.dma_start(out=w1[:], in_=d_w1[:, :])
            w2a = res.tile([P, 2 * 128], w2dt, tag="w2a")
            nc.sync.dma_start(out=w2a[:], in_=d_w2a[:, :])
            w2b = res.tile([P, 2 * 128], w2dt, tag="w2b")
            nc.sync.dma_start(out=w2b[:], in_=d_w2b[:, :])
            w3a = res.tile([P, 14], bf16, tag="w3a")
            nc.sync.dma_start(out=w3a[:], in_=d_w3a[:, :])
            w3b = res.tile([68, 14], bf16, tag="w3b")
            nc.sync.dma_start(out=w3b[:], in_=d_w3b[:, :])
            w4b = res.tile([110, 4], bf16, tag="w4b")
            nc.sync.dma_start(out=w4b[:], in_=d_w4[:, :])
            w2av = w2a[:].rearrange("p (k m) -> p k m", k=2, m=128)
            w2bv = w2b[:].rearrange("p (k m) -> p k m", k=2, m=128)
            b1ca = res.tile([P, 1], f32, tag="b1ca")
            nc.sync.dma_start(out=b1ca[:], in_=d_b1[0:128, None])
            b1cb = res.tile([68, 1], f32, tag="b1cb")
            nc.sync.dma_start(out=b1cb[:], in_=d_b1[128:196, None])
            b2ca = res.tile([P, 1], f32, tag="b2ca")
            nc.sync.dma_start(out=b2ca[:], in_=d_b2[0:128, None])
            b2cb = res.tile([68, 1], f32, tag="b2cb")
            nc.sync.dma_start(out=b2cb[:], in_=d_b2[128:196, None])
            b3c = res.tile([14, 1], f32, tag="b3c")
            nc.sync.dma_start(out=b3c[:], in_=d_b3[:, None])
            b4c = res.tile([4, 1], f32, tag="b4c")
            nc.sync.dma_start(out=b4c[:], in_=d_b4[:, None])
            h2 = res.tile([P, nt * 4], f32)
            den = res.tile([P, nt * 2], f32)
            m32 = res.tile([P, nt * 32], bf16)
            nc.gpsimd.memset(m32[:], 0.0)
            r3st = res.tile([110, ngrp * 512], bf16, tag="r3st")
            adt = fp8 if FP8_W2 else bf16
            r1t = [res.tile([P, 2048], adt, tag=f"r1_{i}", name=f"r1_{i}")
                   for i in range(2)]
            for t_ in r1t:
                nc.gpsimd.memset(t_[:], 0.0)
            h2v_all = h2[:].rearrange("p (t c) -> p t c", t=nt, c=4)
            denv_all = den[:].rearrange("p (t c) -> p t c", t=nt, c=2)
            fdp_all = fdp[:].rearrange("p (t c j) -> p t c j", t=nt, c=4, j=2)
            npv_all = npad[:].rearrange("p (t o) -> p t o", t=nt, o=1)
            m3_all = m32[:].rearrange("p (t c) -> p t c", t=nt, c=32)
            hav_all = hat[:].rearrange("p (t c) -> p t c", t=nt, c=2)
            xv_all = xpm[:].rearrange("p (t c) -> p t c", t=nt, c=8)
            nc.vector.tensor_copy(out=m3_all[:, :, 0:2], in_=hav_all)
            nc.vector.tensor_copy(out=m3_all[:, :, 6:14], in_=xv_all)
            for g0, g1 in groups:
                for st in range(g0, g1):
                    L = Ls[st]
                    t0 = st * SUPER
                    emit_gat_st_C(nc, sbG, sbZ, sbEX, sbT, d_g, int(offs4[st]),
                                  L, SUPER, fdp_all[:, t0:t0 + SUPER],
                                  runs4,
                                  h2v_all[:, t0:t0 + SUPER, :],
                                  denv_all[:, t0:t0 + SUPER, :])
                T = (g1 - g0) * SUPER
                t0 = g0 * SUPER
                emit_fixup(nc, sbS, h2v_all[:, t0:t0 + T, :],
                           h2[:, t0 * 4:(t0 + T) * 4],
                           denv_all[:, t0:t0 + T, :],
                           den[:, t0 * 2:(t0 + T) * 2],
                           fdp_all[:, t0:t0 + T, :, 0],
                           rst[:, t0 * 4:(t0 + T) * 4],
                           npv_all[:, t0:t0 + T, 0], isc[:], T, 4, 2, 2, runs4)
                nc.vector.tensor_copy(out=m3_all[:, t0:t0 + T, 2:6],
                                      in_=h2v_all[:, t0:t0 + T, :])
                # MLP over this group's chunks, processed in pairs so each
                # activation instruction covers 1024 nodes (half the Act
                # instruction count)
                mc1 = min((t0 + T) // 4, nmc)
                for mcp in range(t0 // 4, mc1, 2):
                    pair = [mc for mc in (mcp, mcp + 1) if mc < mc1]
                    npair = len(pair)
                    p1a = psA.tile([P, 1024], f32, tag="pA")
                    p1b = psB.tile([P, 1024], f32, tag="pB")
                    for ci_, mc in enumerate(pair):
                        pT = psT.tile([64, 256], bf16, tag="pT")
                        nc.tensor.transpose(out=pT[:, 0:128],
                                            in_=m32[:, (mc * 4) * 32:(mc * 4 + 2) * 32],
                                            identity=ident[:])
                        nc.tensor.transpose(out=pT[:, 128:256],
                                            in_=m32[:, (mc * 4 + 2) * 32:(mc * 4 + 4) * 32],
                                            identity=ident[:])
                        r0 = sbM.tile([32, 512], bf16, tag=f"r0_{ci_}")
                        r0v = r0[:].rearrange("r (b c) -> r b c", b=4, c=128)
                        pTv0 = pT[0:32, :].rearrange("r (b c) -> r b c", b=2, c=128)
                        pTv1 = pT[32:64, :].rearrange("r (b c) -> r b c", b=2, c=128)
                        nc.vector.tensor_copy(out=r0v[:, 0:4:2, :], in_=pTv0)
                        nc.vector.tensor_copy(out=r0v[:, 1:4:2, :], in_=pTv1)
                        nc.tensor.matmul(out=p1a[:, ci_ * 512:(ci_ + 1) * 512],
                                         lhsT=w1[:, 0:128], rhs=r0[:], start=True, stop=True)
                        nc.tensor.matmul(out=p1b[0:68, ci_ * 512:(ci_ + 1) * 512],
                                         lhsT=w1[:, 128:196], rhs=r0[:], start=True, stop=True)
                    r1p = r1t[(mcp // 2) % 2]          # [P, 2048] fp8: cols (k, chunk, 512)
                    w = npair * 512
                    nc.scalar.activation(out=r1p[:, 0:w], in_=p1a[:, 0:w],
                                         func=AF.Prelu, alpha=NEG_MLP, bias=b1ca[:])
                    nc.scalar.activation(out=r1p[0:68, 1024:1024 + w], in_=p1b[0:68, 0:w],
                                         func=AF.Prelu, alpha=NEG_MLP, bias=b1cb[:])
                    r1v = r1p[:].rearrange("p (k c n) -> p k c n", k=2, c=2, n=512)
                    p2a = psA.tile([P, 1024], f32, tag="pA")
                    p2b = psB.tile([P, 1024], f32, tag="pB")
                    for ci_, mc in enumerate(pair):
                        nc.tensor.matmul(out=p2a[:, ci_ * 512:(ci_ + 1) * 512], lhsT=w2av,
                                         rhs=r1v[:, :, ci_, :],
                                         start=True, stop=True, perf_mode=MPM.DoubleRow)
                        nc.tensor.matmul(out=p2b[:, ci_ * 512:(ci_ + 1) * 512], lhsT=w2bv,
                                         rhs=r1v[:, :, ci_, :],
                                         start=True, stop=True, perf_mode=MPM.DoubleRow)
                    r2a = sbM.tile([P, 1024], bf16, tag="r2a")
                    nc.scalar.activation(out=r2a[:, 0:w], in_=p2a[:, 0:w], func=AF.Prelu,
                                         alpha=NEG_MLP, bias=b2ca[:])
                    # r2b leaky-relu on DVE (Act is the launch bottleneck)
                    r2b = sbM.tile([68, 1024], bf16, tag="r2b")
                    u2b = sbM.tile([68, 1024], bf16, tag="u2b")
                    t2b = sbM.tile([68, 1024], bf16, tag="t2b")
                    nc.vector.tensor_scalar(out=u2b[:, 0:w], in0=p2b[0:68, 0:w],
                                            scalar1=b2cb[:], scalar2=None, op0=AL.add)
                    nc.vector.tensor_scalar(out=t2b[:, 0:w], in0=u2b[:, 0:w],
                                            scalar1=0.0, scalar2=NEG_MLP - 1.0,
                                            op0=AL.min, op1=AL.mult)
                    nc.vector.tensor_tensor(out=r2b[:, 0:w], in0=u2b[:, 0:w],
                                            in1=t2b[:, 0:w], op=AL.add)
                    for ci_, mc in enumerate(pair):
                        p3 = psC.tile([14, 512], f32, tag="p3")
                        nc.tensor.matmul(out=p3[:], lhsT=w3a[:],
                                         rhs=r2a[:, ci_ * 512:(ci_ + 1) * 512], start=True, stop=False)
                        nc.tensor.matmul(out=p3[:], lhsT=w3b[:],
                                         rhs=r2b[0:68, ci_ * 512:(ci_ + 1) * 512], start=False, stop=True)
                        ro = 32 * (mc % 4)
                        nc.scalar.activation(out=r3st[ro:ro + 14, (mc // 4) * 512:(mc // 4 + 1) * 512],
                                             in_=p3[:], func=AF.Prelu,
                                             alpha=NEG_MLP, bias=b3c[:])
            # tail: stacked 14->1 matmuls + sigmoids (one act-table switch)
            for g in range(ngrp):
                k = min(4, nmc - 4 * g)
                kp = 32 * (k - 1) + 14
                po = psC.tile([14, 512], f32, tag="p3")
                nc.tensor.matmul(out=po[0:k, 0:512], lhsT=w4b[0:kp, 0:k],
                                 rhs=r3st[0:kp, g * 512:(g + 1) * 512],
                                 start=True, stop=True)
                sg = sbM.tile([4, 512], f32, tag="sg")
                nc.scalar.activation(out=sg[0:k, :], in_=po[0:k, 0:512], func=AF.Sigmoid,
                                     bias=b4c[0:k, :])
                nc.sync.dma_start(out=d_out[4 * g:4 * g + k, :], in_=sg[0:k, :])
    nc.compile()
    return nc


# ================================================================== kernel
_cache = {}


def kernel(**inputs):
    x = np.asarray(inputs['x'], np.float32)
    src = np.asarray(inputs['src'], np.int32)
    dst = np.asarray(inputs['dst'], np.int32)
    n = x.shape[0]

    scheds, nst, Ls = build_schedule(dst, n, 8)          # B geometry
    schedsC, nstC, LsC = build_schedule(dst, n, 16)      # C geometry
    nt = scheds[0]['nt']
    nloc = scheds[0]['nloc']
    nmc = -(-nloc // 512)
    offs12 = np.concatenate([[0], np.cumsum([8 * L * 12 for L in Ls])]).astype(np.int64)
    offs4 = np.concatenate([[0], np.cumsum([16 * L * 4 for L in LsC])]).astype(np.int64)

    # ---- layer-1 attn folding: channels [d1h0(5), d1h1(5), a1h0, a1h1]
    d1_attn = np.asarray(inputs['d1_attn'], np.float64)     # [2, 5]
    a1_attn = np.asarray(inputs['a1_attn'], np.float64)     # [2, 1]
    perm_d1, scale_d1, alpha_d1 = attn_fold(d1_attn, 2, 5)
    perm_a1, scale_a1, alpha_a1 = attn_fold(a1_attn[:, :], 2, 1)
    scale12 = np.concatenate([scale_d1, scale_a1])
    alpha12 = alpha_d1 + alpha_a1
    runs12 = alpha_runs(alpha12)

    def l1_pack(a1_w, d1_w, scale=None):
        w = np.zeros((a1_w.shape[0], 12), np.float64)
        for p_, j in enumerate(perm_d1):
            w[:, p_] = d1_w[:, j]
        for p_, j in enumerate(perm_a1):
            w[:, 10 + p_] = a1_w[:, j]
        if scale is not None:
            w = w * scale[None, :]
        return w

    a1_Wsrc = np.asarray(inputs['a1_Wsrc'], np.float64)
    d1_Wsrc = np.asarray(inputs['d1_Wsrc'], np.float64)
    a1_Wdst = np.asarray(inputs['a1_Wdst'], np.float64)
    d1_Wdst = np.asarray(inputs['d1_Wdst'], np.float64)
    a1_Wres = np.asarray(inputs['a1_Wres'], np.float64)
    d1_Wres = np.asarray(inputs['d1_Wres'], np.float64)
    bY = l1_pack(np.asarray(inputs['a1_bsrc'], np.float64)[None, :],
                 np.asarray(inputs['d1_bsrc'], np.float64)[None, :], scale12)[0]
    bD = l1_pack(np.asarray(inputs['a1_bdst'], np.float64)[None, :],
                 np.asarray(inputs['d1_bdst'], np.float64)[None, :], scale12)[0]
    bR = l1_pack(np.asarray(inputs['a1_bias'], np.float64)[None, :],
                 np.asarray(inputs['d1_bias'], np.float64)[None, :])[0]
    bd_fs = blockdiag(l1_pack(a1_Wsrc, d1_Wsrc, scale12).astype(np.float32), bY.astype(np.float32), 6)
    bd_fd = blockdiag(l1_pack(a1_Wdst, d1_Wdst, scale12).astype(np.float32), bD.astype(np.float32), 6)
    bd_rs = blockdiag(l1_pack(a1_Wres, d1_Wres).astype(np.float32), bR.astype(np.float32), 6)
    isc12 = np.tile((1.0 / scale12).astype(np.float32), (P, 1))

    # ---- layer-2 attn folding: channels [d2h0(2), d2h1(2)]
    d2_attn = np.asarray(inputs['d2_attn'], np.float64)     # [2, 2]
    perm_d2, scale4, alpha4 = attn_fold(d2_attn, 2, 2)
    runs4 = alpha_runs(alpha4)

    def d2w(name):
        w = np.asarray(inputs[name], np.float64)            # [10, 4] native cols j=2h+f
        out = np.zeros((10, 4), np.float64)
        for p_, j in enumerate(perm_d2):
            out[:, p_] = w[:, j]
        return out

    def d2b(name):
        b = np.asarray(inputs[name], np.float64)
        return b[perm_d2]

    # rows of the [10, 12] projection are h_def1 in MY permuted order
    rowperm = perm_d1                                       # position -> native j=5h+f
    ws2 = d2w('d2_Wsrc')[rowperm] * scale4[None, :]
    wd2 = d2w('d2_Wdst')[rowperm] * scale4[None, :]
    wr2 = d2w('d2_Wres')[rowperm]
    w2all = np.concatenate([ws2, wd2, wr2], axis=1).astype(np.float32)
    bc2 = np.concatenate([d2b('d2_bsrc') * scale4, d2b('d2_bdst') * scale4,
                          d2b('d2_bias')]).astype(np.float32)
    isc4 = np.tile((1.0 / scale4).astype(np.float32), (P, 1))

    # ---- MLP weights: W1 rows 2:6 permuted to h_def2 order
    w1p = np.asarray(inputs['W1'], np.float64).copy()
    W1n = np.asarray(inputs['W1'], np.float64)
    for p_, j in enumerate(perm_d2):
        w1p[2 + p_] = W1n[2 + j]
    w1x = np.zeros((32, 196), np.float32)
    w1x[0:14] = w1p.astype(np.float32)
    W2 = np.asarray(inputs['W2'], np.float32)
    w2dra = np.zeros((P, 2, 128), np.float32)
    w2dra[:, 0, :] = W2[0:128, 0:128]
    w2dra[0:68, 1, :] = W2[128:196, 0:128]
    w2drb = np.zeros((P, 2, 128), np.float32)
    w2drb[:, 0, 0:68] = W2[0:128, 128:196]
    w2drb[0:68, 1, 0:68] = W2[128:196, 128:196]
    FPW = FP8 if FP8_W2 else BF
    w4 = np.asarray(inputs['W4'], np.float32)               # [14, 1]
    w4blk = np.zeros((110, 4), np.float32)
    for k in range(4):
        w4blk[32 * k:32 * k + 14, k] = w4[:, 0]
    b4r = np.full(4, float(np.asarray(inputs['b4'])[0]), np.float32)

    key = (n, len(src), nst, tuple(Ls), tuple(LsC), tuple(runs12), tuple(runs4))
    if key not in _cache:
        _cache.clear()
        _cache[key] = (build_launchA(nt), build_launchB(nst, Ls, offs12, nt, runs12),
                       build_launchC(nstC, LsC, offs4, nt, nmc, runs4))
    ncA, ncB, ncC = _cache[key]

    # ---------------- launch A: per-node projections of x
    inA = []
    for s in scheds:
        orig = s['order']
        valid = orig < nloc
        xl = np.zeros((nt * P, 5), np.float32)
        xl[valid] = x[s['core'] * nloc + orig[valid], :5]
        inA.append(dict(x5l=pack_local(xl, 6, nt), bd_fs=bd_fs, bd_fd=bd_fd, bd_rs=bd_rs))
    rA = run_bass_kernel_spmd(ncA, inA, core_ids=list(range(NCORE)))
    tA = rA.exec_time_ns or 0

    i_all = np.arange(nt * P)
    a_i = (i_all // P) % 8
    col_i = (i_all // (8 * P)) * P + i_all % P
    rows12 = a_i[:, None] * 16 + np.arange(12)[None, :]
    fs1g = np.zeros((n, 12), BF)
    geoms, fdp1_l, rs1n_l, npad_l = [], [], [], []
    geomsC, npadC_l = [], []
    for ci, s in enumerate(scheds):
        fs_sorted = rA.results[ci]['fs1cm'][rows12, col_i[:, None]]
        fd_sorted = rA.results[ci]['fd1cm'][rows12, col_i[:, None]]
        rs_sorted = rA.results[ci]['rs1cm'][rows12, col_i[:, None]]
        orig = s['order']
        valid = orig < nloc
        fs1g[s['core'] * nloc + orig[valid]] = fs_sorted[valid]
        fdp1_l.append(pm_pair(fd_sorted, nt))
        rs1n_l.append(pm(rs_sorted.astype(np.float32), nt))
        geoms.append(edge_slot_geom(s, Ls))
        npad_l.append(make_npad(s, Ls, nt))
        geomsC.append(edge_slot_geom(schedsC[ci], LsC))
        npadC_l.append(make_npad(schedsC[ci], LsC, nt))

    inB = []
    for ci, s in enumerate(scheds):
        eo, st_of, s_of, rank, p_of = geoms[ci]
        v = fs1g[src[s['em']][eo]]
        g1 = pack_G(v, st_of, s_of, rank, p_of, offs12, 12, Ls, int(offs12[-1]))
        inB.append(dict(g1=g1, fdp1=fdp1_l[ci], rs1n=rs1n_l[ci], isc12=isc12,
                        npad=npad_l[ci], w2all=w2all.astype(BF), bc2=bc2))
    rB = run_bass_kernel_spmd(ncB, inB, core_ids=list(range(NCORE)))
    tB = rB.exec_time_ns or 0

    SB = 8
    fgw = -(-nst // 4) * SB * P
    fs2g = np.zeros((n, 4), BF)
    fdp2_l, rs2n_l, ha_l, xpm_l = [], [], [], []
    for ci, s in enumerate(scheds):
        fb = rB.results[ci]['f2cm']              # [48, fgw]
        f2 = np.zeros((12, nt * P), np.float32)
        for st in range(nst):
            f2[:, st * SB * P:(st + 1) * SB * P] = \
                fb[12 * (st % 4):12 * (st % 4) + 12,
                   (st // 4) * SB * P:(st // 4 + 1) * SB * P]
        orig = s['order']
        valid = orig < nloc
        fs2g[s['core'] * nloc + orig[valid]] = f2[0:4, :].T[valid].astype(BF)
        fdp2_l.append(pm_pair(f2[4:8, :].T.astype(BF), nt))
        rs2n_l.append(pm(np.ascontiguousarray(f2[8:12, :].T), nt))
        ha_l.append(rB.results[ci]['hattn'])
        xl8 = np.zeros((nt * P, 8), np.float32)
        xl8[valid] = x[s['core'] * nloc + orig[valid], :]
        xpm_l.append(pm(xl8, nt).astype(BF))

    inC = []
    for ci, s in enumerate(scheds):
        eo, st_of, s_of, rank, p_of = geomsC[ci]
        v = fs2g[src[s['em']][eo]]
        g2 = pack_G(v, st_of, s_of, rank, p_of, offs4, 4, LsC, int(offs4[-1]))
        inC.append(dict(g2=g2, fdp2=fdp2_l[ci], rs2n=rs2n_l[ci], isc4=isc4,
                        npad=npadC_l[ci], hattn=ha_l[ci], xpm=xpm_l[ci],
                        w1x=w1x.astype(BF),
                        w2dra=w2dra.reshape(P, 256).astype(FPW),
                        w2drb=w2drb.reshape(P, 256).astype(FPW),
                        w3a=np.asarray(inputs['W3'], np.float32)[0:128].astype(BF),
                        w3b=np.asarray(inputs['W3'], np.float32)[128:196].astype(BF),
                        w4blk=w4blk.astype(BF),
                        b1=np.asarray(inputs['b1'], np.float32),
                        b2=np.asarray(inputs['b2'], np.float32),
                        b3=np.asarray(inputs['b3'], np.float32),
                        b4r=b4r))
    rC = run_bass_kernel_spmd(ncC, inC, core_ids=list(range(NCORE)))
    tC = rC.exec_time_ns or 0

    out = np.zeros((n, 1), np.float32)
    for ci, s in enumerate(scheds):
        y = rC.results[ci]['out'].reshape(nmc * 512)
        orig = s['order']
        valid = orig < nloc
        idx = np.arange(nt * P)[valid]
        out[s['core'] * nloc + orig[valid], 0] = y[idx]
    kernel.last_exec_ns = tA + tB + tC
    kernel.last_t12 = (tA, tB, tC)
    kernel.last_results = (rA, rB, rC)
    return out
